# revision 1
# baseline (speedup 1.0000x reference)
import numpy as np

import concourse.bacc as bacc
import concourse.tile as tile
import concourse.mybir as mybir
from concourse.bass_utils import run_bass_kernel_spmd

F32 = mybir.dt.float32
F32R = mybir.dt.float32r

B = 4
N = 2048
PD = 512
CD = 128
ID = 512
OUT_D = 512
H_PER_CORE = 4
DH = 64
SCALE = 0.125          # dim_head ** -0.5
NT = 4                 # n chunks of 512
NP = 4                 # pd chunks of 128
NKT = 16               # key tiles of 128
VW = 65 * H_PER_CORE   # vtile columns per key tile: 4 x (64 V | 1 one)

_NC = None
LAST_EXEC_NS = None


def _build():
    nc = bacc.Bacc("TRN2", target_bir_lowering=False, debug=False, num_devices=8)
    XT = nc.declare_dram_parameter("XT", [PD, N], F32, isOutput=False)
    CT = nc.declare_dram_parameter("CT", [CD, N], F32, isOutput=False)
    Wqp = nc.declare_dram_parameter("Wqp", [128, 1024], F32, isOutput=False)
    Wkp = nc.declare_dram_parameter("Wkp", [128, 1024], F32, isOutput=False)
    Wv = nc.declare_dram_parameter("Wv", [128, 1024], F32, isOutput=False)
    Wqc = nc.declare_dram_parameter("Wqc", [CD, 256], F32, isOutput=False)
    Wkc = nc.declare_dram_parameter("Wkc", [CD, 256], F32, isOutput=False)
    Wo = nc.declare_dram_parameter("Wo", [128, 1024], F32, isOutput=False)
    Y = nc.declare_dram_parameter("Y", [N, OUT_D], F32, isOutput=True)

    MIN = mybir.AluOpType.min
    MAX = mybir.AluOpType.max
    MULT = mybir.AluOpType.mult
    ADD = mybir.AluOpType.add
    EXP = mybir.ActivationFunctionType.Exp

    with tile.TileContext(nc) as tc, \
         tc.tile_pool(name="persist", bufs=1) as pp, \
         tc.tile_pool(name="work", bufs=2) as wk, \
         tc.tile_pool(name="pb_s", bufs=2, space="PSUM") as psp, \
         tc.tile_pool(name="pb_o", bufs=2, space="PSUM") as pop, \
         tc.tile_pool(name="pc_y", bufs=2, space="PSUM") as pyp:
        xt = [pp.tile([128, N], F32R, name=f"xt{p}", tag=f"xt{p}") for p in range(NP)]
        ct = pp.tile([128, N], F32R, name="ct", tag="ct")
        wqp = pp.tile([128, 1024], F32R, name="wqp", tag="wqp")
        wkp = pp.tile([128, 1024], F32R, name="wkp", tag="wkp")
        wv = pp.tile([128, 1024], F32R, name="wv", tag="wv")
        wqc = pp.tile([128, 256], F32R, name="wqc", tag="wqc")
        wkc = pp.tile([128, 256], F32R, name="wkc", tag="wkc")
        wo = pp.tile([128, 1024], F32R, name="wo", tag="wo")
        qcat = [pp.tile([128, N], F32R, name=f"qcat{h}", tag=f"qcat{h}")
                for h in range(H_PER_CORE)]
        kcat = [pp.tile([128, N], F32R, name=f"kcat{h}", tag=f"kcat{h}")
                for h in range(H_PER_CORE)]
        vtile = pp.tile([128, NKT * VW], F32R, name="vtile", tag="vtile")
        ocat = [pp.tile([128, N], F32R, name=f"ocat{j}", tag=f"ocat{j}")
                for j in range(2)]

        # Input DMAs split across two queues (SP + Pool doorbell) so the
        # critical prologue tensors land sooner.
        nc.sync.dma_start(wkp[:], Wkp[:].bitcast(F32R))
        nc.gpsimd.dma_start(wkc[:], Wkc[:].bitcast(F32R))
        nc.sync.dma_start(wqp[:], Wqp[:].bitcast(F32R))
        nc.gpsimd.dma_start(wqc[:], Wqc[:].bitcast(F32R))
        nc.sync.dma_start(xt[0][:], XT[0:128, :].bitcast(F32R))
        nc.gpsimd.dma_start(xt[2][:], XT[256:384, :].bitcast(F32R))
        nc.sync.dma_start(xt[1][:], XT[128:256, :].bitcast(F32R))
        nc.gpsimd.dma_start(xt[3][:], XT[384:512, :].bitcast(F32R))
        nc.sync.dma_start(ct[:], CT[:].bitcast(F32R))
        nc.gpsimd.dma_start(wv[:], Wv[:].bitcast(F32R))
        nc.sync.dma_start(wo[:], Wo[:].bitcast(F32R))

        # Everything (QKV projections, attention, output projection) runs in
        # one flat software pipeline over 128 kt-pairs (4 qi x 4 h x 8 j).
        # Steady state per pair g: ACT(exp) of pair g, AV matmuls of pair g,
        # dots matmuls of pair g+2 (so ACT never starves behind the in-order
        # PE queue). K/V/Q projections, norm chains, and the output
        # projection are spread into PE slack with emission-order deadlines.
        NPAIR = NT * H_PER_CORE * (NKT // 2)
        if True:
            ps_tiles = {}
            po_tiles = {}
            qstate = {}

            def proj_tg(t, g, wpix, wcrd, dest):
                # qk projection for column block t, head-group g (heads
                # 2g, 2g+1): pixel part into rows 0:64, coord into 64:128.
                ps = pyp.tile([128, 512], F32, name="py", tag="py")
                for c in range(NP):
                    nc.tensor.matmul(
                        ps[:], wpix[:, c * 256 + g * 128:c * 256 + (g + 1) * 128],
                        xt[c][:, t * 512:(t + 1) * 512],
                        start=(c == 0), stop=(c == NP - 1))
                for jj in range(2):
                    nc.vector.tensor_scalar(
                        dest[2 * g + jj][0:64, t * 512:(t + 1) * 512],
                        ps[jj * 64:(jj + 1) * 64, :], 5.0, -5.0, op0=MIN, op1=MAX)
                ps2 = pyp.tile([128, 512], F32, name="py", tag="py")
                nc.tensor.matmul(ps2[:, 0:512], wcrd[:, g * 128:(g + 1) * 128],
                                 ct[:, t * 512:(t + 1) * 512], start=True, stop=True)
                for jj in range(2):
                    nc.vector.tensor_scalar(
                        dest[2 * g + jj][64:128, t * 512:(t + 1) * 512],
                        ps2[jj * 64:(jj + 1) * 64, :], 5.0, -5.0, op0=MIN, op1=MAX)

            def v_kt(kt):
                t, i = divmod(kt, 4)
                pv = pyp.tile([128, 512], F32, name="py", tag="py")
                for p in range(NP):
                    nc.tensor.matmul(
                        pv[:, 0:256],
                        xt[p][:, t * 512 + i * 128:t * 512 + (i + 1) * 128],
                        wv[:, p * 256:(p + 1) * 256],
                        start=(p == 0), stop=(p == NP - 1))
                for h in range(H_PER_CORE):
                    nc.vector.tensor_copy(
                        vtile[:, kt * VW + h * 65:kt * VW + h * 65 + 64],
                        pv[:, h * 64:(h + 1) * 64])

            def loc(g):
                qi, r = divmod(g, 32)
                h, j = divmod(r, 8)
                return qi, h, j

            def emit_dots(g):
                qi, h, j = loc(g)
                ps = psp.tile([128, 1024], F32, name="ps", tag="ps")
                ps_tiles[g] = ps
                k0, k1 = 2 * j, 2 * j + 1
                nc.tensor.matmul(
                    ps[:, 0:512], kcat[h][:, k0 * 128:(k0 + 1) * 128],
                    qcat[h][:, qi * 512:(qi + 1) * 512], start=True, stop=True)
                nc.tensor.matmul(
                    ps[:, 512:1024], kcat[h][:, k1 * 128:(k1 + 1) * 128],
                    qcat[h][:, qi * 512:(qi + 1) * 512], start=True, stop=True)

            def emit_av(g, pe):
                qi, h, j = loc(g)
                if j == 0:
                    po_tiles[(qi, h)] = pop.tile([65, 512], F32, name="po", tag="po")
                po = po_tiles[(qi, h)]
                k0, k1 = 2 * j, 2 * j + 1
                nc.tensor.matmul(
                    po[:], vtile[:, k0 * VW + h * 65:k0 * VW + h * 65 + 65],
                    pe[:, 0:512], start=(j == 0), stop=False)
                nc.tensor.matmul(
                    po[:], vtile[:, k1 * VW + h * 65:k1 * VW + h * 65 + 65],
                    pe[:, 512:1024], start=False, stop=(j == NKT // 2 - 1))

            def emit_norm(qi, h):
                po = po_tiles[(qi, h)]
                r = wk.tile([1, 512], F32, name="r", tag="r")
                rb = wk.tile([64, 512], F32, name="rb", tag="rb")
                nc.vector.reciprocal(r[:], po[64:65, :])
                nc.gpsimd.partition_broadcast(rb[:], r[:])
                oj, rr = h // 2, (h % 2) * 64
                nc.vector.tensor_tensor(
                    ocat[oj][rr:rr + 64, qi * 512:(qi + 1) * 512],
                    po[0:64, :], rb[:], op=MULT)

            def emit_phasec(qi, i):
                n0 = qi * 512 + i * 128
                py = pyp.tile([128, 512], F32, name="py", tag="py")
                nc.tensor.matmul(py[:], ocat[0][:, n0:n0 + 128], wo[:, 0:512],
                                 start=True, stop=False)
                nc.tensor.matmul(py[:], ocat[1][:, n0:n0 + 128], wo[:, 512:1024],
                                 start=False, stop=True)
                st = wk.tile([128, 512], F32, name="st", tag="st")
                nc.vector.tensor_copy(st[:], py[:])
                nc.sync.dma_start(Y[n0:n0 + 128, :], st[:])

            def emit_qproj_piece(t, p):
                # p 0..7: pixel-part matmul (group p//4, x chunk p%4);
                # p 8..9: coord-part matmul for group p-8. Clamps follow the
                # final accumulating matmul (they run on DVE, not PE).
                if p < 8:
                    g, c = divmod(p, 4)
                    if c == 0:
                        qstate[(t, g)] = pyp.tile([128, 512], F32,
                                                  name="py", tag="py")
                    ps = qstate[(t, g)]
                    nc.tensor.matmul(
                        ps[:], wqp[:, c * 256 + g * 128:c * 256 + (g + 1) * 128],
                        xt[c][:, t * 512:(t + 1) * 512],
                        start=(c == 0), stop=(c == 3))
                    if c == 3:
                        for jj in range(2):
                            nc.vector.tensor_scalar(
                                qcat[2 * g + jj][0:64, t * 512:(t + 1) * 512],
                                ps[jj * 64:(jj + 1) * 64, :], 5.0, -5.0,
                                op0=MIN, op1=MAX)
                else:
                    g = p - 8
                    ps = pyp.tile([128, 512], F32, name="py", tag="py")
                    nc.tensor.matmul(ps[:], wqc[:, g * 128:(g + 1) * 128],
                                     ct[:, t * 512:(t + 1) * 512],
                                     start=True, stop=True)
                    for jj in range(2):
                        nc.vector.tensor_scalar(
                            qcat[2 * g + jj][64:128, t * 512:(t + 1) * 512],
                            ps[jj * 64:(jj + 1) * 64, :], 5.0, -5.0,
                            op0=MIN, op1=MAX)

            def ones_cols(c0, c1, w):
                # vtile ones-columns: in*0 + 1, seeded from xt[0]
                nc.vector.tensor_scalar(vtile[:, c0:c1], xt[0][:, 0:w], 0.0, 1.0,
                                        op0=MULT, op1=ADD)

            # Prologue: minimal K/Q prefix so exp can start ASAP, then the
            # V tiles and K columns needed by the first few pipeline pairs.
            proj_tg(0, 0, wkp, wkc, kcat)
            proj_tg(0, 0, wqp, wqc, qcat)
            emit_dots(0)
            emit_dots(1)
            ones_cols(0, 2048, 2048)
            for kt in range(4):
                v_kt(kt)
            proj_tg(1, 0, wkp, wkc, kcat)
            ones_cols(2048, 4096, 2048)
            ones_cols(4096, NKT * VW, 64)
            for kt in range(4, 8):
                v_kt(kt)

            # Filler pieces for the qi=0 block, keyed by pipeline position;
            # each fits the ~1us PE slack without starving ACT, and lands
            # before its first consumer's emission point.
            fill0 = {
                0: [lambda: proj_tg(2, 0, wkp, wkc, kcat), lambda: v_kt(8)],
                1: [lambda: v_kt(9), lambda: v_kt(10)],
                2: [lambda: proj_tg(3, 0, wkp, wkc, kcat), lambda: v_kt(11)],
                3: [lambda: v_kt(12), lambda: v_kt(13)],
                4: [lambda: v_kt(14), lambda: v_kt(15)],
                5: [lambda: proj_tg(0, 1, wkp, wkc, kcat)],
                6: [lambda: proj_tg(1, 1, wkp, wkc, kcat)],
                7: [lambda: proj_tg(2, 1, wkp, wkc, kcat)],
                8: [lambda: proj_tg(3, 1, wkp, wkc, kcat)],
                9: [lambda: proj_tg(0, 1, wqp, wqc, qcat)],
            }

            for g in range(NPAIR):
                qi, h, j = loc(g)
                pe = wk.tile([128, 1024], F32R, name="pe", tag="pe", bufs=3)
                nc.scalar.activation(pe[:], ps_tiles[g][:], EXP, scale=SCALE)
                emit_av(g, pe)
                if g + 2 < NPAIR:
                    emit_dots(g + 2)
                r32 = g % 32
                if qi == 0 and r32 in fill0:
                    for f in fill0[r32]:
                        f()
                if qi >= 1 and r32 in (2, 6, 10, 14):
                    emit_phasec(qi - 1, (r32 - 2) // 4)
                if qi + 1 < NT and 16 <= r32 <= 25:
                    emit_qproj_piece(qi + 1, r32 - 16)
                if j == NKT // 2 - 1:
                    emit_norm(qi, h)
            for i in range(4):
                emit_phasec(NT - 1, i)
    nc.compile()
    return nc


def _get_nc():
    global _NC
    if _NC is None:
        _NC = _build()
    return _NC


def _pack(w, nblk, blk):
    w = np.asarray(w, dtype=np.float32)
    return np.ascontiguousarray(
        w.reshape(nblk, 128, blk).transpose(1, 0, 2).reshape(128, nblk * blk))


def kernel(pixels, coords, mask, W_qkv, W_qkc, W_out, b_out):
    global LAST_EXEC_NS
    pixels = np.asarray(pixels, dtype=np.float32)
    coords = np.asarray(coords, dtype=np.float32)
    W_qkv = np.asarray(W_qkv, dtype=np.float32)
    W_qkc = np.asarray(W_qkc, dtype=np.float32)
    W_out = np.asarray(W_out, dtype=np.float32)
    b_out = np.asarray(b_out, dtype=np.float32)

    nc = _get_nc()

    XT = [np.ascontiguousarray(pixels[b].T) for b in range(B)]
    CT = [np.ascontiguousarray(coords[b].T) for b in range(B)]

    in_maps = []
    for c in range(8):
        b = c // 2
        h0 = (c % 2) * H_PER_CORE * DH     # 0 or 256: col offset within split
        in_maps.append({
            "XT": XT[b],
            "CT": CT[b],
            "Wqp": _pack(W_qkv[:, h0:h0 + 256], 4, 256),
            "Wkp": _pack(W_qkv[:, ID + h0:ID + h0 + 256], 4, 256),
            "Wv": _pack(W_qkv[:, 2 * ID + h0:2 * ID + h0 + 256], 4, 256),
            "Wqc": np.ascontiguousarray(W_qkc[:, h0:h0 + 256]),
            "Wkc": np.ascontiguousarray(W_qkc[:, ID + h0:ID + h0 + 256]),
            "Wo": _pack(W_out[h0:h0 + 256, :], 2, 512),
        })

    res = run_bass_kernel_spmd(nc, in_maps, core_ids=list(range(8)))
    LAST_EXEC_NS = getattr(res, "exec_time_ns", None)

    out = np.empty((B, N, OUT_D), np.float32)
    for b in range(B):
        out[b] = res.results[2 * b]["Y"] + res.results[2 * b + 1]["Y"]
    out += b_out[None, None, :]
    return tuple(np.split(out, [1024], axis=1))



# revision 50
# speedup vs baseline: 1.1956x; 1.1956x over previous
import numpy as np
import ml_dtypes

import concourse.bacc as bacc
import concourse.tile as tile
import concourse.mybir as mybir
from concourse.bass_utils import run_bass_kernel_spmd

F32 = mybir.dt.float32
F32R = mybir.dt.float32r
BF16 = mybir.dt.bfloat16
F8E4 = mybir.dt.float8e4
F8E5 = mybir.dt.float8e5

B = 4
N = 2048
PD = 512
CD = 128
ID = 512
OUT_D = 512
H_PER_CORE = 4
DH = 64
SCALE = 0.125          # dim_head ** -0.5
NT = 4                 # n chunks of 512
NP = 4                 # pd chunks of 128
NKT = 16               # key tiles of 128
EXP_BIAS = -2.0        # softmax logits shifted so exp() fits fp8e5 range

# column offsets inside the packed weight block (bf16), head-group-major:
# [kp_g0 512][kc_g0 128][qp_g0 512][qc_g0 128][v 1024]
# [kp_g1 512][kc_g1 128][qp_g1 512][qc_g1 128]
KP = {0: 0, 1: 2304}
KC = {0: 512, 1: 2816}
QP = {0: 640, 1: 2944}
QC = {0: 1152, 1: 3456}
WV = 1280
WALL_COLS = 3584

_NC = None
LAST_EXEC_NS = None


def _build():
    nc = bacc.Bacc("TRN2", target_bir_lowering=False, debug=False, num_devices=8)
    XTg = nc.declare_dram_parameter("XTg", [128, 5, N], BF16, isOutput=False)
    Wall = nc.declare_dram_parameter("Wall", [128, WALL_COLS], BF16, isOutput=False)
    Wo = nc.declare_dram_parameter("Wo", [128, 1024], F32, isOutput=False)
    Ident = nc.declare_dram_parameter("Ident", [128, 128], F32, isOutput=False)
    Y = nc.declare_dram_parameter("Y", [N, OUT_D], F32, isOutput=True)

    MIN = mybir.AluOpType.min
    MAX = mybir.AluOpType.max
    MULT = mybir.AluOpType.mult
    EXP = mybir.ActivationFunctionType.Exp
    DR = mybir.MatmulPerfMode.DoubleRow

    with tile.TileContext(nc) as tc, \
         tc.tile_pool(name="persist", bufs=1) as pp, \
         tc.tile_pool(name="work", bufs=2) as wk, \
         tc.tile_pool(name="pb_s", bufs=2, space="PSUM") as psp, \
         tc.tile_pool(name="pb_o", bufs=2, space="PSUM") as pop, \
         tc.tile_pool(name="pc_y", bufs=2, space="PSUM") as pyp:
        xtall = pp.tile([128, 5, N], BF16, name="xtall", tag="xtall")
        wall = pp.tile([128, WALL_COLS], BF16, name="wall", tag="wall")
        wo = pp.tile([128, 1024], F32R, name="wo", tag="wo")
        ident = pp.tile([128, 128], F32R, name="ident", tag="ident")
        qcat = [pp.tile([128, N], F32R, name=f"qcat{h}", tag=f"qcat{h}")
                for h in range(H_PER_CORE)]
        kcat = [pp.tile([128, N], F32R, name=f"kcat{h}", tag=f"kcat{h}")
                for h in range(H_PER_CORE)]
        # per key tile: 4 heads x (64 V columns | 1 ones column for the
        # softmax denominator), bf16
        vtall = pp.tile([128, NKT, 4, 65], BF16, name="vtall", tag="vtall")
        ebias = pp.tile([128, 1], F32, name="ebias", tag="ebias")
        ocat = [pp.tile([128, N], F32R, name=f"ocat{j}", tag=f"ocat{j}")
                for j in range(2)]

        # Input DMAs. Queue A (sync) carries the critical stream in priority
        # order; queue B (gpsimd) slips the later weights into bus gaps.
        # Big packed transfers amortize the ~650ns per-DMA issue cost.
        def dma_xt_t(t, a, b):
            nc.sync.dma_start(xtall[:, a:b, t * 512:(t + 1) * 512],
                              XTg[:, a:b, t * 512:(t + 1) * 512])

        nc.sync.dma_start(wall[:, 0:1280], Wall[:, 0:1280])        # g0 weights
        dma_xt_t(0, 0, 2)
        dma_xt_t(0, 4, 5)
        dma_xt_t(0, 2, 4)
        nc.sync.dma_start(wall[:, WV:WV + 512], Wall[:, WV:WV + 512])
        dma_xt_t(1, 0, 5)
        nc.sync.dma_start(wall[:, WV + 512:WV + 1024],
                          Wall[:, WV + 512:WV + 1024])
        dma_xt_t(2, 0, 5)
        dma_xt_t(3, 0, 5)
        nc.sync.dma_start(ident[:], Ident[:].bitcast(F32R))
        nc.sync.dma_start(wall[:, 2304:3584], Wall[:, 2304:3584])  # g1
        nc.sync.dma_start(wo[:], Wo[:].bitcast(F32R))

        NPAIR = NT * H_PER_CORE * (NKT // 2)
        ps_tiles = {}
        po_tiles = {}
        qstate = {}
        vstate = {}
        ph_state = {}
        trans_pending = {}

        def emit_clamps(ps, dest, h0, row0, t, direct):
            # GPSIMD cannot read PSUM: either clamp directly on DVE
            # (prologue: shortest chain to the first dots) or stage through
            # SBUF once on DVE and let Pool do both clamps (steady state:
            # halves the DVE load).
            if direct:
                for jj in (0, 1):
                    nc.vector.tensor_scalar(
                        dest[h0 + jj][row0:row0 + 64, t * 512:(t + 1) * 512],
                        ps[jj * 64:(jj + 1) * 64, :], 5.0, -5.0,
                        op0=MIN, op1=MAX)
            else:
                scr = wk.tile([128, 512], F32, name="scr", tag="scr", bufs=3)
                nc.vector.tensor_copy(scr[:], ps[:])
                for jj in (0, 1):
                    nc.gpsimd.tensor_scalar(
                        dest[h0 + jj][row0:row0 + 64, t * 512:(t + 1) * 512],
                        scr[jj * 64:(jj + 1) * 64, :], 5.0, -5.0,
                        op0=MIN, op1=MAX)

        def kq_piece(t, g, pbase, cbase, dest, p, ps_ap=None, ps2_ap=None,
                     direct=False):
            # p 4: coord matmul + clamps (emitted first: its clamp chain is
            # independent of the pixel pieces). p 0..3: pixel-chunk matmuls
            # accumulating over pd; p 3 also clamps. One clamp goes to DVE,
            # the other to Pool, halving the chain latency.
            key = (t, g, id(dest))
            if p < 4:
                if p == 0:
                    qstate[key] = (ps_ap if ps_ap is not None else
                                   pyp.tile([128, 512], F32, name="py", tag="py"))
                ps = qstate[key]
                nc.tensor.matmul(
                    ps[:], wall[:, pbase + p * 128:pbase + (p + 1) * 128],
                    xtall[:, p, t * 512:(t + 1) * 512],
                    start=(p == 0), stop=(p == 3))
                if p == 3:
                    emit_clamps(ps, dest, 2 * g, 0, t, direct)
                    del qstate[key]
            else:
                ps2 = (ps2_ap if ps2_ap is not None else
                       pyp.tile([128, 512], F32, name="py", tag="py"))
                nc.tensor.matmul(ps2[:], wall[:, cbase:cbase + 128],
                                 xtall[:, 4, t * 512:(t + 1) * 512],
                                 start=True, stop=True)
                emit_clamps(ps2, dest, 2 * g, 64, t, direct)

        def kq_all(t, g, kind):
            for p in (4, 0, 1, 2, 3):
                kq_piece_k(t, g, kind, p)

        def kq_piece_k(t, g, kind, p):
            if kind == 'k':
                kq_piece(t, g, KP[g], KC[g], kcat, p)
            else:
                kq_piece(t, g, QP[g], QC[g], qcat, p)

        def v_piece(kt, half):
            # half 0: pd chunks 0,1; half 1: pd chunks 2,3 + one fp8 copy
            # moving all four heads' V columns out in a single instruction.
            t, i = divmod(kt, 4)
            if half == 0:
                vstate[kt] = pyp.tile([128, 512], F32, name="py", tag="py")
            pv = vstate[kt]
            for p in (0, 1) if half == 0 else (2, 3):
                nc.tensor.matmul(
                    pv[:, 0:256],
                    xtall[:, p, t * 512 + i * 128:t * 512 + (i + 1) * 128],
                    wall[:, WV + p * 256:WV + (p + 1) * 256],
                    start=(p == 0), stop=(p == NP - 1))
            if half == 1:
                nc.vector.tensor_copy(vtall[:, kt, :, 0:64], pv[:, 0:256])
                del vstate[kt]

        def loc(g):
            qi, r = divmod(g, 32)
            h, j = divmod(r, 8)
            return qi, h, j

        def emit_dots(g):
            qi, h, j = loc(g)
            ps = psp.tile([128, 2, 512], F32, name="ps", tag="ps")
            ps_tiles[g] = ps
            k0, k1 = 2 * j, 2 * j + 1
            nc.tensor.matmul(
                ps[:, 0, :], kcat[h][:, k0 * 128:(k0 + 1) * 128],
                qcat[h][:, qi * 512:(qi + 1) * 512], start=True, stop=True)
            nc.tensor.matmul(
                ps[:, 1, :], kcat[h][:, k1 * 128:(k1 + 1) * 128],
                qcat[h][:, qi * 512:(qi + 1) * 512], start=True, stop=True)

        def emit_av(g, pe):
            # fp8 DoubleRow: one matmul covers both key tiles of the pair
            # (K=256) at 0.5 cyc/row; a second 1-row matmul accumulates the
            # softmax denominator into po[64].
            # Transposed AV in bf16: the exp scores [keys, 128q] are the
            # full-width stationary, V plus a ones column (denominator) is
            # the 65-wide moving operand. Output [q, 64v|den] accumulates
            # over all 16 key tiles. Four query subtiles share one PSUM
            # bank: since PSUM zero-on-start is bank-granular, the bank is
            # pre-zeroed by a DVE memset (two blocks ahead) and every
            # matmul is a pure accumulate.
            qi, h, j = loc(g)
            po = po_tiles[(qi, h)]
            for i in (0, 1):
                for q in range(4):
                    nc.tensor.matmul(
                        po[:, q, :], pe[:, i, q * 128:(q + 1) * 128],
                        vtall[:, 2 * j + i, h, :],
                        start=False,
                        stop=(j == NKT // 2 - 1 and i == 1),
                        skip_group_check=True)

        def emit_norm(qi, h):
            # scale the V columns by 1/den per query row; the transposes
            # back to [d, n] layout are deferred two iterations so the PE
            # queue never blocks on this DVE chain.
            po = po_tiles.pop((qi, h))
            rb = wk.tile([128, 4], F32, name="rb", tag="rb")
            oc_t = wk.tile([128, 4, 64], F32R, name="oct", tag="oct")
            nc.vector.reciprocal(rb[:], po[:, :, 64])
            for q in range(4):
                nc.vector.tensor_scalar(
                    oc_t[:, q, :], po[:, q, 0:64], rb[:, q:q + 1], None,
                    op0=MULT)

            def flush(qi=qi, h=h, oc_t=oc_t):
                pt = pyp.tile([128, 512], F32, name="py", tag="py")
                for q in range(4):
                    nc.tensor.transpose(
                        pt[0:64, q * 128:(q + 1) * 128].bitcast(F32R),
                        oc_t[:, q, :], ident[:])
                oj, rr = h // 2, (h % 2) * 64
                nc.vector.tensor_copy(
                    ocat[oj][rr:rr + 64, qi * 512:(qi + 1) * 512],
                    pt[0:64, :].bitcast(F32R))
                m = 4 * qi + h + 2
                if m < NT * H_PER_CORE:
                    alloc_po(m)
            trans_pending.setdefault(32 * qi + 8 * h + 9, []).append(flush)

        def alloc_po(m):
            # pre-allocate + zero block m's accumulator bank (reuses the
            # buffer its predecessor's norm just released)
            key = (m // 4, m % 4)
            po = pop.tile([128, 4, 65], F32, name="po", tag="po")
            po_tiles[key] = po
            nc.vector.memset(po[:], 0.0)

        def phasec_piece(qi, i, p):
            # out-projection for a 128-row output chunk, two matmul pieces;
            # the staging copy runs on Pool to keep DVE off the critical path.
            n0 = qi * 512 + i * 128
            if p == 0:
                ph_state[(qi, i)] = pyp.tile([128, 512], F32, name="py", tag="py")
                nc.tensor.matmul(ph_state[(qi, i)][:], ocat[0][:, n0:n0 + 128],
                                 wo[:, 0:512], start=True, stop=False)
            else:
                py = ph_state.pop((qi, i))
                nc.tensor.matmul(py[:], ocat[1][:, n0:n0 + 128],
                                 wo[:, 512:1024], start=False, stop=True)
                st = wk.tile([128, 512], F32, name="st", tag="st", bufs=4)
                nc.vector.tensor_copy(st[:], py[:])
                nc.sync.dma_start(Y[n0:n0 + 128, :], st[:])

        # Static filler schedule: iteration -> emitters. PE is in-order, so
        # a block's pieces must all be emitted before the first dots that
        # reads its output (dots(g) is emitted at iter g-2), with ~2 iters
        # of margin for the clamp chain to land.
        sched = {}

        def at(g, *fs):
            sched.setdefault(g, []).extend(fs)

        # qi=0: V tiles (pair 2j,2j+1 before iter j), K for later t-blocks.
        at(0, lambda: v_piece(2, 0), lambda: v_piece(2, 1),
            lambda: v_piece(3, 0), lambda: v_piece(3, 1))
        at(1, lambda: v_piece(4, 0), lambda: v_piece(4, 1),
            lambda: v_piece(5, 0), lambda: v_piece(5, 1),
            lambda: kq_all(2, 0, 'k'))
        at(2, lambda: v_piece(6, 0), lambda: v_piece(6, 1),
            lambda: v_piece(7, 0), lambda: v_piece(7, 1))
        at(3, lambda: v_piece(8, 0), lambda: v_piece(8, 1),
            lambda: v_piece(9, 0), lambda: v_piece(9, 1),
            lambda: kq_all(3, 0, 'k'))
        at(4, lambda: v_piece(10, 0), lambda: v_piece(10, 1),
            lambda: v_piece(11, 0), lambda: v_piece(11, 1))
        at(5, lambda: v_piece(12, 0), lambda: v_piece(12, 1),
            lambda: v_piece(13, 0), lambda: v_piece(13, 1))
        at(6, lambda: v_piece(14, 0), lambda: v_piece(14, 1),
            lambda: v_piece(15, 0), lambda: v_piece(15, 1))
        # head-group-1 K/Q projections, fine-spread with clamp margin
        PLAN1 = {
            7: [(0, 'k', 4), (0, 'k', 0)],
            8: [(0, 'k', 1), (0, 'k', 2)],
            9: [(0, 'k', 3), (0, 'q', 4)],
            10: [(0, 'q', 0), (0, 'q', 1)],
            11: [(0, 'q', 2), (0, 'q', 3)],
            12: [(1, 'k', 4), (1, 'k', 0)],
            13: [(1, 'k', 1), (1, 'k', 2), (1, 'k', 3)],
            14: [(2, 'k', 4), (2, 'k', 0), (2, 'k', 1)],
            15: [(2, 'k', 2), (2, 'k', 3)],
            16: [(3, 'k', 4), (3, 'k', 0)],
            17: [(3, 'k', 1), (3, 'k', 2)],
            18: [(3, 'k', 3)],
        }
        for it, pieces in PLAN1.items():
            for (t, kind, p) in pieces:
                at(it, lambda t=t, kind=kind, p=p: kq_piece_k(t, 1, kind, p))

        # Q projections for the next query block (both head-groups)
        def q_sched(base, t):
            for p, off in ((4, 0), (0, 1), (1, 2), (2, 3), (3, 4)):
                at(base + 21 + off, lambda t=t, p=p: kq_piece_k(t, 0, 'q', p))
                at(base + 25 + off, lambda t=t, p=p: kq_piece_k(t, 1, 'q', p))

        q_sched(0, 1)
        q_sched(32, 2)
        q_sched(64, 3)
        # out-projection for the previous block, after its last norm
        for qi in range(1, NT):
            for i in range(4):
                at(32 * qi + 2 + 2 * i, lambda qi=qi, i=i: phasec_piece(qi - 1, i, 0))
                at(32 * qi + 3 + 2 * i, lambda qi=qi, i=i: phasec_piece(qi - 1, i, 1))

        # Prologue. The dummy exp preloads the ACT table during DMA wait.
        # The Q(0,0) block borrows the first dots PSUM buffer so all four
        # projection blocks run without pool-rotation WAR stalls; pieces are
        # ordered so the ones needing only the first half of the t0 DMA
        # start earliest.
        nc.vector.memset(vtall[:, :, :, 64], 1.0)   # denominator ones cols
        nc.vector.memset(ebias[:], EXP_BIAS)
        alloc_po(0)
        alloc_po(1)
        warm = wk.tile([128, 1], F32, name="warm", tag="warm")
        nc.scalar.activation(warm[:], ebias[:], EXP)
        qtile = psp.tile([128, 2, 512], F32, name="ps", tag="ps")
        # PE warm-up: dummy bf16 matmuls on a zeroed tile keep the PE busy
        # through the input-DMA wait so the p-state ramp (3us of continuous
        # execution) completes before the real projections start.
        pe_w = wk.tile([128, 2, 512], BF16, name="pe", tag="pe", bufs=8)
        nc.gpsimd.memset(pe_w[:], 0.0)
        for _ in range(16):
            nc.tensor.matmul(qtile[:, 0, :], pe_w[:, 0, 0:128], pe_w[:, 1, :],
                             start=True, stop=True)
        qtile2 = psp.tile([128, 2, 512], F32, name="ps", tag="ps")
        for p in (0, 1):
            kq_piece(0, 0, KP[0], KC[0], kcat, p, direct=True)
            kq_piece(0, 0, QP[0], QC[0], qcat, p, ps_ap=qtile[:, 0, :],
                     direct=True)
        kq_piece(0, 0, KP[0], KC[0], kcat, 4, direct=True)
        kq_piece(0, 0, QP[0], QC[0], qcat, 4, ps2_ap=qtile2[:, 0, :],
                 direct=True)
        for p in (2, 3):
            kq_piece(0, 0, KP[0], KC[0], kcat, p, direct=True)
            kq_piece(0, 0, QP[0], QC[0], qcat, p, ps_ap=qtile[:, 0, :],
                     direct=True)
        emit_dots(0)
        emit_dots(1)
        kq_all(1, 0, 'k')
        v_piece(0, 0)
        v_piece(0, 1)
        v_piece(1, 0)
        v_piece(1, 1)

        for g in range(NPAIR):
            qi, h, j = loc(g)
            pe = wk.tile([128, 2, 512], BF16, name="pe", tag="pe", bufs=8)
            nc.scalar.activation(pe[:], ps_tiles.pop(g)[:], EXP, scale=SCALE)
            if g + 2 < NPAIR:
                emit_dots(g + 2)
            emit_av(g, pe)
            if j == NKT // 2 - 1:
                emit_norm(qi, h)
            for f in trans_pending.pop(g, ()):
                f()
            for f in sched.get(g, ()):
                f()
        for fs in [trans_pending.pop(k) for k in sorted(trans_pending)]:
            for f in fs:
                f()
        for i in range(4):
            phasec_piece(NT - 1, i, 0)
            phasec_piece(NT - 1, i, 1)
    nc.compile()
    return nc


def _get_nc():
    global _NC
    if _NC is None:
        _NC = _build()
    return _NC


def _bf16(a):
    return np.ascontiguousarray(np.asarray(a, np.float32).astype(ml_dtypes.bfloat16))


def _pack_gcd(w):
    # [512, 256] -> [128, (g, c, d)]: head-group-major weight layout so the
    # first group's columns are one contiguous DMA.
    return (np.asarray(w, np.float32)
            .reshape(4, 128, 2, 128).transpose(1, 2, 0, 3).reshape(128, 1024))


def _pack_chd(w):
    # [512, 256] -> [128, (c, h, d)]: pd-chunk-major layout for the V weights.
    return (np.asarray(w, np.float32)
            .reshape(4, 128, 256).transpose(1, 0, 2).reshape(128, 1024))


def _pack_wo(w):
    w = np.asarray(w, dtype=np.float32)
    return np.ascontiguousarray(
        w.reshape(2, 128, 512).transpose(1, 0, 2).reshape(128, 1024))


def kernel(pixels, coords, mask, W_qkv, W_qkc, W_out, b_out):
    global LAST_EXEC_NS
    pixels = np.asarray(pixels, dtype=np.float32)
    coords = np.asarray(coords, dtype=np.float32)
    W_qkv = np.asarray(W_qkv, dtype=np.float32)
    W_qkc = np.asarray(W_qkc, dtype=np.float32)
    W_out = np.asarray(W_out, dtype=np.float32)
    b_out = np.asarray(b_out, dtype=np.float32)

    nc = _get_nc()

    XTg = [np.ascontiguousarray(
        np.concatenate([pixels[b].T.reshape(4, 128, N),
                        coords[b].T.reshape(1, 128, N)], axis=0)
        .transpose(1, 0, 2).astype(ml_dtypes.bfloat16)) for b in range(B)]

    in_maps = []
    for c in range(8):
        b = c // 2
        h0 = (c % 2) * H_PER_CORE * DH     # 0 or 256: col offset within split
        kp = _pack_gcd(W_qkv[:, ID + h0:ID + h0 + 256])
        qp = _pack_gcd(W_qkv[:, h0:h0 + 256])
        v = _pack_chd(W_qkv[:, 2 * ID + h0:2 * ID + h0 + 256])
        kc = np.asarray(W_qkc[:, ID + h0:ID + h0 + 256], np.float32)
        qc = np.asarray(W_qkc[:, h0:h0 + 256], np.float32)
        wall = np.concatenate([
            kp[:, 0:512], kc[:, 0:128], qp[:, 0:512], qc[:, 0:128], v,
            kp[:, 512:1024], kc[:, 128:256], qp[:, 512:1024], qc[:, 128:256],
        ], axis=1)
        in_maps.append({
            "XTg": XTg[b],
            "Wall": _bf16(wall),
            "Wo": _pack_wo(W_out[h0:h0 + 256, :]),
            "Ident": np.eye(128, dtype=np.float32),
        })

    res = run_bass_kernel_spmd(nc, in_maps, core_ids=list(range(8)))
    LAST_EXEC_NS = getattr(res, "exec_time_ns", None)

    out = np.empty((B, N, OUT_D), np.float32)
    for b in range(B):
        out[b] = res.results[2 * b]["Y"] + res.results[2 * b + 1]["Y"]
    out += b_out[None, None, :]
    return tuple(np.split(out, [1024], axis=1))


# revision 55
# speedup vs baseline: 1.2235x; 1.0234x over previous
import numpy as np
import ml_dtypes

import concourse.bacc as bacc
import concourse.tile as tile
import concourse.mybir as mybir
from concourse.bass_utils import run_bass_kernel_spmd

F32 = mybir.dt.float32
F32R = mybir.dt.float32r
BF16 = mybir.dt.bfloat16
F8E4 = mybir.dt.float8e4
F8E5 = mybir.dt.float8e5

B = 4
N = 2048
PD = 512
CD = 128
ID = 512
OUT_D = 512
H_PER_CORE = 4
DH = 64
SCALE = 0.125          # dim_head ** -0.5
NT = 4                 # n chunks of 512
NP = 4                 # pd chunks of 128
NKT = 16               # key tiles of 128
EXP_BIAS = -2.0        # softmax logits shifted so exp() fits fp8e5 range

# column offsets inside the packed weight block (bf16), head-group-major:
# [kp_g0 512][kc_g0 128][qp_g0 512][qc_g0 128][v 1024]
# [kp_g1 512][kc_g1 128][qp_g1 512][qc_g1 128]
KP = {0: 0, 1: 2304}
KC = {0: 512, 1: 2816}
QP = {0: 640, 1: 2944}
QC = {0: 1152, 1: 3456}
WV = 1280
WALL_COLS = 3584

_NC = None
LAST_EXEC_NS = None


def _build():
    nc = bacc.Bacc("TRN2", target_bir_lowering=False, debug=False, num_devices=8)
    XTg = nc.declare_dram_parameter("XTg", [128, 5, N], BF16, isOutput=False)
    Wall = nc.declare_dram_parameter("Wall", [128, WALL_COLS], BF16, isOutput=False)
    Wo = nc.declare_dram_parameter("Wo", [128, 1024], F32, isOutput=False)
    Ident = nc.declare_dram_parameter("Ident", [128, 128], F32, isOutput=False)
    Y = nc.declare_dram_parameter("Y", [N, OUT_D], F32, isOutput=True)

    MIN = mybir.AluOpType.min
    MAX = mybir.AluOpType.max
    MULT = mybir.AluOpType.mult
    EXP = mybir.ActivationFunctionType.Exp
    DR = mybir.MatmulPerfMode.DoubleRow

    with tile.TileContext(nc) as tc, \
         tc.tile_pool(name="persist", bufs=1) as pp, \
         tc.tile_pool(name="work", bufs=2) as wk, \
         tc.tile_pool(name="pb_s", bufs=2, space="PSUM") as psp, \
         tc.tile_pool(name="pb_o", bufs=2, space="PSUM") as pop, \
         tc.tile_pool(name="pc_y", bufs=2, space="PSUM") as pyp:
        xtall = pp.tile([128, 5, N], BF16, name="xtall", tag="xtall")
        wall = pp.tile([128, WALL_COLS], BF16, name="wall", tag="wall")
        wo = pp.tile([128, 1024], F32R, name="wo", tag="wo")
        ident = pp.tile([128, 128], F32R, name="ident", tag="ident")
        qcat = [pp.tile([128, N], F32R, name=f"qcat{h}", tag=f"qcat{h}")
                for h in range(H_PER_CORE)]
        kcat = [pp.tile([128, N], F32R, name=f"kcat{h}", tag=f"kcat{h}")
                for h in range(H_PER_CORE)]
        # per key tile: 4 heads x (64 V columns | 1 ones column for the
        # softmax denominator), bf16
        vtall = pp.tile([128, NKT, 4, 65], BF16, name="vtall", tag="vtall")
        ebias = pp.tile([128, 1], F32, name="ebias", tag="ebias")
        ocat = [pp.tile([128, N], F32R, name=f"ocat{j}", tag=f"ocat{j}")
                for j in range(2)]

        # Input DMAs. Queue A (sync) carries the critical stream in priority
        # order; queue B (gpsimd) slips the later weights into bus gaps.
        # Big packed transfers amortize the ~650ns per-DMA issue cost.
        def dma_xt_t(t, a, b):
            nc.sync.dma_start(xtall[:, a:b, t * 512:(t + 1) * 512],
                              XTg[:, a:b, t * 512:(t + 1) * 512])

        nc.sync.dma_start(wall[:, 0:1280], Wall[:, 0:1280])        # g0 weights
        dma_xt_t(0, 0, 2)
        dma_xt_t(0, 4, 5)
        dma_xt_t(0, 2, 4)
        nc.sync.dma_start(wall[:, WV:WV + 512], Wall[:, WV:WV + 512])
        dma_xt_t(1, 0, 5)
        nc.sync.dma_start(wall[:, WV + 512:WV + 1024],
                          Wall[:, WV + 512:WV + 1024])
        dma_xt_t(2, 0, 5)
        dma_xt_t(3, 0, 5)
        nc.sync.dma_start(ident[:], Ident[:].bitcast(F32R))
        nc.sync.dma_start(wall[:, 2304:3584], Wall[:, 2304:3584])  # g1
        nc.sync.dma_start(wo[:], Wo[:].bitcast(F32R))

        NPAIR = NT * H_PER_CORE * (NKT // 2)
        ps_tiles = {}
        po_tiles = {}
        qstate = {}
        vstate = {}
        ph_state = {}
        trans_pending = {}

        deferred_clamps = []

        def emit_clamps(ps, dest, h0, row0, t, direct):
            # GPSIMD cannot read PSUM: either clamp directly on DVE
            # (prologue: shortest chain to the first dots — the odd-head
            # clamps are deferred past the first two dots since those only
            # read head 0) or stage through SBUF once on DVE and let Pool
            # do both clamps (steady state: halves the DVE load).
            if direct:
                def one(jj):
                    nc.vector.tensor_scalar(
                        dest[h0 + jj][row0:row0 + 64, t * 512:(t + 1) * 512],
                        ps[jj * 64:(jj + 1) * 64, :], 5.0, -5.0,
                        op0=MIN, op1=MAX)
                one(0)
                deferred_clamps.append(lambda: one(1))
            else:
                scr = wk.tile([128, 512], F32, name="scr", tag="scr", bufs=3)
                nc.vector.tensor_copy(scr[:], ps[:])
                for jj in (0, 1):
                    nc.gpsimd.tensor_scalar(
                        dest[h0 + jj][row0:row0 + 64, t * 512:(t + 1) * 512],
                        scr[jj * 64:(jj + 1) * 64, :], 5.0, -5.0,
                        op0=MIN, op1=MAX)

        def kq_piece(t, g, pbase, cbase, dest, p, ps_ap=None, ps2_ap=None,
                     direct=False):
            # p 4: coord matmul + clamps (emitted first: its clamp chain is
            # independent of the pixel pieces). p 0..3: pixel-chunk matmuls
            # accumulating over pd; p 3 also clamps. One clamp goes to DVE,
            # the other to Pool, halving the chain latency.
            key = (t, g, id(dest))
            if p < 4:
                if p == 0:
                    qstate[key] = (ps_ap if ps_ap is not None else
                                   pyp.tile([128, 512], F32, name="py", tag="py"))
                ps = qstate[key]
                nc.tensor.matmul(
                    ps[:], wall[:, pbase + p * 128:pbase + (p + 1) * 128],
                    xtall[:, p, t * 512:(t + 1) * 512],
                    start=(p == 0), stop=(p == 3))
                if p == 3:
                    emit_clamps(ps, dest, 2 * g, 0, t, direct)
                    del qstate[key]
            else:
                ps2 = (ps2_ap if ps2_ap is not None else
                       pyp.tile([128, 512], F32, name="py", tag="py"))
                nc.tensor.matmul(ps2[:], wall[:, cbase:cbase + 128],
                                 xtall[:, 4, t * 512:(t + 1) * 512],
                                 start=True, stop=True)
                emit_clamps(ps2, dest, 2 * g, 64, t, direct)

        def kq_all(t, g, kind):
            for p in (4, 0, 1, 2, 3):
                kq_piece_k(t, g, kind, p)

        def kq_piece_k(t, g, kind, p):
            if kind == 'k':
                kq_piece(t, g, KP[g], KC[g], kcat, p)
            else:
                kq_piece(t, g, QP[g], QC[g], qcat, p)

        def v_piece(kt, half):
            # half 0: pd chunks 0,1; half 1: pd chunks 2,3 + one fp8 copy
            # moving all four heads' V columns out in a single instruction.
            t, i = divmod(kt, 4)
            if half == 0:
                vstate[kt] = pyp.tile([128, 512], F32, name="py", tag="py")
            pv = vstate[kt]
            for p in (0, 1) if half == 0 else (2, 3):
                nc.tensor.matmul(
                    pv[:, 0:256],
                    xtall[:, p, t * 512 + i * 128:t * 512 + (i + 1) * 128],
                    wall[:, WV + p * 256:WV + (p + 1) * 256],
                    start=(p == 0), stop=(p == NP - 1))
            if half == 1:
                nc.vector.tensor_copy(vtall[:, kt, :, 0:64], pv[:, 0:256])
                del vstate[kt]

        def loc(g):
            qi, r = divmod(g, 32)
            h, j = divmod(r, 8)
            return qi, h, j

        def emit_dots(g):
            qi, h, j = loc(g)
            ps = psp.tile([128, 2, 512], F32, name="ps", tag="ps")
            ps_tiles[g] = ps
            k0, k1 = 2 * j, 2 * j + 1
            nc.tensor.matmul(
                ps[:, 0, :], kcat[h][:, k0 * 128:(k0 + 1) * 128],
                qcat[h][:, qi * 512:(qi + 1) * 512], start=True, stop=True)
            nc.tensor.matmul(
                ps[:, 1, :], kcat[h][:, k1 * 128:(k1 + 1) * 128],
                qcat[h][:, qi * 512:(qi + 1) * 512], start=True, stop=True)

        def emit_av(g, pe):
            # fp8 DoubleRow: one matmul covers both key tiles of the pair
            # (K=256) at 0.5 cyc/row; a second 1-row matmul accumulates the
            # softmax denominator into po[64].
            # Transposed AV in bf16: the exp scores [keys, 128q] are the
            # full-width stationary, V plus a ones column (denominator) is
            # the 65-wide moving operand. Output [q, 64v|den] accumulates
            # over all 16 key tiles. Four query subtiles share one PSUM
            # bank: since PSUM zero-on-start is bank-granular, the bank is
            # pre-zeroed by a DVE memset (two blocks ahead) and every
            # matmul is a pure accumulate.
            qi, h, j = loc(g)
            po = po_tiles[(qi, h)]
            for i in (0, 1):
                for q in range(4):
                    nc.tensor.matmul(
                        po[:, q, :], pe[:, i, q * 128:(q + 1) * 128],
                        vtall[:, 2 * j + i, h, :],
                        start=False,
                        stop=(j == NKT // 2 - 1 and i == 1),
                        skip_group_check=True)

        def emit_norm(qi, h, g):
            # scale the V columns by 1/den per query row; the transposes
            # back to [d, n] layout are deferred two iterations so the PE
            # queue never blocks on this DVE chain.
            po = po_tiles.pop((qi, h))
            rb = wk.tile([128, 4], F32, name="rb", tag="rb")
            oc_t = wk.tile([128, 4, 64], F32R, name="oct", tag="oct")
            nc.vector.reciprocal(rb[:], po[:, :, 64])
            for q in range(4):
                nc.vector.tensor_scalar(
                    oc_t[:, q, :], po[:, q, 0:64], rb[:, q:q + 1], None,
                    op0=MULT)

            def flush(qi=qi, h=h, oc_t=oc_t):
                pt = pyp.tile([128, 512], F32, name="py", tag="py")
                for q in range(4):
                    nc.tensor.transpose(
                        pt[0:64, q * 128:(q + 1) * 128].bitcast(F32R),
                        oc_t[:, q, :], ident[:])
                oj, rr = h // 2, (h % 2) * 64
                nc.vector.tensor_copy(
                    ocat[oj][rr:rr + 64, qi * 512:(qi + 1) * 512],
                    pt[0:64, :].bitcast(F32R))
                m = g // 8 + 2
                if m < NT * H_PER_CORE:
                    alloc_po(m)
            trans_pending.setdefault(g + 2, []).append(flush)

        def alloc_po(m):
            # pre-allocate + zero block m's accumulator bank (reuses the
            # buffer its predecessor's norm just released)
            qi_m, h_m, _ = loc(8 * m)
            po = pop.tile([128, 4, 65], F32, name="po", tag="po")
            po_tiles[(qi_m, h_m)] = po
            nc.vector.memset(po[:], 0.0)

        def phasec_piece(qi, i, p):
            # out-projection for a 128-row output chunk, two matmul pieces.
            # For the last query block the ocat[1] half (heads 2,3) finishes
            # first (HPERM3), so it goes in the first matmul.
            n0 = qi * 512 + i * 128
            oa, ob = 0, 1
            if p == 0:
                ph_state[(qi, i)] = pyp.tile([128, 512], F32, name="py", tag="py")
                nc.tensor.matmul(ph_state[(qi, i)][:], ocat[oa][:, n0:n0 + 128],
                                 wo[:, oa * 512:oa * 512 + 512],
                                 start=True, stop=False)
            else:
                py = ph_state.pop((qi, i))
                nc.tensor.matmul(py[:], ocat[ob][:, n0:n0 + 128],
                                 wo[:, ob * 512:ob * 512 + 512],
                                 start=False, stop=True)
                st = wk.tile([128, 512], F32, name="st", tag="st", bufs=4)
                nc.vector.tensor_copy(st[:], py[:])
                nc.sync.dma_start(Y[n0:n0 + 128, :], st[:])

        # Static filler schedule: iteration -> emitters. PE is in-order, so
        # a block's pieces must all be emitted before the first dots that
        # reads its output (dots(g) is emitted at iter g-2), with ~2 iters
        # of margin for the clamp chain to land.
        sched = {}

        def at(g, *fs):
            sched.setdefault(g, []).extend(fs)

        # qi=0: V tiles (pair 2j,2j+1 before iter j), K for later t-blocks.
        at(0, lambda: v_piece(2, 0), lambda: v_piece(2, 1),
            lambda: v_piece(3, 0), lambda: v_piece(3, 1))
        at(1, lambda: v_piece(4, 0), lambda: v_piece(4, 1),
            lambda: v_piece(5, 0), lambda: v_piece(5, 1),
            lambda: kq_all(2, 0, 'k'))
        at(2, lambda: v_piece(6, 0), lambda: v_piece(6, 1),
            lambda: v_piece(7, 0), lambda: v_piece(7, 1))
        at(3, lambda: v_piece(8, 0), lambda: v_piece(8, 1),
            lambda: v_piece(9, 0), lambda: v_piece(9, 1),
            lambda: kq_all(3, 0, 'k'))
        at(4, lambda: v_piece(10, 0), lambda: v_piece(10, 1),
            lambda: v_piece(11, 0), lambda: v_piece(11, 1))
        at(5, lambda: v_piece(12, 0), lambda: v_piece(12, 1),
            lambda: v_piece(13, 0), lambda: v_piece(13, 1))
        at(6, lambda: v_piece(14, 0), lambda: v_piece(14, 1),
            lambda: v_piece(15, 0), lambda: v_piece(15, 1))
        # head-group-1 K/Q projections, fine-spread with clamp margin
        PLAN1 = {
            7: [(0, 'k', 4), (0, 'k', 0)],
            8: [(0, 'k', 1), (0, 'k', 2)],
            9: [(0, 'k', 3), (0, 'q', 4)],
            10: [(0, 'q', 0), (0, 'q', 1)],
            11: [(0, 'q', 2), (0, 'q', 3)],
            12: [(1, 'k', 4), (1, 'k', 0)],
            13: [(1, 'k', 1), (1, 'k', 2), (1, 'k', 3)],
            14: [(2, 'k', 4), (2, 'k', 0), (2, 'k', 1)],
            15: [(2, 'k', 2), (2, 'k', 3)],
            16: [(3, 'k', 4), (3, 'k', 0)],
            17: [(3, 'k', 1), (3, 'k', 2)],
            18: [(3, 'k', 3)],
        }
        for it, pieces in PLAN1.items():
            for (t, kind, p) in pieces:
                at(it, lambda t=t, kind=kind, p=p: kq_piece_k(t, 1, kind, p))

        # Q projections for the next query block (both head-groups)
        def q_sched(base, t, off1=25):
            for p, off in ((4, 0), (0, 1), (1, 2), (2, 3), (3, 4)):
                at(base + 21 + off, lambda t=t, p=p: kq_piece_k(t, 0, 'q', p))
                at(base + off1 + off, lambda t=t, p=p: kq_piece_k(t, 1, 'q', p))

        q_sched(0, 1)
        q_sched(32, 2)
        q_sched(64, 3)
        # out-projection for the previous block, after its last norm
        for qi in range(1, NT):
            for i in range(4):
                at(32 * qi + 2 + 2 * i, lambda qi=qi, i=i: phasec_piece(qi - 1, i, 0))
                at(32 * qi + 3 + 2 * i, lambda qi=qi, i=i: phasec_piece(qi - 1, i, 1))

        # Prologue. The dummy exp preloads the ACT table during DMA wait.
        # The Q(0,0) block borrows the first dots PSUM buffer so all four
        # projection blocks run without pool-rotation WAR stalls; pieces are
        # ordered so the ones needing only the first half of the t0 DMA
        # start earliest.
        nc.vector.memset(vtall[:, :, :, 64], 1.0)   # denominator ones cols
        nc.vector.memset(ebias[:], EXP_BIAS)
        warm = wk.tile([128, 1], F32, name="warm", tag="warm")
        nc.scalar.activation(warm[:], ebias[:], EXP)
        # Q staging packs pixel/coord into the two banks of ONE dots-pool
        # tile, so the second tile is free for dots(0) with no
        # write-after-read stall on the Q clamps.
        qtile = psp.tile([128, 2, 512], F32, name="ps", tag="ps")
        # PE warm-up: dummy bf16 matmuls on a zeroed tile keep the PE busy
        # through the input-DMA wait so the p-state ramp (3us of continuous
        # execution) completes before the real projections start.
        pe_w = wk.tile([128, 2, 512], BF16, name="pe", tag="pe", bufs=8)
        nc.gpsimd.memset(pe_w[:], 0.0)
        for _ in range(12):
            nc.tensor.matmul(qtile[:, 0, 0:256], pe_w[:, 0, 0:128],
                             pe_w[:, 1, 0:256], start=True, stop=True)
        for p in (0, 1):
            kq_piece(0, 0, KP[0], KC[0], kcat, p, direct=True)
            kq_piece(0, 0, QP[0], QC[0], qcat, p, ps_ap=qtile[:, 0, :],
                     direct=True)
        kq_piece(0, 0, KP[0], KC[0], kcat, 4, direct=True)
        kq_piece(0, 0, QP[0], QC[0], qcat, 4, ps2_ap=qtile[:, 1, :],
                 direct=True)
        for p in (2, 3):
            kq_piece(0, 0, KP[0], KC[0], kcat, p, direct=True)
            kq_piece(0, 0, QP[0], QC[0], qcat, p, ps_ap=qtile[:, 0, :],
                     direct=True)
        # odd-head clamps, Q side first: dots(1) reuses the Q staging banks
        for f in deferred_clamps[1::2] + deferred_clamps[0::2]:
            f()
        deferred_clamps.clear()
        emit_dots(0)
        emit_dots(1)
        kq_all(1, 0, 'k')
        v_piece(0, 0)
        v_piece(0, 1)
        v_piece(1, 0)
        v_piece(1, 1)
        alloc_po(0)
        alloc_po(1)

        for g in range(NPAIR):
            qi, h, j = loc(g)
            pe = wk.tile([128, 2, 512], BF16, name="pe", tag="pe", bufs=8)
            nc.scalar.activation(pe[:], ps_tiles.pop(g)[:], EXP, scale=SCALE)
            if g + 2 < NPAIR:
                emit_dots(g + 2)
            emit_av(g, pe)
            if j == NKT // 2 - 1:
                emit_norm(qi, h, g)
            for f in trans_pending.pop(g, ()):
                f()
            for f in sched.get(g, ()):
                f()
        for fs in [trans_pending.pop(k) for k in sorted(trans_pending)]:
            for f in fs:
                f()
        for i in range(4):
            phasec_piece(NT - 1, i, 0)
            phasec_piece(NT - 1, i, 1)
    nc.compile()
    return nc


def _get_nc():
    global _NC
    if _NC is None:
        _NC = _build()
    return _NC


def _bf16(a):
    return np.ascontiguousarray(np.asarray(a, np.float32).astype(ml_dtypes.bfloat16))


def _pack_gcd(w):
    # [512, 256] -> [128, (g, c, d)]: head-group-major weight layout so the
    # first group's columns are one contiguous DMA.
    return (np.asarray(w, np.float32)
            .reshape(4, 128, 2, 128).transpose(1, 2, 0, 3).reshape(128, 1024))


def _pack_chd(w):
    # [512, 256] -> [128, (c, h, d)]: pd-chunk-major layout for the V weights.
    return (np.asarray(w, np.float32)
            .reshape(4, 128, 256).transpose(1, 0, 2).reshape(128, 1024))


def _pack_wo(w):
    w = np.asarray(w, dtype=np.float32)
    return np.ascontiguousarray(
        w.reshape(2, 128, 512).transpose(1, 0, 2).reshape(128, 1024))


def kernel(pixels, coords, mask, W_qkv, W_qkc, W_out, b_out):
    global LAST_EXEC_NS
    pixels = np.asarray(pixels, dtype=np.float32)
    coords = np.asarray(coords, dtype=np.float32)
    W_qkv = np.asarray(W_qkv, dtype=np.float32)
    W_qkc = np.asarray(W_qkc, dtype=np.float32)
    W_out = np.asarray(W_out, dtype=np.float32)
    b_out = np.asarray(b_out, dtype=np.float32)

    nc = _get_nc()

    XTg = [np.ascontiguousarray(
        np.concatenate([pixels[b].T.reshape(4, 128, N),
                        coords[b].T.reshape(1, 128, N)], axis=0)
        .transpose(1, 0, 2).astype(ml_dtypes.bfloat16)) for b in range(B)]

    in_maps = []
    for c in range(8):
        b = c // 2
        h0 = (c % 2) * H_PER_CORE * DH     # 0 or 256: col offset within split
        kp = _pack_gcd(W_qkv[:, ID + h0:ID + h0 + 256])
        qp = _pack_gcd(W_qkv[:, h0:h0 + 256])
        v = _pack_chd(W_qkv[:, 2 * ID + h0:2 * ID + h0 + 256])
        kc = np.asarray(W_qkc[:, ID + h0:ID + h0 + 256], np.float32)
        qc = np.asarray(W_qkc[:, h0:h0 + 256], np.float32)
        wall = np.concatenate([
            kp[:, 0:512], kc[:, 0:128], qp[:, 0:512], qc[:, 0:128], v,
            kp[:, 512:1024], kc[:, 128:256], qp[:, 512:1024], qc[:, 128:256],
        ], axis=1)
        in_maps.append({
            "XTg": XTg[b],
            "Wall": _bf16(wall),
            "Wo": _pack_wo(W_out[h0:h0 + 256, :]),
            "Ident": np.eye(128, dtype=np.float32),
        })

    res = run_bass_kernel_spmd(nc, in_maps, core_ids=list(range(8)))
    LAST_EXEC_NS = getattr(res, "exec_time_ns", None)

    out = np.empty((B, N, OUT_D), np.float32)
    for b in range(B):
        out[b] = res.results[2 * b]["Y"] + res.results[2 * b + 1]["Y"]
    out += b_out[None, None, :]
    return tuple(np.split(out, [1024], axis=1))


# revision 61
# speedup vs baseline: 1.2320x; 1.0070x over previous
import numpy as np
import ml_dtypes

import concourse.bacc as bacc
import concourse.tile as tile
import concourse.mybir as mybir
from concourse.bass_utils import run_bass_kernel_spmd

F32 = mybir.dt.float32
F32R = mybir.dt.float32r
BF16 = mybir.dt.bfloat16
F8E4 = mybir.dt.float8e4
F8E5 = mybir.dt.float8e5

B = 4
N = 2048
PD = 512
CD = 128
ID = 512
OUT_D = 512
H_PER_CORE = 4
DH = 64
SCALE = 0.125          # dim_head ** -0.5
NT = 4                 # n chunks of 512
NP = 4                 # pd chunks of 128
NKT = 16               # key tiles of 128
EXP_BIAS = -2.0        # softmax logits shifted so exp() fits fp8e5 range

# column offsets inside the packed weight block (bf16), head-group-major:
# [kp_g0 512][kc_g0 128][qp_g0 512][qc_g0 128][v 1024]
# [kp_g1 512][kc_g1 128][qp_g1 512][qc_g1 128]
KP = {0: 0, 1: 2304}
KC = {0: 512, 1: 2816}
QP = {0: 640, 1: 2944}
QC = {0: 1152, 1: 3456}
WV = 1280
WALL_COLS = 3584

_NC = None
LAST_EXEC_NS = None


def _build():
    nc = bacc.Bacc("TRN2", target_bir_lowering=False, debug=False, num_devices=8)
    XTg = nc.declare_dram_parameter("XTg", [128, 5, N], BF16, isOutput=False)
    Wall = nc.declare_dram_parameter("Wall", [128, WALL_COLS], BF16, isOutput=False)
    Wo = nc.declare_dram_parameter("Wo", [128, 1024], F32, isOutput=False)
    Ident = nc.declare_dram_parameter("Ident", [128, 128], F32, isOutput=False)
    Y = nc.declare_dram_parameter("Y", [N, OUT_D], F32, isOutput=True)

    MIN = mybir.AluOpType.min
    MAX = mybir.AluOpType.max
    MULT = mybir.AluOpType.mult
    EXP = mybir.ActivationFunctionType.Exp
    DR = mybir.MatmulPerfMode.DoubleRow

    with tile.TileContext(nc) as tc, \
         tc.tile_pool(name="persist", bufs=1) as pp, \
         tc.tile_pool(name="work", bufs=2) as wk, \
         tc.tile_pool(name="pb_s", bufs=2, space="PSUM") as psp, \
         tc.tile_pool(name="pb_o", bufs=2, space="PSUM") as pop, \
         tc.tile_pool(name="pc_y", bufs=2, space="PSUM") as pyp:
        xtall = pp.tile([128, 5, N], BF16, name="xtall", tag="xtall")
        wall = pp.tile([128, WALL_COLS], BF16, name="wall", tag="wall")
        wo = pp.tile([128, 1024], F32R, name="wo", tag="wo")
        ident = pp.tile([128, 128], F32R, name="ident", tag="ident")
        qcat = [pp.tile([128, N], F32R, name=f"qcat{h}", tag=f"qcat{h}")
                for h in range(H_PER_CORE)]
        kcat = [pp.tile([128, N], F32R, name=f"kcat{h}", tag=f"kcat{h}")
                for h in range(H_PER_CORE)]
        # per key tile: 4 heads x (64 V columns | 1 ones column for the
        # softmax denominator), bf16
        vtall = pp.tile([128, NKT, 4, 65], BF16, name="vtall", tag="vtall")
        ebias = pp.tile([128, 1], F32, name="ebias", tag="ebias")
        ocat = [pp.tile([128, N], F32R, name=f"ocat{j}", tag=f"ocat{j}")
                for j in range(2)]

        # Input DMAs. Queue A (sync) carries the critical stream in priority
        # order; queue B (gpsimd) slips the later weights into bus gaps.
        # Big packed transfers amortize the ~650ns per-DMA issue cost.
        def dma_xt_t(t, a, b):
            nc.sync.dma_start(xtall[:, a:b, t * 512:(t + 1) * 512],
                              XTg[:, a:b, t * 512:(t + 1) * 512])

        nc.sync.dma_start(wall[:, 0:1280], Wall[:, 0:1280])        # g0 weights
        dma_xt_t(0, 0, 2)
        dma_xt_t(0, 4, 5)
        dma_xt_t(0, 2, 4)
        nc.sync.dma_start(wall[:, WV:WV + 512], Wall[:, WV:WV + 512])
        dma_xt_t(1, 0, 5)
        nc.sync.dma_start(wall[:, WV + 512:WV + 1024],
                          Wall[:, WV + 512:WV + 1024])
        dma_xt_t(2, 0, 5)
        dma_xt_t(3, 0, 5)
        nc.sync.dma_start(ident[:], Ident[:].bitcast(F32R))
        nc.sync.dma_start(wall[:, 2304:3584], Wall[:, 2304:3584])  # g1
        nc.sync.dma_start(wo[:], Wo[:].bitcast(F32R))

        NPAIR = NT * H_PER_CORE * (NKT // 2)
        ps_tiles = {}
        po_tiles = {}
        qstate = {}
        vstate = {}
        ph_state = {}
        trans_pending = {}

        CP = mybir.ActivationFunctionType.Copy

        def emit_clamps(ps, dest, h0, row0, t, mode):
            # GPSIMD cannot read PSUM, so clamps from PSUM route through:
            # 'prologue' - even-head clamp direct on DVE (shortest chain to
            #   the first dots); odd-head rows staged via an ACT Copy (Copy
            #   shares the Exp table, and ACT idles here) and clamped on
            #   Pool. 'act' - both halves ACT-staged + Pool-clamped (for
            #   blocks landing in ACT stall windows). 'steady' - one DVE
            #   copy + two Pool clamps.
            def clamp(eng, jj, src, srow):
                eng.tensor_scalar(
                    dest[h0 + jj][row0:row0 + 64, t * 512:(t + 1) * 512],
                    src[srow:srow + 64, :], 5.0, -5.0, op0=MIN, op1=MAX)

            if mode == 'prologue':
                clamp(nc.vector, 0, ps, 0)
                scr = wk.tile([64, 512], F32, name="sc6", tag="sc6", bufs=4)
                nc.scalar.activation(scr[:], ps[64:128, :], CP)
                clamp(nc.gpsimd, 1, scr, 0)
            elif mode == 'act':
                scr = wk.tile([128, 512], F32, name="scr", tag="scr", bufs=3)
                nc.scalar.activation(scr[:], ps[:], CP)
                for jj in (0, 1):
                    clamp(nc.gpsimd, jj, scr, jj * 64)
            else:
                scr = wk.tile([128, 512], F32, name="scr", tag="scr", bufs=3)
                nc.vector.tensor_copy(scr[:], ps[:])
                for jj in (0, 1):
                    clamp(nc.gpsimd, jj, scr, jj * 64)

        def kq_piece(t, g, pbase, cbase, dest, p, ps_ap=None, ps2_ap=None,
                     mode='steady'):
            # p 4: coord matmul + clamps (emitted first: its clamp chain is
            # independent of the pixel pieces). p 0..3: pixel-chunk matmuls
            # accumulating over pd; p 3 also clamps. One clamp goes to DVE,
            # the other to Pool, halving the chain latency.
            key = (t, g, id(dest))
            if p < 4:
                if p == 0:
                    qstate[key] = (ps_ap if ps_ap is not None else
                                   pyp.tile([128, 512], F32, name="py", tag="py"))
                ps = qstate[key]
                nc.tensor.matmul(
                    ps[:], wall[:, pbase + p * 128:pbase + (p + 1) * 128],
                    xtall[:, p, t * 512:(t + 1) * 512],
                    start=(p == 0), stop=(p == 3))
                if p == 3:
                    emit_clamps(ps, dest, 2 * g, 0, t, mode)
                    del qstate[key]
            else:
                ps2 = (ps2_ap if ps2_ap is not None else
                       pyp.tile([128, 512], F32, name="py", tag="py"))
                nc.tensor.matmul(ps2[:], wall[:, cbase:cbase + 128],
                                 xtall[:, 4, t * 512:(t + 1) * 512],
                                 start=True, stop=True)
                emit_clamps(ps2, dest, 2 * g, 64, t, mode)

        def kq_all(t, g, kind, mode='steady'):
            for p in (4, 0, 1, 2, 3):
                kq_piece_k(t, g, kind, p, mode)

        def kq_piece_k(t, g, kind, p, mode='steady'):
            if kind == 'k':
                kq_piece(t, g, KP[g], KC[g], kcat, p, mode=mode)
            else:
                kq_piece(t, g, QP[g], QC[g], qcat, p, mode=mode)

        def v_piece(kt, half):
            # half 0: pd chunks 0,1; half 1: pd chunks 2,3 + one fp8 copy
            # moving all four heads' V columns out in a single instruction.
            t, i = divmod(kt, 4)
            if half == 0:
                vstate[kt] = pyp.tile([128, 512], F32, name="py", tag="py")
            pv = vstate[kt]
            for p in (0, 1) if half == 0 else (2, 3):
                nc.tensor.matmul(
                    pv[:, 0:256],
                    xtall[:, p, t * 512 + i * 128:t * 512 + (i + 1) * 128],
                    wall[:, WV + p * 256:WV + (p + 1) * 256],
                    start=(p == 0), stop=(p == NP - 1))
            if half == 1:
                if kt < 0:
                    nc.scalar.activation(vtall[:, kt, :, 0:64], pv[:, 0:256],
                                         CP)
                else:
                    nc.vector.tensor_copy(vtall[:, kt, :, 0:64], pv[:, 0:256])
                del vstate[kt]

        def loc(g):
            qi, r = divmod(g, 32)
            h, j = divmod(r, 8)
            return qi, h, j

        def emit_dots(g):
            qi, h, j = loc(g)
            ps = psp.tile([128, 2, 512], F32, name="ps", tag="ps")
            ps_tiles[g] = ps
            k0, k1 = 2 * j, 2 * j + 1
            nc.tensor.matmul(
                ps[:, 0, :], kcat[h][:, k0 * 128:(k0 + 1) * 128],
                qcat[h][:, qi * 512:(qi + 1) * 512], start=True, stop=True)
            nc.tensor.matmul(
                ps[:, 1, :], kcat[h][:, k1 * 128:(k1 + 1) * 128],
                qcat[h][:, qi * 512:(qi + 1) * 512], start=True, stop=True)

        def emit_av(g, pe):
            # fp8 DoubleRow: one matmul covers both key tiles of the pair
            # (K=256) at 0.5 cyc/row; a second 1-row matmul accumulates the
            # softmax denominator into po[64].
            # Transposed AV in bf16: the exp scores [keys, 128q] are the
            # full-width stationary, V plus a ones column (denominator) is
            # the 65-wide moving operand. Output [q, 64v|den] accumulates
            # over all 16 key tiles. Four query subtiles share one PSUM
            # bank: since PSUM zero-on-start is bank-granular, the bank is
            # pre-zeroed by a DVE memset (two blocks ahead) and every
            # matmul is a pure accumulate.
            qi, h, j = loc(g)
            po = po_tiles[(qi, h)]
            for i in (0, 1):
                for q in range(4):
                    nc.tensor.matmul(
                        po[:, q, :], pe[:, i, q * 128:(q + 1) * 128],
                        vtall[:, 2 * j + i, h, :],
                        start=False,
                        stop=(j == NKT // 2 - 1 and i == 1),
                        skip_group_check=True)

        def emit_norm(qi, h, g):
            # scale the V columns by 1/den per query row; the transposes
            # back to [d, n] layout are deferred two iterations so the PE
            # queue never blocks on this DVE chain.
            po = po_tiles.pop((qi, h))
            rb = wk.tile([128, 4], F32, name="rb", tag="rb")
            oc_t = wk.tile([128, 4, 64], F32R, name="oct", tag="oct")
            nc.vector.reciprocal(rb[:], po[:, :, 64])
            for q in range(4):
                nc.vector.tensor_scalar(
                    oc_t[:, q, :], po[:, q, 0:64], rb[:, q:q + 1], None,
                    op0=MULT)

            def flush(qi=qi, h=h, oc_t=oc_t):
                pt = pyp.tile([128, 512], F32, name="py", tag="py")
                for q in range(4):
                    nc.tensor.transpose(
                        pt[0:64, q * 128:(q + 1) * 128].bitcast(F32R),
                        oc_t[:, q, :], ident[:])
                oj, rr = h // 2, (h % 2) * 64
                nc.vector.tensor_copy(
                    ocat[oj][rr:rr + 64, qi * 512:(qi + 1) * 512],
                    pt[0:64, :].bitcast(F32R))
                m = g // 8 + 2
                if m < NT * H_PER_CORE:
                    alloc_po(m)
            trans_pending.setdefault(g + 2, []).append(flush)

        def alloc_po(m):
            # pre-allocate + zero block m's accumulator bank (reuses the
            # buffer its predecessor's norm just released)
            qi_m, h_m, _ = loc(8 * m)
            po = pop.tile([128, 4, 65], F32, name="po", tag="po")
            po_tiles[(qi_m, h_m)] = po
            nc.vector.memset(po[:], 0.0)

        def phasec_piece(qi, i, p):
            # out-projection for a 128-row output chunk, two matmul pieces.
            # For the last query block the ocat[1] half (heads 2,3) finishes
            # first (HPERM3), so it goes in the first matmul.
            n0 = qi * 512 + i * 128
            oa, ob = 0, 1
            if p == 0:
                ph_state[(qi, i)] = pyp.tile([128, 512], F32, name="py", tag="py")
                nc.tensor.matmul(ph_state[(qi, i)][:], ocat[oa][:, n0:n0 + 128],
                                 wo[:, oa * 512:oa * 512 + 512],
                                 start=True, stop=False)
            else:
                py = ph_state.pop((qi, i))
                nc.tensor.matmul(py[:], ocat[ob][:, n0:n0 + 128],
                                 wo[:, ob * 512:ob * 512 + 512],
                                 start=False, stop=True)
                st = wk.tile([128, 512], F32, name="st", tag="st", bufs=4)
                nc.vector.tensor_copy(st[:], py[:])
                nc.sync.dma_start(Y[n0:n0 + 128, :], st[:])

        # Static filler schedule: iteration -> emitters. PE is in-order, so
        # a block's pieces must all be emitted before the first dots that
        # reads its output (dots(g) is emitted at iter g-2), with ~2 iters
        # of margin for the clamp chain to land.
        sched = {}

        def at(g, *fs):
            sched.setdefault(g, []).extend(fs)

        # qi=0: V tiles (pair 2j,2j+1 before iter j), K for later t-blocks.
        at(0, lambda: v_piece(2, 0), lambda: v_piece(2, 1),
            lambda: v_piece(3, 0), lambda: v_piece(3, 1))
        at(1, lambda: v_piece(4, 0), lambda: v_piece(4, 1),
            lambda: v_piece(5, 0), lambda: v_piece(5, 1),
            lambda: kq_all(2, 0, 'k', 'act'))
        at(2, lambda: v_piece(6, 0), lambda: v_piece(6, 1),
            lambda: v_piece(7, 0), lambda: v_piece(7, 1))
        at(3, lambda: v_piece(8, 0), lambda: v_piece(8, 1),
            lambda: v_piece(9, 0), lambda: v_piece(9, 1),
            lambda: kq_all(3, 0, 'k', 'act'))
        at(4, lambda: v_piece(10, 0), lambda: v_piece(10, 1),
            lambda: v_piece(11, 0), lambda: v_piece(11, 1))
        at(5, lambda: v_piece(12, 0), lambda: v_piece(12, 1),
            lambda: v_piece(13, 0), lambda: v_piece(13, 1))
        at(6, lambda: v_piece(14, 0), lambda: v_piece(14, 1),
            lambda: v_piece(15, 0), lambda: v_piece(15, 1))
        # head-group-1 K/Q projections, fine-spread with clamp margin
        PLAN1 = {
            7: [(0, 'k', 4), (0, 'k', 0)],
            8: [(0, 'k', 1), (0, 'k', 2)],
            9: [(0, 'k', 3), (0, 'q', 4)],
            10: [(0, 'q', 0), (0, 'q', 1)],
            11: [(0, 'q', 2), (0, 'q', 3)],
            12: [(1, 'k', 4), (1, 'k', 0)],
            13: [(1, 'k', 1), (1, 'k', 2), (1, 'k', 3)],
            14: [(2, 'k', 4), (2, 'k', 0), (2, 'k', 1)],
            15: [(2, 'k', 2), (2, 'k', 3)],
            16: [(3, 'k', 4), (3, 'k', 0)],
            17: [(3, 'k', 1), (3, 'k', 2)],
            18: [(3, 'k', 3)],
        }
        for it, pieces in PLAN1.items():
            for (t, kind, p) in pieces:
                at(it, lambda t=t, kind=kind, p=p: kq_piece_k(t, 1, kind, p))

        # Q projections for the next query block (both head-groups)
        def q_sched(base, t, off1=25):
            for p, off in ((4, 0), (0, 1), (1, 2), (2, 3), (3, 4)):
                at(base + 21 + off, lambda t=t, p=p: kq_piece_k(t, 0, 'q', p))
                at(base + off1 + off, lambda t=t, p=p: kq_piece_k(t, 1, 'q', p))

        q_sched(0, 1)
        q_sched(32, 2)
        q_sched(64, 3)
        # out-projection for the previous block, after its last norm
        for qi in range(1, NT):
            for i in range(4):
                at(32 * qi + 2 + 3 * i, lambda qi=qi, i=i: phasec_piece(qi - 1, i, 0))
                at(32 * qi + 3 + 3 * i, lambda qi=qi, i=i: phasec_piece(qi - 1, i, 1))

        # Prologue. The dummy exp preloads the ACT table during DMA wait.
        # The Q(0,0) block borrows the first dots PSUM buffer so all four
        # projection blocks run without pool-rotation WAR stalls; pieces are
        # ordered so the ones needing only the first half of the t0 DMA
        # start earliest.
        nc.vector.memset(vtall[:, :, :, 64], 1.0)   # denominator ones cols
        nc.vector.memset(ebias[:], EXP_BIAS)
        warm = wk.tile([128, 1], F32, name="warm", tag="warm")
        nc.scalar.activation(warm[:], ebias[:], EXP)
        # Q staging packs pixel/coord into the two banks of ONE dots-pool
        # tile, so the second tile is free for dots(0) with no
        # write-after-read stall on the Q clamps.
        qtile = psp.tile([128, 2, 512], F32, name="ps", tag="ps")
        # PE warm-up: dummy bf16 matmuls on a zeroed tile keep the PE busy
        # through the input-DMA wait so the p-state ramp (3us of continuous
        # execution) completes before the real projections start.
        pe_w = wk.tile([128, 2, 512], BF16, name="pe", tag="pe", bufs=8)
        nc.gpsimd.memset(pe_w[:], 0.0)
        for _ in range(12):
            nc.tensor.matmul(qtile[:, 0, 0:256], pe_w[:, 0, 0:128],
                             pe_w[:, 1, 0:256], start=True, stop=True)
        for p in (0, 1):
            kq_piece(0, 0, KP[0], KC[0], kcat, p, mode='prologue')
            kq_piece(0, 0, QP[0], QC[0], qcat, p, ps_ap=qtile[:, 0, :],
                     mode='prologue')
        kq_piece(0, 0, KP[0], KC[0], kcat, 4, mode='prologue')
        kq_piece(0, 0, QP[0], QC[0], qcat, 4, ps2_ap=qtile[:, 1, :],
                 mode='prologue')
        for p in (2, 3):
            kq_piece(0, 0, KP[0], KC[0], kcat, p, mode='prologue')
            kq_piece(0, 0, QP[0], QC[0], qcat, p, ps_ap=qtile[:, 0, :],
                     mode='prologue')
        emit_dots(0)
        emit_dots(1)
        kq_all(1, 0, 'k', mode='prologue')
        v_piece(0, 0)
        v_piece(0, 1)
        v_piece(1, 0)
        v_piece(1, 1)
        alloc_po(0)
        alloc_po(1)

        for g in range(NPAIR):
            qi, h, j = loc(g)
            pe = wk.tile([128, 2, 512], BF16, name="pe", tag="pe", bufs=8)
            nc.scalar.activation(pe[:], ps_tiles.pop(g)[:], EXP, scale=SCALE)
            if g + 2 < NPAIR:
                emit_dots(g + 2)
            emit_av(g, pe)
            if j == NKT // 2 - 1:
                emit_norm(qi, h, g)
            for f in trans_pending.pop(g, ()):
                f()
            for f in sched.get(g, ()):
                f()
        for fs in [trans_pending.pop(k) for k in sorted(trans_pending)]:
            for f in fs:
                f()
        for i in range(4):
            phasec_piece(NT - 1, i, 0)
            phasec_piece(NT - 1, i, 1)
    nc.compile()
    return nc


def _get_nc():
    global _NC
    if _NC is None:
        _NC = _build()
    return _NC


def _bf16(a):
    return np.ascontiguousarray(np.asarray(a, np.float32).astype(ml_dtypes.bfloat16))


def _pack_gcd(w):
    # [512, 256] -> [128, (g, c, d)]: head-group-major weight layout so the
    # first group's columns are one contiguous DMA.
    return (np.asarray(w, np.float32)
            .reshape(4, 128, 2, 128).transpose(1, 2, 0, 3).reshape(128, 1024))


def _pack_chd(w):
    # [512, 256] -> [128, (c, h, d)]: pd-chunk-major layout for the V weights.
    return (np.asarray(w, np.float32)
            .reshape(4, 128, 256).transpose(1, 0, 2).reshape(128, 1024))


def _pack_wo(w):
    w = np.asarray(w, dtype=np.float32)
    return np.ascontiguousarray(
        w.reshape(2, 128, 512).transpose(1, 0, 2).reshape(128, 1024))


def kernel(pixels, coords, mask, W_qkv, W_qkc, W_out, b_out):
    global LAST_EXEC_NS
    pixels = np.asarray(pixels, dtype=np.float32)
    coords = np.asarray(coords, dtype=np.float32)
    W_qkv = np.asarray(W_qkv, dtype=np.float32)
    W_qkc = np.asarray(W_qkc, dtype=np.float32)
    W_out = np.asarray(W_out, dtype=np.float32)
    b_out = np.asarray(b_out, dtype=np.float32)

    nc = _get_nc()

    XTg = [np.ascontiguousarray(
        np.concatenate([pixels[b].T.reshape(4, 128, N),
                        coords[b].T.reshape(1, 128, N)], axis=0)
        .transpose(1, 0, 2).astype(ml_dtypes.bfloat16)) for b in range(B)]

    in_maps = []
    for c in range(8):
        b = c // 2
        h0 = (c % 2) * H_PER_CORE * DH     # 0 or 256: col offset within split
        kp = _pack_gcd(W_qkv[:, ID + h0:ID + h0 + 256])
        qp = _pack_gcd(W_qkv[:, h0:h0 + 256])
        v = _pack_chd(W_qkv[:, 2 * ID + h0:2 * ID + h0 + 256])
        kc = np.asarray(W_qkc[:, ID + h0:ID + h0 + 256], np.float32)
        qc = np.asarray(W_qkc[:, h0:h0 + 256], np.float32)
        wall = np.concatenate([
            kp[:, 0:512], kc[:, 0:128], qp[:, 0:512], qc[:, 0:128], v,
            kp[:, 512:1024], kc[:, 128:256], qp[:, 512:1024], qc[:, 128:256],
        ], axis=1)
        in_maps.append({
            "XTg": XTg[b],
            "Wall": _bf16(wall),
            "Wo": _pack_wo(W_out[h0:h0 + 256, :]),
            "Ident": np.eye(128, dtype=np.float32),
        })

    res = run_bass_kernel_spmd(nc, in_maps, core_ids=list(range(8)))
    LAST_EXEC_NS = getattr(res, "exec_time_ns", None)

    out = np.empty((B, N, OUT_D), np.float32)
    for b in range(B):
        out[b] = res.results[2 * b]["Y"] + res.results[2 * b + 1]["Y"]
    out += b_out[None, None, :]
    return tuple(np.split(out, [1024], axis=1))


# revision 64
# speedup vs baseline: 1.2659x; 1.0275x over previous
import numpy as np
import ml_dtypes

import concourse.bacc as bacc
import concourse.tile as tile
import concourse.mybir as mybir
from concourse.bass_utils import run_bass_kernel_spmd

F32 = mybir.dt.float32
F32R = mybir.dt.float32r
BF16 = mybir.dt.bfloat16
F8E4 = mybir.dt.float8e4
F8E5 = mybir.dt.float8e5

B = 4
N = 2048
PD = 512
CD = 128
ID = 512
OUT_D = 512
H_PER_CORE = 4
DH = 64
SCALE = 0.125          # dim_head ** -0.5
NT = 4                 # n chunks of 512
NP = 4                 # pd chunks of 128
NKT = 16               # key tiles of 128
EXP_BIAS = -2.0        # softmax logits shifted so exp() fits fp8e5 range

# column offsets inside the packed weight block (bf16), head-group-major:
# [kp_g0 512][kc_g0 128][qp_g0 512][qc_g0 128][v 1024]
# [kp_g1 512][kc_g1 128][qp_g1 512][qc_g1 128]
KP = {0: 0, 1: 2304}
KC = {0: 512, 1: 2816}
QP = {0: 640, 1: 2944}
QC = {0: 1152, 1: 3456}
WV = 1280
WALL_COLS = 3584

_NC = None
LAST_EXEC_NS = None


def _build():
    nc = bacc.Bacc("TRN2", target_bir_lowering=False, debug=False, num_devices=8)
    XTg = nc.declare_dram_parameter("XTg", [128, 5, N], BF16, isOutput=False)
    Wall = nc.declare_dram_parameter("Wall", [128, WALL_COLS], BF16, isOutput=False)
    Wo = nc.declare_dram_parameter("Wo", [128, 1024], F32, isOutput=False)
    Ident = nc.declare_dram_parameter("Ident", [128, 128], F32, isOutput=False)
    Y = nc.declare_dram_parameter("Y", [N, OUT_D], F32, isOutput=True)

    MIN = mybir.AluOpType.min
    MAX = mybir.AluOpType.max
    MULT = mybir.AluOpType.mult
    EXP = mybir.ActivationFunctionType.Exp
    DR = mybir.MatmulPerfMode.DoubleRow

    with tile.TileContext(nc) as tc, \
         tc.tile_pool(name="persist", bufs=1) as pp, \
         tc.tile_pool(name="work", bufs=2) as wk, \
         tc.tile_pool(name="pb_s", bufs=2, space="PSUM") as psp, \
         tc.tile_pool(name="pb_o", bufs=2, space="PSUM") as pop, \
         tc.tile_pool(name="pc_y", bufs=2, space="PSUM") as pyp:
        xtall = pp.tile([128, 5, N], BF16, name="xtall", tag="xtall")
        wall = pp.tile([128, WALL_COLS], BF16, name="wall", tag="wall")
        wo = pp.tile([128, 1024], F32R, name="wo", tag="wo")
        ident = pp.tile([128, 128], F32R, name="ident", tag="ident")
        qcat = [pp.tile([128, N], F32R, name=f"qcat{h}", tag=f"qcat{h}")
                for h in range(H_PER_CORE)]
        kcat = [pp.tile([128, N], F32R, name=f"kcat{h}", tag=f"kcat{h}")
                for h in range(H_PER_CORE)]
        # per key tile: 4 heads x (64 V columns | 1 ones column for the
        # softmax denominator), bf16
        vtall = pp.tile([128, NKT, 4, 65], BF16, name="vtall", tag="vtall")
        ebias = pp.tile([128, 1], F32, name="ebias", tag="ebias")
        ocat = [pp.tile([128, N], F32R, name=f"ocat{j}", tag=f"ocat{j}")
                for j in range(2)]

        # Input DMAs. Queue A (sync) carries the critical stream in priority
        # order; queue B (gpsimd) slips the later weights into bus gaps.
        # Big packed transfers amortize the ~650ns per-DMA issue cost.
        def dma_xt_t(t, a, b):
            nc.sync.dma_start(xtall[:, a:b, t * 512:(t + 1) * 512],
                              XTg[:, a:b, t * 512:(t + 1) * 512])

        nc.sync.dma_start(wall[:, 0:1280], Wall[:, 0:1280])        # g0 weights
        dma_xt_t(0, 0, 2)
        dma_xt_t(0, 4, 5)
        dma_xt_t(0, 2, 4)
        nc.sync.dma_start(wall[:, WV:WV + 512], Wall[:, WV:WV + 512])
        dma_xt_t(1, 0, 5)
        nc.sync.dma_start(wall[:, WV + 512:WV + 1024],
                          Wall[:, WV + 512:WV + 1024])
        dma_xt_t(2, 0, 5)
        dma_xt_t(3, 0, 5)
        nc.sync.dma_start(ident[:], Ident[:].bitcast(F32R))
        nc.sync.dma_start(wall[:, 2304:3584], Wall[:, 2304:3584])  # g1
        nc.sync.dma_start(wo[:], Wo[:].bitcast(F32R))

        NPAIR = NT * H_PER_CORE * (NKT // 2)
        ps_tiles = {}
        po_tiles = {}
        qstate = {}
        vstate = {}
        ph_state = {}
        trans_pending = {}

        CP = mybir.ActivationFunctionType.Copy

        def emit_clamps(ps, dest, h0, row0, t, mode):
            # GPSIMD cannot read PSUM, so clamps from PSUM route through:
            # 'prologue' - even-head clamp direct on DVE (shortest chain to
            #   the first dots); odd-head rows staged via an ACT Copy (Copy
            #   shares the Exp table, and ACT idles here) and clamped on
            #   Pool. 'act' - both halves ACT-staged + Pool-clamped (for
            #   blocks landing in ACT stall windows). 'steady' - one DVE
            #   copy + two Pool clamps.
            def clamp(eng, jj, src, srow):
                eng.tensor_scalar(
                    dest[h0 + jj][row0:row0 + 64, t * 512:(t + 1) * 512],
                    src[srow:srow + 64, :], 5.0, -5.0, op0=MIN, op1=MAX)

            if mode == 'prologue':
                clamp(nc.vector, 0, ps, 0)
                scr = wk.tile([64, 512], F32, name="sc6", tag="sc6", bufs=4)
                nc.scalar.activation(scr[:], ps[64:128, :], CP)
                clamp(nc.gpsimd, 1, scr, 0)
            elif mode == 'act':
                scr = wk.tile([128, 512], F32, name="scr", tag="scr", bufs=3)
                nc.scalar.activation(scr[:], ps[:], CP)
                for jj in (0, 1):
                    clamp(nc.gpsimd, jj, scr, jj * 64)
            else:
                scr = wk.tile([128, 512], F32, name="scr", tag="scr", bufs=3)
                nc.vector.tensor_copy(scr[:], ps[:])
                for jj in (0, 1):
                    clamp(nc.gpsimd, jj, scr, jj * 64)

        def kq_piece(t, g, pbase, cbase, dest, p, ps_ap=None, ps2_ap=None,
                     mode='steady'):
            # p 4: coord matmul + clamps (emitted first: its clamp chain is
            # independent of the pixel pieces). p 0..3: pixel-chunk matmuls
            # accumulating over pd; p 3 also clamps. One clamp goes to DVE,
            # the other to Pool, halving the chain latency.
            key = (t, g, id(dest))
            if p < 4:
                if p == 0:
                    qstate[key] = (ps_ap if ps_ap is not None else
                                   pyp.tile([128, 512], F32, name="py", tag="py"))
                ps = qstate[key]
                nc.tensor.matmul(
                    ps[:], wall[:, pbase + p * 128:pbase + (p + 1) * 128],
                    xtall[:, p, t * 512:(t + 1) * 512],
                    start=(p == 0), stop=(p == 3))
                if p == 3:
                    emit_clamps(ps, dest, 2 * g, 0, t, mode)
                    del qstate[key]
            else:
                ps2 = (ps2_ap if ps2_ap is not None else
                       pyp.tile([128, 512], F32, name="py", tag="py"))
                nc.tensor.matmul(ps2[:], wall[:, cbase:cbase + 128],
                                 xtall[:, 4, t * 512:(t + 1) * 512],
                                 start=True, stop=True)
                emit_clamps(ps2, dest, 2 * g, 64, t, mode)

        def kq_all(t, g, kind, mode='steady'):
            for p in (4, 0, 1, 2, 3):
                kq_piece_k(t, g, kind, p, mode)

        def kq_piece_k(t, g, kind, p, mode='steady'):
            if kind == 'k':
                kq_piece(t, g, KP[g], KC[g], kcat, p, mode=mode)
            else:
                kq_piece(t, g, QP[g], QC[g], qcat, p, mode=mode)

        def v_piece(kt, half):
            # half 0: pd chunks 0,1; half 1: pd chunks 2,3 + one fp8 copy
            # moving all four heads' V columns out in a single instruction.
            t, i = divmod(kt, 4)
            if half == 0:
                vstate[kt] = pyp.tile([128, 512], F32, name="py", tag="py")
            pv = vstate[kt]
            for p in (0, 1) if half == 0 else (2, 3):
                nc.tensor.matmul(
                    pv[:, 0:256],
                    xtall[:, p, t * 512 + i * 128:t * 512 + (i + 1) * 128],
                    wall[:, WV + p * 256:WV + (p + 1) * 256],
                    start=(p == 0), stop=(p == NP - 1))
            if half == 1:
                if kt < 0:
                    nc.scalar.activation(vtall[:, kt, :, 0:64], pv[:, 0:256],
                                         CP)
                else:
                    nc.vector.tensor_copy(vtall[:, kt, :, 0:64], pv[:, 0:256])
                del vstate[kt]

        def loc(g):
            # head-pair-major block order: all head-group-0 blocks (for all
            # query tiles) run first, so group-1 projection prep moves out
            # of the overloaded qi=0 window into the mid-kernel surplus.
            m, j = divmod(g, 8)
            hp, r = divmod(m, 8)
            qi, hl = divmod(r, 2)
            return qi, 2 * hp + hl, j

        def emit_dots(g):
            qi, h, j = loc(g)
            ps = psp.tile([128, 2, 512], F32, name="ps", tag="ps")
            ps_tiles[g] = ps
            k0, k1 = 2 * j, 2 * j + 1
            nc.tensor.matmul(
                ps[:, 0, :], kcat[h][:, k0 * 128:(k0 + 1) * 128],
                qcat[h][:, qi * 512:(qi + 1) * 512], start=True, stop=True)
            nc.tensor.matmul(
                ps[:, 1, :], kcat[h][:, k1 * 128:(k1 + 1) * 128],
                qcat[h][:, qi * 512:(qi + 1) * 512], start=True, stop=True)

        def emit_av(g, pe):
            # fp8 DoubleRow: one matmul covers both key tiles of the pair
            # (K=256) at 0.5 cyc/row; a second 1-row matmul accumulates the
            # softmax denominator into po[64].
            # Transposed AV in bf16: the exp scores [keys, 128q] are the
            # full-width stationary, V plus a ones column (denominator) is
            # the 65-wide moving operand. Output [q, 64v|den] accumulates
            # over all 16 key tiles. Four query subtiles share one PSUM
            # bank: since PSUM zero-on-start is bank-granular, the bank is
            # pre-zeroed by a DVE memset (two blocks ahead) and every
            # matmul is a pure accumulate.
            qi, h, j = loc(g)
            po = po_tiles[(qi, h)]
            for i in (0, 1):
                for q in range(4):
                    nc.tensor.matmul(
                        po[:, q, :], pe[:, i, q * 128:(q + 1) * 128],
                        vtall[:, 2 * j + i, h, :],
                        start=False,
                        stop=(j == NKT // 2 - 1 and i == 1),
                        skip_group_check=True)

        def emit_norm(qi, h, g):
            # scale the V columns by 1/den per query row; the transposes
            # back to [d, n] layout are deferred two iterations so the PE
            # queue never blocks on this DVE chain.
            po = po_tiles.pop((qi, h))
            rb = wk.tile([128, 4], F32, name="rb", tag="rb")
            oc_t = wk.tile([128, 4, 64], F32R, name="oct", tag="oct")
            nc.vector.reciprocal(rb[:], po[:, :, 64])
            for q in range(4):
                nc.vector.tensor_scalar(
                    oc_t[:, q, :], po[:, q, 0:64], rb[:, q:q + 1], None,
                    op0=MULT)

            def flush(qi=qi, h=h, oc_t=oc_t):
                pt = pyp.tile([128, 512], F32, name="py", tag="py")
                for q in range(4):
                    nc.tensor.transpose(
                        pt[0:64, q * 128:(q + 1) * 128].bitcast(F32R),
                        oc_t[:, q, :], ident[:])
                oj, rr = h // 2, (h % 2) * 64
                nc.vector.tensor_copy(
                    ocat[oj][rr:rr + 64, qi * 512:(qi + 1) * 512],
                    pt[0:64, :].bitcast(F32R))
                m = g // 8 + 2
                if m < NT * H_PER_CORE:
                    alloc_po(m)
            trans_pending.setdefault(g + 2, []).append(flush)

        def alloc_po(m):
            # pre-allocate + zero block m's accumulator bank (reuses the
            # buffer its predecessor's norm just released)
            qi_m, h_m, _ = loc(8 * m)
            po = pop.tile([128, 4, 65], F32, name="po", tag="po")
            po_tiles[(qi_m, h_m)] = po
            nc.vector.memset(po[:], 0.0)

        def phasec_piece(qi, i, p):
            # out-projection for a 128-row output chunk, two matmul pieces.
            # For the last query block the ocat[1] half (heads 2,3) finishes
            # first (HPERM3), so it goes in the first matmul.
            n0 = qi * 512 + i * 128
            oa, ob = 0, 1
            if p == 0:
                ph_state[(qi, i)] = pyp.tile([128, 512], F32, name="py", tag="py")
                nc.tensor.matmul(ph_state[(qi, i)][:], ocat[oa][:, n0:n0 + 128],
                                 wo[:, oa * 512:oa * 512 + 512],
                                 start=True, stop=False)
            else:
                py = ph_state.pop((qi, i))
                nc.tensor.matmul(py[:], ocat[ob][:, n0:n0 + 128],
                                 wo[:, ob * 512:ob * 512 + 512],
                                 start=False, stop=True)
                st = wk.tile([128, 512], F32, name="st", tag="st", bufs=4)
                nc.vector.tensor_copy(st[:], py[:])
                nc.sync.dma_start(Y[n0:n0 + 128, :], st[:])

        # Static filler schedule: iteration -> emitters. PE is in-order, so
        # a block's pieces must all be emitted before the first dots that
        # reads its output (dots(g) is emitted at iter g-2), with ~2 iters
        # of margin for the clamp chain to land.
        sched = {}

        def at(g, *fs):
            sched.setdefault(g, []).extend(fs)

        # qi=0: V tiles (pair 2j,2j+1 before iter j), K for later t-blocks.
        at(0, lambda: v_piece(2, 0), lambda: v_piece(2, 1),
            lambda: v_piece(3, 0), lambda: v_piece(3, 1))
        at(1, lambda: v_piece(4, 0), lambda: v_piece(4, 1),
            lambda: v_piece(5, 0), lambda: v_piece(5, 1),
            lambda: kq_all(2, 0, 'k', 'act'))
        at(2, lambda: v_piece(6, 0), lambda: v_piece(6, 1),
            lambda: v_piece(7, 0), lambda: v_piece(7, 1))
        at(3, lambda: v_piece(8, 0), lambda: v_piece(8, 1),
            lambda: v_piece(9, 0), lambda: v_piece(9, 1),
            lambda: kq_all(3, 0, 'k', 'act'))
        at(4, lambda: v_piece(10, 0), lambda: v_piece(10, 1),
            lambda: v_piece(11, 0), lambda: v_piece(11, 1))
        at(5, lambda: v_piece(12, 0), lambda: v_piece(12, 1),
            lambda: v_piece(13, 0), lambda: v_piece(13, 1))
        at(6, lambda: v_piece(14, 0), lambda: v_piece(14, 1),
            lambda: v_piece(15, 0), lambda: v_piece(15, 1))
        # group-0 Q projections feed blocks (qi, h0/h1) at iters 16*qi
        def q_sched(base, t, g):
            for p, off in ((4, 0), (0, 1), (1, 2), (2, 3), (3, 4)):
                at(base + off, lambda t=t, p=p, g=g: kq_piece_k(t, g, 'q', p))

        q_sched(7, 1, 0)
        q_sched(18, 2, 0)
        q_sched(24, 3, 0)
        # group-1 K/Q projections: deadlines start at iter 62 (block q0,h2)
        KQ1 = [(33, 0, 'k'), (36, 1, 'k'), (39, 2, 'k'), (42, 3, 'k'),
               (45, 0, 'q')]
        for base, t, kind in KQ1:
            at(base, lambda t=t, kind=kind: kq_piece_k(t, 1, kind, 4),
               lambda t=t, kind=kind: kq_piece_k(t, 1, kind, 0))
            at(base + 1, lambda t=t, kind=kind: kq_piece_k(t, 1, kind, 1),
               lambda t=t, kind=kind: kq_piece_k(t, 1, kind, 2))
            at(base + 2, lambda t=t, kind=kind: kq_piece_k(t, 1, kind, 3))
        q_sched(55, 1, 1)
        q_sched(60, 2, 1)
        q_sched(65, 3, 1)
        # out-projection: query block qi complete after its (qi,h3) flush
        for qi in range(NT - 1):
            base = 84 + 16 * qi
            for i in range(4):
                at(base + 3 * i, lambda qi=qi, i=i: phasec_piece(qi, i, 0))
                at(base + 1 + 3 * i, lambda qi=qi, i=i: phasec_piece(qi, i, 1))

        # Prologue. The dummy exp preloads the ACT table during DMA wait.
        # The Q(0,0) block borrows the first dots PSUM buffer so all four
        # projection blocks run without pool-rotation WAR stalls; pieces are
        # ordered so the ones needing only the first half of the t0 DMA
        # start earliest.
        nc.vector.memset(vtall[:, :, :, 64], 1.0)   # denominator ones cols
        nc.vector.memset(ebias[:], EXP_BIAS)
        warm = wk.tile([128, 1], F32, name="warm", tag="warm")
        nc.scalar.activation(warm[:], ebias[:], EXP)
        # Q staging packs pixel/coord into the two banks of ONE dots-pool
        # tile, so the second tile is free for dots(0) with no
        # write-after-read stall on the Q clamps.
        qtile = psp.tile([128, 2, 512], F32, name="ps", tag="ps")
        # PE warm-up: dummy bf16 matmuls on a zeroed tile keep the PE busy
        # through the input-DMA wait so the p-state ramp (3us of continuous
        # execution) completes before the real projections start.
        pe_w = wk.tile([128, 2, 512], BF16, name="pe", tag="pe", bufs=8)
        nc.gpsimd.memset(pe_w[:], 0.0)
        for _ in range(12):
            nc.tensor.matmul(qtile[:, 0, 0:256], pe_w[:, 0, 0:128],
                             pe_w[:, 1, 0:256], start=True, stop=True)
        for p in (0, 1):
            kq_piece(0, 0, KP[0], KC[0], kcat, p, mode='prologue')
            kq_piece(0, 0, QP[0], QC[0], qcat, p, ps_ap=qtile[:, 0, :],
                     mode='prologue')
        kq_piece(0, 0, KP[0], KC[0], kcat, 4, mode='prologue')
        kq_piece(0, 0, QP[0], QC[0], qcat, 4, ps2_ap=qtile[:, 1, :],
                 mode='prologue')
        for p in (2, 3):
            kq_piece(0, 0, KP[0], KC[0], kcat, p, mode='prologue')
            kq_piece(0, 0, QP[0], QC[0], qcat, p, ps_ap=qtile[:, 0, :],
                     mode='prologue')
        emit_dots(0)
        emit_dots(1)
        kq_all(1, 0, 'k', mode='prologue')
        v_piece(0, 0)
        v_piece(0, 1)
        v_piece(1, 0)
        v_piece(1, 1)
        alloc_po(0)
        alloc_po(1)

        for g in range(NPAIR):
            qi, h, j = loc(g)
            pe = wk.tile([128, 2, 512], BF16, name="pe", tag="pe", bufs=8)
            nc.scalar.activation(pe[:], ps_tiles.pop(g)[:], EXP, scale=SCALE)
            if g + 2 < NPAIR:
                emit_dots(g + 2)
            emit_av(g, pe)
            if j == NKT // 2 - 1:
                emit_norm(qi, h, g)
            for f in trans_pending.pop(g, ()):
                f()
            for f in sched.get(g, ()):
                f()
        for fs in [trans_pending.pop(k) for k in sorted(trans_pending)]:
            for f in fs:
                f()
        for i in range(4):
            phasec_piece(NT - 1, i, 0)
            phasec_piece(NT - 1, i, 1)
    nc.compile()
    return nc


def _get_nc():
    global _NC
    if _NC is None:
        _NC = _build()
    return _NC


def _bf16(a):
    return np.ascontiguousarray(np.asarray(a, np.float32).astype(ml_dtypes.bfloat16))


def _pack_gcd(w):
    # [512, 256] -> [128, (g, c, d)]: head-group-major weight layout so the
    # first group's columns are one contiguous DMA.
    return (np.asarray(w, np.float32)
            .reshape(4, 128, 2, 128).transpose(1, 2, 0, 3).reshape(128, 1024))


def _pack_chd(w):
    # [512, 256] -> [128, (c, h, d)]: pd-chunk-major layout for the V weights.
    return (np.asarray(w, np.float32)
            .reshape(4, 128, 256).transpose(1, 0, 2).reshape(128, 1024))


def _pack_wo(w):
    w = np.asarray(w, dtype=np.float32)
    return np.ascontiguousarray(
        w.reshape(2, 128, 512).transpose(1, 0, 2).reshape(128, 1024))


def kernel(pixels, coords, mask, W_qkv, W_qkc, W_out, b_out):
    global LAST_EXEC_NS
    pixels = np.asarray(pixels, dtype=np.float32)
    coords = np.asarray(coords, dtype=np.float32)
    W_qkv = np.asarray(W_qkv, dtype=np.float32)
    W_qkc = np.asarray(W_qkc, dtype=np.float32)
    W_out = np.asarray(W_out, dtype=np.float32)
    b_out = np.asarray(b_out, dtype=np.float32)

    nc = _get_nc()

    XTg = [np.ascontiguousarray(
        np.concatenate([pixels[b].T.reshape(4, 128, N),
                        coords[b].T.reshape(1, 128, N)], axis=0)
        .transpose(1, 0, 2).astype(ml_dtypes.bfloat16)) for b in range(B)]

    in_maps = []
    for c in range(8):
        b = c // 2
        h0 = (c % 2) * H_PER_CORE * DH     # 0 or 256: col offset within split
        kp = _pack_gcd(W_qkv[:, ID + h0:ID + h0 + 256])
        qp = _pack_gcd(W_qkv[:, h0:h0 + 256])
        v = _pack_chd(W_qkv[:, 2 * ID + h0:2 * ID + h0 + 256])
        kc = np.asarray(W_qkc[:, ID + h0:ID + h0 + 256], np.float32)
        qc = np.asarray(W_qkc[:, h0:h0 + 256], np.float32)
        wall = np.concatenate([
            kp[:, 0:512], kc[:, 0:128], qp[:, 0:512], qc[:, 0:128], v,
            kp[:, 512:1024], kc[:, 128:256], qp[:, 512:1024], qc[:, 128:256],
        ], axis=1)
        in_maps.append({
            "XTg": XTg[b],
            "Wall": _bf16(wall),
            "Wo": _pack_wo(W_out[h0:h0 + 256, :]),
            "Ident": np.eye(128, dtype=np.float32),
        })

    res = run_bass_kernel_spmd(nc, in_maps, core_ids=list(range(8)))
    LAST_EXEC_NS = getattr(res, "exec_time_ns", None)

    out = np.empty((B, N, OUT_D), np.float32)
    for b in range(B):
        out[b] = res.results[2 * b]["Y"] + res.results[2 * b + 1]["Y"]
    out += b_out[None, None, :]
    return tuple(np.split(out, [1024], axis=1))


# revision 68
# speedup vs baseline: 1.2758x; 1.0078x over previous
import numpy as np
import ml_dtypes

import concourse.bacc as bacc
import concourse.tile as tile
import concourse.mybir as mybir
from concourse.bass_utils import run_bass_kernel_spmd

F32 = mybir.dt.float32
F32R = mybir.dt.float32r
BF16 = mybir.dt.bfloat16
F8E4 = mybir.dt.float8e4
F8E5 = mybir.dt.float8e5

B = 4
N = 2048
PD = 512
CD = 128
ID = 512
OUT_D = 512
H_PER_CORE = 4
DH = 64
SCALE = 0.125          # dim_head ** -0.5
NT = 4                 # n chunks of 512
NP = 4                 # pd chunks of 128
NKT = 16               # key tiles of 128
EXP_BIAS = -2.0        # softmax logits shifted so exp() fits fp8e5 range

# column offsets inside the packed weight block (bf16), head-group-major:
# [kp_g0 512][kc_g0 128][qp_g0 512][qc_g0 128][v 1024]
# [kp_g1 512][kc_g1 128][qp_g1 512][qc_g1 128]
KP = {0: 0, 1: 2304}
KC = {0: 512, 1: 2816}
QP = {0: 640, 1: 2944}
QC = {0: 1152, 1: 3456}
WV = 1280
WALL_COLS = 3584

_NC = None
LAST_EXEC_NS = None


def _build():
    nc = bacc.Bacc("TRN2", target_bir_lowering=False, debug=False, num_devices=8)
    XTg = nc.declare_dram_parameter("XTg", [128, 5, N], BF16, isOutput=False)
    Wall = nc.declare_dram_parameter("Wall", [128, WALL_COLS], BF16, isOutput=False)
    Wo = nc.declare_dram_parameter("Wo", [128, 1024], F32, isOutput=False)
    Ident = nc.declare_dram_parameter("Ident", [128, 128], F32, isOutput=False)
    Y = nc.declare_dram_parameter("Y", [N, OUT_D], F32, isOutput=True)

    MIN = mybir.AluOpType.min
    MAX = mybir.AluOpType.max
    MULT = mybir.AluOpType.mult
    EXP = mybir.ActivationFunctionType.Exp
    DR = mybir.MatmulPerfMode.DoubleRow

    with tile.TileContext(nc) as tc, \
         tc.tile_pool(name="persist", bufs=1) as pp, \
         tc.tile_pool(name="work", bufs=2) as wk, \
         tc.tile_pool(name="pb_s", bufs=2, space="PSUM") as psp, \
         tc.tile_pool(name="pb_o", bufs=2, space="PSUM") as pop, \
         tc.tile_pool(name="pc_y", bufs=2, space="PSUM") as pyp:
        xtall = pp.tile([128, 5, N], BF16, name="xtall", tag="xtall")
        wall = pp.tile([128, WALL_COLS], BF16, name="wall", tag="wall")
        wo = pp.tile([128, 1024], F32R, name="wo", tag="wo")
        ident = pp.tile([128, 128], F32R, name="ident", tag="ident")
        qcat = [pp.tile([128, N], F32R, name=f"qcat{h}", tag=f"qcat{h}")
                for h in range(H_PER_CORE)]
        kcat = [pp.tile([128, N], F32R, name=f"kcat{h}", tag=f"kcat{h}")
                for h in range(H_PER_CORE)]
        # per key tile: 4 heads x (64 V columns | 1 ones column for the
        # softmax denominator), bf16
        vtall = pp.tile([128, NKT, 4, 65], BF16, name="vtall", tag="vtall")
        ebias = pp.tile([128, 1], F32, name="ebias", tag="ebias")
        ocat = [pp.tile([128, N], F32R, name=f"ocat{j}", tag=f"ocat{j}")
                for j in range(2)]

        # Input DMAs. Queue A (sync) carries the critical stream in priority
        # order; queue B (gpsimd) slips the later weights into bus gaps.
        # Big packed transfers amortize the ~650ns per-DMA issue cost.
        def dma_xt_t(t, a, b):
            nc.sync.dma_start(xtall[:, a:b, t * 512:(t + 1) * 512],
                              XTg[:, a:b, t * 512:(t + 1) * 512])

        nc.sync.dma_start(wall[:, 0:1280], Wall[:, 0:1280])        # g0 weights
        dma_xt_t(0, 0, 2)
        dma_xt_t(0, 4, 5)
        dma_xt_t(0, 2, 4)
        nc.sync.dma_start(wall[:, WV:WV + 512], Wall[:, WV:WV + 512])
        dma_xt_t(1, 0, 5)
        nc.sync.dma_start(wall[:, WV + 512:WV + 1024],
                          Wall[:, WV + 512:WV + 1024])
        dma_xt_t(2, 0, 5)
        dma_xt_t(3, 0, 5)
        nc.sync.dma_start(ident[:], Ident[:].bitcast(F32R))
        nc.sync.dma_start(wall[:, 2304:3584], Wall[:, 2304:3584])  # g1
        nc.sync.dma_start(wo[:], Wo[:].bitcast(F32R))

        NPAIR = NT * H_PER_CORE * (NKT // 2)
        ps_tiles = {}
        po_tiles = {}
        qstate = {}
        vstate = {}
        ph_state = {}
        trans_pending = {}

        CP = mybir.ActivationFunctionType.Copy

        def emit_clamps(ps, dest, h0, row0, t, mode):
            # GPSIMD cannot read PSUM, so clamps from PSUM route through:
            # 'prologue' - even-head clamp direct on DVE (shortest chain to
            #   the first dots); odd-head rows staged via an ACT Copy (Copy
            #   shares the Exp table, and ACT idles here) and clamped on
            #   Pool. 'act' - both halves ACT-staged + Pool-clamped (for
            #   blocks landing in ACT stall windows). 'steady' - one DVE
            #   copy + two Pool clamps.
            def clamp(eng, jj, src, srow):
                eng.tensor_scalar(
                    dest[h0 + jj][row0:row0 + 64, t * 512:(t + 1) * 512],
                    src[srow:srow + 64, :], 5.0, -5.0, op0=MIN, op1=MAX)

            if mode == 'prologue':
                clamp(nc.vector, 0, ps, 0)
                scr = wk.tile([64, 512], F32, name="sc6", tag="sc6", bufs=4)
                nc.scalar.activation(scr[:], ps[64:128, :], CP)
                clamp(nc.gpsimd, 1, scr, 0)
            elif mode == 'act':
                scr = wk.tile([128, 512], F32, name="scr", tag="scr", bufs=3)
                nc.scalar.activation(scr[:], ps[:], CP)
                for jj in (0, 1):
                    clamp(nc.gpsimd, jj, scr, jj * 64)
            else:
                scr = wk.tile([128, 512], F32, name="scr", tag="scr", bufs=3)
                nc.vector.tensor_copy(scr[:], ps[:])
                for jj in (0, 1):
                    clamp(nc.gpsimd, jj, scr, jj * 64)

        def kq_piece(t, g, pbase, cbase, dest, p, ps_ap=None, ps2_ap=None,
                     mode='steady'):
            # p 4: coord matmul + clamps (emitted first: its clamp chain is
            # independent of the pixel pieces). p 0..3: pixel-chunk matmuls
            # accumulating over pd; p 3 also clamps. One clamp goes to DVE,
            # the other to Pool, halving the chain latency.
            key = (t, g, id(dest))
            if p < 4:
                if p == 0:
                    qstate[key] = (ps_ap if ps_ap is not None else
                                   pyp.tile([128, 512], F32, name="py", tag="py"))
                ps = qstate[key]
                nc.tensor.matmul(
                    ps[:], wall[:, pbase + p * 128:pbase + (p + 1) * 128],
                    xtall[:, p, t * 512:(t + 1) * 512],
                    start=(p == 0), stop=(p == 3))
                if p == 3:
                    emit_clamps(ps, dest, 2 * g, 0, t, mode)
                    del qstate[key]
            else:
                ps2 = (ps2_ap if ps2_ap is not None else
                       pyp.tile([128, 512], F32, name="py", tag="py"))
                nc.tensor.matmul(ps2[:], wall[:, cbase:cbase + 128],
                                 xtall[:, 4, t * 512:(t + 1) * 512],
                                 start=True, stop=True)
                emit_clamps(ps2, dest, 2 * g, 64, t, mode)

        def kq_all(t, g, kind, mode='steady'):
            for p in (4, 0, 1, 2, 3):
                kq_piece_k(t, g, kind, p, mode)

        def kq_piece_k(t, g, kind, p, mode='steady'):
            if kind == 'k':
                kq_piece(t, g, KP[g], KC[g], kcat, p, mode=mode)
            else:
                kq_piece(t, g, QP[g], QC[g], qcat, p, mode=mode)

        def v_piece(kt, half):
            # half 0: pd chunks 0,1; half 1: pd chunks 2,3 + one fp8 copy
            # moving all four heads' V columns out in a single instruction.
            t, i = divmod(kt, 4)
            if half == 0:
                vstate[kt] = pyp.tile([128, 512], F32, name="py", tag="py")
            pv = vstate[kt]
            for p in (0, 1) if half == 0 else (2, 3):
                nc.tensor.matmul(
                    pv[:, 0:256],
                    xtall[:, p, t * 512 + i * 128:t * 512 + (i + 1) * 128],
                    wall[:, WV + p * 256:WV + (p + 1) * 256],
                    start=(p == 0), stop=(p == NP - 1))
            if half == 1:
                if kt < 0:
                    nc.scalar.activation(vtall[:, kt, :, 0:64], pv[:, 0:256],
                                         CP)
                else:
                    nc.vector.tensor_copy(vtall[:, kt, :, 0:64], pv[:, 0:256])
                del vstate[kt]

        def loc(g):
            # head-pair-major block order: all head-group-0 blocks (for all
            # query tiles) run first, so group-1 projection prep moves out
            # of the overloaded qi=0 window into the mid-kernel surplus.
            m, j = divmod(g, 8)
            hp, r = divmod(m, 8)
            qi, hl = divmod(r, 2)
            return qi, 2 * hp + hl, j

        def emit_dots(g):
            qi, h, j = loc(g)
            ps = psp.tile([128, 2, 512], F32, name="ps", tag="ps")
            ps_tiles[g] = ps
            k0, k1 = 2 * j, 2 * j + 1
            nc.tensor.matmul(
                ps[:, 0, :], kcat[h][:, k0 * 128:(k0 + 1) * 128],
                qcat[h][:, qi * 512:(qi + 1) * 512], start=True, stop=True)
            nc.tensor.matmul(
                ps[:, 1, :], kcat[h][:, k1 * 128:(k1 + 1) * 128],
                qcat[h][:, qi * 512:(qi + 1) * 512], start=True, stop=True)

        def emit_av(g, pe):
            # fp8 DoubleRow: one matmul covers both key tiles of the pair
            # (K=256) at 0.5 cyc/row; a second 1-row matmul accumulates the
            # softmax denominator into po[64].
            # Transposed AV in bf16: the exp scores [keys, 128q] are the
            # full-width stationary, V plus a ones column (denominator) is
            # the 65-wide moving operand. Output [q, 64v|den] accumulates
            # over all 16 key tiles. Four query subtiles share one PSUM
            # bank: since PSUM zero-on-start is bank-granular, the bank is
            # pre-zeroed by a DVE memset (two blocks ahead) and every
            # matmul is a pure accumulate.
            qi, h, j = loc(g)
            po = po_tiles[(qi, h)]
            for i in (0, 1):
                for q in range(4):
                    nc.tensor.matmul(
                        po[:, q, :], pe[:, i, q * 128:(q + 1) * 128],
                        vtall[:, 2 * j + i, h, :],
                        start=False,
                        stop=(j == NKT // 2 - 1 and i == 1),
                        skip_group_check=True)

        def emit_norm(qi, h, g):
            # scale the V columns by 1/den per query row; the transposes
            # back to [d, n] layout are deferred two iterations so the PE
            # queue never blocks on this DVE chain.
            po = po_tiles.pop((qi, h))
            rb = wk.tile([128, 4, 1], F32, name="rb", tag="rb")
            oc_t = wk.tile([128, 4, 64], F32R, name="oct", tag="oct")
            nc.vector.reciprocal(rb[:], po[:, :, 64])
            nc.vector.tensor_tensor(
                oc_t[:, :, :], po[:, :, 0:64],
                rb[:].to_broadcast([128, 4, 64]), op=MULT)

            def flush(qi=qi, h=h, oc_t=oc_t):
                pt = pyp.tile([128, 512], F32, name="py", tag="py")
                for q in range(4):
                    nc.tensor.transpose(
                        pt[0:64, q * 128:(q + 1) * 128].bitcast(F32R),
                        oc_t[:, q, :], ident[:])
                oj, rr = h // 2, (h % 2) * 64
                eng = nc.scalar if (qi, h) == (NT - 1, 3) else None
                if eng is not None:
                    eng.activation(
                        ocat[oj][rr:rr + 64, qi * 512:(qi + 1) * 512],
                        pt[0:64, :].bitcast(F32R), CP)
                else:
                    nc.vector.tensor_copy(
                        ocat[oj][rr:rr + 64, qi * 512:(qi + 1) * 512],
                        pt[0:64, :].bitcast(F32R))
                m = g // 8 + 2
                if m < NT * H_PER_CORE:
                    alloc_po(m)
            trans_pending.setdefault(g + 2, []).append(flush)

        def alloc_po(m):
            # pre-allocate + zero block m's accumulator bank (reuses the
            # buffer its predecessor's norm just released)
            qi_m, h_m, _ = loc(8 * m)
            po = pop.tile([128, 4, 65], F32, name="po", tag="po")
            po_tiles[(qi_m, h_m)] = po
            nc.vector.memset(po[:], 0.0)

        def phasec_piece(qi, i, p):
            # out-projection for a 128-row output chunk, two matmul pieces.
            # For the last query block the ocat[1] half (heads 2,3) finishes
            # first (HPERM3), so it goes in the first matmul.
            n0 = qi * 512 + i * 128
            oa, ob = 0, 1
            if p == 0:
                ph_state[(qi, i)] = pyp.tile([128, 512], F32, name="py", tag="py")
                nc.tensor.matmul(ph_state[(qi, i)][:], ocat[oa][:, n0:n0 + 128],
                                 wo[:, oa * 512:oa * 512 + 512],
                                 start=True, stop=False)
            else:
                py = ph_state.pop((qi, i))
                nc.tensor.matmul(py[:], ocat[ob][:, n0:n0 + 128],
                                 wo[:, ob * 512:ob * 512 + 512],
                                 start=False, stop=True)
                st = wk.tile([128, 512], F32, name="st", tag="st", bufs=4)
                if qi == NT - 1 and i % 2 == 1:
                    # ACT idles after the last exp; let it stage half the
                    # final output chunks so the DMAs start sooner
                    nc.scalar.activation(st[:], py[:], CP)
                else:
                    nc.vector.tensor_copy(st[:], py[:])
                nc.sync.dma_start(Y[n0:n0 + 128, :], st[:])

        # Static filler schedule: iteration -> emitters. PE is in-order, so
        # a block's pieces must all be emitted before the first dots that
        # reads its output (dots(g) is emitted at iter g-2), with ~2 iters
        # of margin for the clamp chain to land.
        sched = {}

        def at(g, *fs):
            sched.setdefault(g, []).extend(fs)

        # qi=0: V tiles (pair 2j,2j+1 before iter j), K for later t-blocks.
        at(0, lambda: v_piece(2, 0), lambda: v_piece(2, 1),
            lambda: v_piece(3, 0), lambda: v_piece(3, 1))
        at(1, lambda: v_piece(4, 0), lambda: v_piece(4, 1),
            lambda: v_piece(5, 0), lambda: v_piece(5, 1),
            lambda: kq_all(2, 0, 'k', 'act'))
        at(2, lambda: v_piece(6, 0), lambda: v_piece(6, 1),
            lambda: v_piece(7, 0), lambda: v_piece(7, 1))
        at(3, lambda: v_piece(8, 0), lambda: v_piece(8, 1),
            lambda: v_piece(9, 0), lambda: v_piece(9, 1),
            lambda: kq_all(3, 0, 'k', 'act'))
        at(4, lambda: v_piece(10, 0), lambda: v_piece(10, 1),
            lambda: v_piece(11, 0), lambda: v_piece(11, 1))
        at(5, lambda: v_piece(12, 0), lambda: v_piece(12, 1),
            lambda: v_piece(13, 0), lambda: v_piece(13, 1))
        at(6, lambda: v_piece(14, 0), lambda: v_piece(14, 1),
            lambda: v_piece(15, 0), lambda: v_piece(15, 1))
        # group-0 Q projections feed blocks (qi, h0/h1) at iters 16*qi
        def q_sched(base, t, g):
            for p, off in ((4, 0), (0, 1), (1, 2), (2, 3), (3, 4)):
                at(base + off, lambda t=t, p=p, g=g: kq_piece_k(t, g, 'q', p))

        q_sched(7, 1, 0)
        q_sched(18, 2, 0)
        q_sched(24, 3, 0)
        # group-1 K/Q projections: deadlines start at iter 62 (block q0,h2)
        KQ1 = [(33, 0, 'k'), (36, 1, 'k'), (39, 2, 'k'), (42, 3, 'k'),
               (45, 0, 'q')]
        for base, t, kind in KQ1:
            at(base, lambda t=t, kind=kind: kq_piece_k(t, 1, kind, 4),
               lambda t=t, kind=kind: kq_piece_k(t, 1, kind, 0))
            at(base + 1, lambda t=t, kind=kind: kq_piece_k(t, 1, kind, 1),
               lambda t=t, kind=kind: kq_piece_k(t, 1, kind, 2))
            at(base + 2, lambda t=t, kind=kind: kq_piece_k(t, 1, kind, 3))
        q_sched(55, 1, 1)
        q_sched(60, 2, 1)
        q_sched(65, 3, 1)
        # out-projection: query block qi complete after its (qi,h3) flush
        for qi in range(NT - 1):
            base = 84 + 16 * qi
            for i in range(4):
                at(base + 3 * i, lambda qi=qi, i=i: phasec_piece(qi, i, 0))
                at(base + 1 + 3 * i, lambda qi=qi, i=i: phasec_piece(qi, i, 1))

        # Prologue. The dummy exp preloads the ACT table during DMA wait.
        # The Q(0,0) block borrows the first dots PSUM buffer so all four
        # projection blocks run without pool-rotation WAR stalls; pieces are
        # ordered so the ones needing only the first half of the t0 DMA
        # start earliest.
        nc.vector.memset(vtall[:, :, :, 64], 1.0)   # denominator ones cols
        nc.vector.memset(ebias[:], EXP_BIAS)
        warm = wk.tile([128, 1], F32, name="warm", tag="warm")
        nc.scalar.activation(warm[:], ebias[:], EXP)
        # Q staging packs pixel/coord into the two banks of ONE dots-pool
        # tile, so the second tile is free for dots(0) with no
        # write-after-read stall on the Q clamps.
        qtile = psp.tile([128, 2, 512], F32, name="ps", tag="ps")
        # PE warm-up: dummy bf16 matmuls on a zeroed tile keep the PE busy
        # through the input-DMA wait so the p-state ramp (3us of continuous
        # execution) completes before the real projections start.
        pe_w = wk.tile([128, 2, 512], BF16, name="pe", tag="pe", bufs=8)
        nc.gpsimd.memset(pe_w[:], 0.0)
        for _ in range(12):
            nc.tensor.matmul(qtile[:, 0, 0:256], pe_w[:, 0, 0:128],
                             pe_w[:, 1, 0:256], start=True, stop=True)
        for p in (0, 1):
            kq_piece(0, 0, KP[0], KC[0], kcat, p, mode='prologue')
            kq_piece(0, 0, QP[0], QC[0], qcat, p, ps_ap=qtile[:, 0, :],
                     mode='prologue')
        kq_piece(0, 0, KP[0], KC[0], kcat, 4, mode='prologue')
        kq_piece(0, 0, QP[0], QC[0], qcat, 4, ps2_ap=qtile[:, 1, :],
                 mode='prologue')
        for p in (2, 3):
            kq_piece(0, 0, KP[0], KC[0], kcat, p, mode='prologue')
            kq_piece(0, 0, QP[0], QC[0], qcat, p, ps_ap=qtile[:, 0, :],
                     mode='prologue')
        emit_dots(0)
        emit_dots(1)
        kq_all(1, 0, 'k', mode='prologue')
        v_piece(0, 0)
        v_piece(0, 1)
        v_piece(1, 0)
        v_piece(1, 1)
        alloc_po(0)
        alloc_po(1)

        for g in range(NPAIR):
            qi, h, j = loc(g)
            pe = wk.tile([128, 2, 512], BF16, name="pe", tag="pe", bufs=8)
            nc.scalar.activation(pe[:], ps_tiles.pop(g)[:], EXP, scale=SCALE)
            if g + 2 < NPAIR:
                emit_dots(g + 2)
            emit_av(g, pe)
            if j == NKT // 2 - 1:
                emit_norm(qi, h, g)
            for f in trans_pending.pop(g, ()):
                f()
            for f in sched.get(g, ()):
                f()
        for fs in [trans_pending.pop(k) for k in sorted(trans_pending)]:
            for f in fs:
                f()
        for i in range(4):
            phasec_piece(NT - 1, i, 0)
            phasec_piece(NT - 1, i, 1)
    nc.compile()
    return nc


def _get_nc():
    global _NC
    if _NC is None:
        _NC = _build()
    return _NC


def _bf16(a):
    return np.ascontiguousarray(np.asarray(a, np.float32).astype(ml_dtypes.bfloat16))


def _pack_gcd(w):
    # [512, 256] -> [128, (g, c, d)]: head-group-major weight layout so the
    # first group's columns are one contiguous DMA.
    return (np.asarray(w, np.float32)
            .reshape(4, 128, 2, 128).transpose(1, 2, 0, 3).reshape(128, 1024))


def _pack_chd(w):
    # [512, 256] -> [128, (c, h, d)]: pd-chunk-major layout for the V weights.
    return (np.asarray(w, np.float32)
            .reshape(4, 128, 256).transpose(1, 0, 2).reshape(128, 1024))


def _pack_wo(w):
    w = np.asarray(w, dtype=np.float32)
    return np.ascontiguousarray(
        w.reshape(2, 128, 512).transpose(1, 0, 2).reshape(128, 1024))


def kernel(pixels, coords, mask, W_qkv, W_qkc, W_out, b_out):
    global LAST_EXEC_NS
    pixels = np.asarray(pixels, dtype=np.float32)
    coords = np.asarray(coords, dtype=np.float32)
    W_qkv = np.asarray(W_qkv, dtype=np.float32)
    W_qkc = np.asarray(W_qkc, dtype=np.float32)
    W_out = np.asarray(W_out, dtype=np.float32)
    b_out = np.asarray(b_out, dtype=np.float32)

    nc = _get_nc()

    XTg = [np.ascontiguousarray(
        np.concatenate([pixels[b].T.reshape(4, 128, N),
                        coords[b].T.reshape(1, 128, N)], axis=0)
        .transpose(1, 0, 2).astype(ml_dtypes.bfloat16)) for b in range(B)]

    in_maps = []
    for c in range(8):
        b = c // 2
        h0 = (c % 2) * H_PER_CORE * DH     # 0 or 256: col offset within split
        kp = _pack_gcd(W_qkv[:, ID + h0:ID + h0 + 256])
        qp = _pack_gcd(W_qkv[:, h0:h0 + 256])
        v = _pack_chd(W_qkv[:, 2 * ID + h0:2 * ID + h0 + 256])
        kc = np.asarray(W_qkc[:, ID + h0:ID + h0 + 256], np.float32)
        qc = np.asarray(W_qkc[:, h0:h0 + 256], np.float32)
        wall = np.concatenate([
            kp[:, 0:512], kc[:, 0:128], qp[:, 0:512], qc[:, 0:128], v,
            kp[:, 512:1024], kc[:, 128:256], qp[:, 512:1024], qc[:, 128:256],
        ], axis=1)
        in_maps.append({
            "XTg": XTg[b],
            "Wall": _bf16(wall),
            "Wo": _pack_wo(W_out[h0:h0 + 256, :]),
            "Ident": np.eye(128, dtype=np.float32),
        })

    res = run_bass_kernel_spmd(nc, in_maps, core_ids=list(range(8)))
    LAST_EXEC_NS = getattr(res, "exec_time_ns", None)

    out = np.empty((B, N, OUT_D), np.float32)
    for b in range(B):
        out[b] = res.results[2 * b]["Y"] + res.results[2 * b + 1]["Y"]
    out += b_out[None, None, :]
    return tuple(np.split(out, [1024], axis=1))


# revision 71
# speedup vs baseline: 1.2763x; 1.0004x over previous
import numpy as np
import ml_dtypes

import concourse.bacc as bacc
import concourse.tile as tile
import concourse.mybir as mybir
from concourse.bass_utils import run_bass_kernel_spmd

F32 = mybir.dt.float32
F32R = mybir.dt.float32r
BF16 = mybir.dt.bfloat16
F8E4 = mybir.dt.float8e4
F8E5 = mybir.dt.float8e5

B = 4
N = 2048
PD = 512
CD = 128
ID = 512
OUT_D = 512
H_PER_CORE = 4
DH = 64
SCALE = 0.125          # dim_head ** -0.5
NT = 4                 # n chunks of 512
NP = 4                 # pd chunks of 128
NKT = 16               # key tiles of 128
EXP_BIAS = -2.0        # softmax logits shifted so exp() fits fp8e5 range

# column offsets inside the packed weight block (bf16), head-group-major:
# [kp_g0 512][kc_g0 128][qp_g0 512][qc_g0 128][v 1024]
# [kp_g1 512][kc_g1 128][qp_g1 512][qc_g1 128]
KP = {0: 0, 1: 2304}
KC = {0: 512, 1: 2816}
QP = {0: 640, 1: 2944}
QC = {0: 1152, 1: 3456}
WV = 1280
WALL_COLS = 3584

_NC = None
LAST_EXEC_NS = None


def _build():
    nc = bacc.Bacc("TRN2", target_bir_lowering=False, debug=False, num_devices=8)
    XTg = nc.declare_dram_parameter("XTg", [128, 5, N], BF16, isOutput=False)
    Wall = nc.declare_dram_parameter("Wall", [128, WALL_COLS], BF16, isOutput=False)
    Wo = nc.declare_dram_parameter("Wo", [128, 1024], F32, isOutput=False)
    Ident = nc.declare_dram_parameter("Ident", [128, 128], F32, isOutput=False)
    Y = nc.declare_dram_parameter("Y", [N, OUT_D], F32, isOutput=True)

    MIN = mybir.AluOpType.min
    MAX = mybir.AluOpType.max
    MULT = mybir.AluOpType.mult
    EXP = mybir.ActivationFunctionType.Exp
    DR = mybir.MatmulPerfMode.DoubleRow

    with tile.TileContext(nc) as tc, \
         tc.tile_pool(name="persist", bufs=1) as pp, \
         tc.tile_pool(name="work", bufs=2) as wk, \
         tc.tile_pool(name="pb_s", bufs=2, space="PSUM") as psp, \
         tc.tile_pool(name="pb_o", bufs=2, space="PSUM") as pop, \
         tc.tile_pool(name="pc_y", bufs=2, space="PSUM") as pyp:
        xtall = pp.tile([128, 5, N], BF16, name="xtall", tag="xtall")
        wall = pp.tile([128, WALL_COLS], BF16, name="wall", tag="wall")
        wo = pp.tile([128, 1024], F32R, name="wo", tag="wo")
        ident = pp.tile([128, 128], F32R, name="ident", tag="ident")
        qcat = [pp.tile([128, N], F32R, name=f"qcat{h}", tag=f"qcat{h}")
                for h in range(H_PER_CORE)]
        kcat = [pp.tile([128, N], F32R, name=f"kcat{h}", tag=f"kcat{h}")
                for h in range(H_PER_CORE)]
        # per key tile: 4 heads x (64 V columns | 1 ones column for the
        # softmax denominator), bf16
        vtall = pp.tile([128, NKT, 4, 65], BF16, name="vtall", tag="vtall")
        ebias = pp.tile([128, 1], F32, name="ebias", tag="ebias")
        ocat = [pp.tile([128, N], F32R, name=f"ocat{j}", tag=f"ocat{j}")
                for j in range(2)]

        # Input DMAs. Queue A (sync) carries the critical stream in priority
        # order; queue B (gpsimd) slips the later weights into bus gaps.
        # Big packed transfers amortize the ~650ns per-DMA issue cost.
        def dma_xt_t(t, a, b):
            nc.sync.dma_start(xtall[:, a:b, t * 512:(t + 1) * 512],
                              XTg[:, a:b, t * 512:(t + 1) * 512])

        nc.sync.dma_start(wall[:, 0:1280], Wall[:, 0:1280])        # g0 weights
        dma_xt_t(0, 0, 2)
        dma_xt_t(0, 4, 5)
        dma_xt_t(0, 2, 4)
        nc.sync.dma_start(wall[:, WV:WV + 512], Wall[:, WV:WV + 512])
        dma_xt_t(1, 0, 5)
        nc.sync.dma_start(wall[:, WV + 512:WV + 1024],
                          Wall[:, WV + 512:WV + 1024])
        dma_xt_t(2, 0, 5)
        dma_xt_t(3, 0, 5)
        nc.sync.dma_start(ident[:], Ident[:].bitcast(F32R))
        nc.sync.dma_start(wall[:, 2304:3584], Wall[:, 2304:3584])  # g1
        nc.sync.dma_start(wo[:], Wo[:].bitcast(F32R))

        NPAIR = NT * H_PER_CORE * (NKT // 2)
        ps_tiles = {}
        po_tiles = {}
        qstate = {}
        vstate = {}
        ph_state = {}
        trans_pending = {}

        CP = mybir.ActivationFunctionType.Copy

        def emit_clamps(ps, dest, h0, row0, t, mode):
            # GPSIMD cannot read PSUM, so clamps from PSUM route through:
            # 'prologue' - even-head clamp direct on DVE (shortest chain to
            #   the first dots); odd-head rows staged via an ACT Copy (Copy
            #   shares the Exp table, and ACT idles here) and clamped on
            #   Pool. 'act' - both halves ACT-staged + Pool-clamped (for
            #   blocks landing in ACT stall windows). 'steady' - one DVE
            #   copy + two Pool clamps.
            def clamp(eng, jj, src, srow):
                eng.tensor_scalar(
                    dest[h0 + jj][row0:row0 + 64, t * 512:(t + 1) * 512],
                    src[srow:srow + 64, :], 5.0, -5.0, op0=MIN, op1=MAX)

            if mode == 'prologue':
                clamp(nc.vector, 0, ps, 0)
                scr = wk.tile([64, 512], F32, name="sc6", tag="sc6", bufs=4)
                nc.scalar.activation(scr[:], ps[64:128, :], CP)
                clamp(nc.gpsimd, 1, scr, 0)
            elif mode == 'act':
                scr = wk.tile([128, 512], F32, name="scr", tag="scr", bufs=3)
                nc.scalar.activation(scr[:], ps[:], CP)
                for jj in (0, 1):
                    clamp(nc.gpsimd, jj, scr, jj * 64)
            else:
                scr = wk.tile([128, 512], F32, name="scr", tag="scr", bufs=3)
                nc.vector.tensor_copy(scr[:], ps[:])
                for jj in (0, 1):
                    clamp(nc.gpsimd, jj, scr, jj * 64)

        def kq_piece(t, g, pbase, cbase, dest, p, ps_ap=None, ps2_ap=None,
                     mode='steady'):
            # p 4: coord matmul + clamps (emitted first: its clamp chain is
            # independent of the pixel pieces). p 0..3: pixel-chunk matmuls
            # accumulating over pd; p 3 also clamps. One clamp goes to DVE,
            # the other to Pool, halving the chain latency.
            key = (t, g, id(dest))
            if p < 4:
                if p == 0:
                    qstate[key] = (ps_ap if ps_ap is not None else
                                   pyp.tile([128, 512], F32, name="py", tag="py"))
                ps = qstate[key]
                nc.tensor.matmul(
                    ps[:], wall[:, pbase + p * 128:pbase + (p + 1) * 128],
                    xtall[:, p, t * 512:(t + 1) * 512],
                    start=(p == 0), stop=(p == 3))
                if p == 3:
                    emit_clamps(ps, dest, 2 * g, 0, t, mode)
                    del qstate[key]
            else:
                ps2 = (ps2_ap if ps2_ap is not None else
                       pyp.tile([128, 512], F32, name="py", tag="py"))
                nc.tensor.matmul(ps2[:], wall[:, cbase:cbase + 128],
                                 xtall[:, 4, t * 512:(t + 1) * 512],
                                 start=True, stop=True)
                emit_clamps(ps2, dest, 2 * g, 64, t, mode)

        def kq_all(t, g, kind, mode='steady'):
            for p in (4, 0, 1, 2, 3):
                kq_piece_k(t, g, kind, p, mode)

        def kq_piece_k(t, g, kind, p, mode='steady'):
            if kind == 'k':
                kq_piece(t, g, KP[g], KC[g], kcat, p, mode=mode)
            else:
                kq_piece(t, g, QP[g], QC[g], qcat, p, mode=mode)

        def v_piece(kt, half):
            # half 0: pd chunks 0,1; half 1: pd chunks 2,3 + one fp8 copy
            # moving all four heads' V columns out in a single instruction.
            t, i = divmod(kt, 4)
            if half == 0:
                vstate[kt] = pyp.tile([128, 512], F32, name="py", tag="py")
            pv = vstate[kt]
            for p in (0, 1) if half == 0 else (2, 3):
                nc.tensor.matmul(
                    pv[:, 0:256],
                    xtall[:, p, t * 512 + i * 128:t * 512 + (i + 1) * 128],
                    wall[:, WV + p * 256:WV + (p + 1) * 256],
                    start=(p == 0), stop=(p == NP - 1))
            if half == 1:
                if kt < 0:
                    nc.scalar.activation(vtall[:, kt, :, 0:64], pv[:, 0:256],
                                         CP)
                else:
                    nc.vector.tensor_copy(vtall[:, kt, :, 0:64], pv[:, 0:256])
                del vstate[kt]

        def loc(g):
            # head-pair-major block order: all head-group-0 blocks (for all
            # query tiles) run first, so group-1 projection prep moves out
            # of the overloaded qi=0 window into the mid-kernel surplus.
            m, j = divmod(g, 8)
            hp, r = divmod(m, 8)
            qi, hl = divmod(r, 2)
            return qi, 2 * hp + hl, j

        def emit_dots(g):
            qi, h, j = loc(g)
            ps = psp.tile([128, 2, 512], F32, name="ps", tag="ps")
            ps_tiles[g] = ps
            k0, k1 = 2 * j, 2 * j + 1
            nc.tensor.matmul(
                ps[:, 0, :], kcat[h][:, k0 * 128:(k0 + 1) * 128],
                qcat[h][:, qi * 512:(qi + 1) * 512], start=True, stop=True)
            nc.tensor.matmul(
                ps[:, 1, :], kcat[h][:, k1 * 128:(k1 + 1) * 128],
                qcat[h][:, qi * 512:(qi + 1) * 512], start=True, stop=True)

        def emit_av(g, pe):
            # fp8 DoubleRow: one matmul covers both key tiles of the pair
            # (K=256) at 0.5 cyc/row; a second 1-row matmul accumulates the
            # softmax denominator into po[64].
            # Transposed AV in bf16: the exp scores [keys, 128q] are the
            # full-width stationary, V plus a ones column (denominator) is
            # the 65-wide moving operand. Output [q, 64v|den] accumulates
            # over all 16 key tiles. Four query subtiles share one PSUM
            # bank: since PSUM zero-on-start is bank-granular, the bank is
            # pre-zeroed by a DVE memset (two blocks ahead) and every
            # matmul is a pure accumulate.
            qi, h, j = loc(g)
            po = po_tiles[(qi, h)]
            for i in (0, 1):
                for q in range(4):
                    nc.tensor.matmul(
                        po[:, q, :], pe[:, i, q * 128:(q + 1) * 128],
                        vtall[:, 2 * j + i, h, :],
                        start=False,
                        stop=(j == NKT // 2 - 1 and i == 1),
                        skip_group_check=True)

        def emit_norm(qi, h, g):
            # scale the V columns by 1/den per query row; the transposes
            # back to [d, n] layout are deferred two iterations so the PE
            # queue never blocks on this DVE chain.
            po = po_tiles.pop((qi, h))
            rb = wk.tile([128, 4, 1], F32, name="rb", tag="rb")
            oc_t = wk.tile([128, 4, 64], F32R, name="oct", tag="oct")
            nc.vector.reciprocal(rb[:], po[:, :, 64])
            nc.vector.tensor_tensor(
                oc_t[:, :, :], po[:, :, 0:64],
                rb[:].to_broadcast([128, 4, 64]), op=MULT)

            def flush(qi=qi, h=h, oc_t=oc_t):
                pt = pyp.tile([128, 512], F32, name="py", tag="py")
                for q in range(4):
                    nc.tensor.transpose(
                        pt[0:64, q * 128:(q + 1) * 128].bitcast(F32R),
                        oc_t[:, q, :], ident[:])
                oj, rr = h // 2, (h % 2) * 64
                eng = nc.scalar if (qi, h) == (NT - 1, 3) else None
                if eng is not None:
                    eng.activation(
                        ocat[oj][rr:rr + 64, qi * 512:(qi + 1) * 512],
                        pt[0:64, :].bitcast(F32R), CP)
                else:
                    nc.vector.tensor_copy(
                        ocat[oj][rr:rr + 64, qi * 512:(qi + 1) * 512],
                        pt[0:64, :].bitcast(F32R))
                m = g // 8 + 2
                if m < NT * H_PER_CORE:
                    alloc_po(m)
            trans_pending.setdefault(g + 2, []).append(flush)

        def alloc_po(m):
            # pre-allocate + zero block m's accumulator bank (reuses the
            # buffer its predecessor's norm just released)
            qi_m, h_m, _ = loc(8 * m)
            po = pop.tile([128, 4, 65], F32, name="po", tag="po")
            po_tiles[(qi_m, h_m)] = po
            nc.vector.memset(po[:], 0.0)

        def phasec_piece(qi, i, p):
            # out-projection for a 128-row output chunk, two matmul pieces.
            # For the last query block the ocat[1] half (heads 2,3) finishes
            # first (HPERM3), so it goes in the first matmul.
            n0 = qi * 512 + i * 128
            oa, ob = 0, 1
            if p == 0:
                ph_state[(qi, i)] = pyp.tile([128, 512], F32, name="py", tag="py")
                nc.tensor.matmul(ph_state[(qi, i)][:], ocat[oa][:, n0:n0 + 128],
                                 wo[:, oa * 512:oa * 512 + 512],
                                 start=True, stop=False)
            else:
                py = ph_state.pop((qi, i))
                nc.tensor.matmul(py[:], ocat[ob][:, n0:n0 + 128],
                                 wo[:, ob * 512:ob * 512 + 512],
                                 start=False, stop=True)
                st = wk.tile([128, 512], F32, name="st", tag="st", bufs=4)
                if qi == NT - 1 and i % 2 == 1:
                    # ACT idles after the last exp; let it stage half the
                    # final output chunks so the DMAs start sooner
                    nc.scalar.activation(st[:], py[:], CP)
                else:
                    nc.vector.tensor_copy(st[:], py[:])
                nc.sync.dma_start(Y[n0:n0 + 128, :], st[:])

        # Static filler schedule: iteration -> emitters. PE is in-order, so
        # a block's pieces must all be emitted before the first dots that
        # reads its output (dots(g) is emitted at iter g-2), with ~2 iters
        # of margin for the clamp chain to land.
        sched = {}

        def at(g, *fs):
            sched.setdefault(g, []).extend(fs)

        # qi=0: V tiles (pair 2j,2j+1 before iter j), K for later t-blocks.
        at(0, lambda: v_piece(2, 0), lambda: v_piece(2, 1),
            lambda: v_piece(3, 0), lambda: v_piece(3, 1))
        at(1, lambda: v_piece(4, 0), lambda: v_piece(4, 1),
            lambda: v_piece(5, 0), lambda: v_piece(5, 1),
            lambda: kq_all(2, 0, 'k'))
        at(2, lambda: v_piece(6, 0), lambda: v_piece(6, 1),
            lambda: v_piece(7, 0), lambda: v_piece(7, 1))
        at(3, lambda: v_piece(8, 0), lambda: v_piece(8, 1),
            lambda: v_piece(9, 0), lambda: v_piece(9, 1),
            lambda: kq_all(3, 0, 'k'))
        at(4, lambda: v_piece(10, 0), lambda: v_piece(10, 1),
            lambda: v_piece(11, 0), lambda: v_piece(11, 1))
        at(5, lambda: v_piece(12, 0), lambda: v_piece(12, 1),
            lambda: v_piece(13, 0), lambda: v_piece(13, 1))
        at(6, lambda: v_piece(14, 0), lambda: v_piece(14, 1),
            lambda: v_piece(15, 0), lambda: v_piece(15, 1))
        # group-0 Q projections feed blocks (qi, h0/h1) at iters 16*qi
        def q_sched(base, t, g):
            for p, off in ((4, 0), (0, 1), (1, 2), (2, 3), (3, 4)):
                at(base + off, lambda t=t, p=p, g=g: kq_piece_k(t, g, 'q', p))

        q_sched(7, 1, 0)
        q_sched(18, 2, 0)
        q_sched(24, 3, 0)
        # group-1 K/Q projections: deadlines start at iter 62 (block q0,h2)
        KQ1 = [(33, 0, 'k'), (36, 1, 'k'), (39, 2, 'k'), (42, 3, 'k'),
               (45, 0, 'q')]
        for base, t, kind in KQ1:
            at(base, lambda t=t, kind=kind: kq_piece_k(t, 1, kind, 4),
               lambda t=t, kind=kind: kq_piece_k(t, 1, kind, 0))
            at(base + 1, lambda t=t, kind=kind: kq_piece_k(t, 1, kind, 1),
               lambda t=t, kind=kind: kq_piece_k(t, 1, kind, 2))
            at(base + 2, lambda t=t, kind=kind: kq_piece_k(t, 1, kind, 3))
        q_sched(55, 1, 1)
        q_sched(60, 2, 1)
        q_sched(65, 3, 1)
        # out-projection: query block qi complete after its (qi,h3) flush
        for qi in range(NT - 1):
            base = 84 + 16 * qi
            for i in range(4):
                at(base + 3 * i, lambda qi=qi, i=i: phasec_piece(qi, i, 0))
                at(base + 1 + 3 * i, lambda qi=qi, i=i: phasec_piece(qi, i, 1))

        # Prologue. The dummy exp preloads the ACT table during DMA wait.
        # The Q(0,0) block borrows the first dots PSUM buffer so all four
        # projection blocks run without pool-rotation WAR stalls; pieces are
        # ordered so the ones needing only the first half of the t0 DMA
        # start earliest.
        nc.vector.memset(vtall[:, :, :, 64], 1.0)   # denominator ones cols
        nc.vector.memset(ebias[:], EXP_BIAS)
        warm = wk.tile([128, 1], F32, name="warm", tag="warm")
        nc.scalar.activation(warm[:], ebias[:], EXP)
        # Q staging packs pixel/coord into the two banks of ONE dots-pool
        # tile, so the second tile is free for dots(0) with no
        # write-after-read stall on the Q clamps.
        qtile = psp.tile([128, 2, 512], F32, name="ps", tag="ps")
        # PE warm-up: dummy bf16 matmuls on a zeroed tile keep the PE busy
        # through the input-DMA wait so the p-state ramp (3us of continuous
        # execution) completes before the real projections start.
        pe_w = wk.tile([128, 2, 512], BF16, name="pe", tag="pe", bufs=8)
        nc.gpsimd.memset(pe_w[:], 0.0)
        for _ in range(12):
            nc.tensor.matmul(qtile[:, 0, 0:256], pe_w[:, 0, 0:128],
                             pe_w[:, 1, 0:256], start=True, stop=True)
        for p in (0, 1):
            kq_piece(0, 0, KP[0], KC[0], kcat, p, mode='prologue')
            kq_piece(0, 0, QP[0], QC[0], qcat, p, ps_ap=qtile[:, 0, :],
                     mode='prologue')
        kq_piece(0, 0, KP[0], KC[0], kcat, 4, mode='prologue')
        kq_piece(0, 0, QP[0], QC[0], qcat, 4, ps2_ap=qtile[:, 1, :],
                 mode='prologue')
        for p in (2, 3):
            kq_piece(0, 0, KP[0], KC[0], kcat, p, mode='prologue')
            kq_piece(0, 0, QP[0], QC[0], qcat, p, ps_ap=qtile[:, 0, :],
                     mode='prologue')
        emit_dots(0)
        emit_dots(1)
        kq_all(1, 0, 'k', mode='prologue')
        v_piece(0, 0)
        v_piece(0, 1)
        v_piece(1, 0)
        v_piece(1, 1)
        alloc_po(0)
        alloc_po(1)

        for g in range(NPAIR):
            qi, h, j = loc(g)
            pe = wk.tile([128, 2, 512], BF16, name="pe", tag="pe", bufs=8)
            nc.scalar.activation(pe[:], ps_tiles.pop(g)[:], EXP, scale=SCALE)
            if g + 2 < NPAIR:
                emit_dots(g + 2)
            emit_av(g, pe)
            if j == NKT // 2 - 1:
                emit_norm(qi, h, g)
            for f in trans_pending.pop(g, ()):
                f()
            for f in sched.get(g, ()):
                f()
        for fs in [trans_pending.pop(k) for k in sorted(trans_pending)]:
            for f in fs:
                f()
        for i in range(4):
            phasec_piece(NT - 1, i, 0)
            phasec_piece(NT - 1, i, 1)
    nc.compile()
    return nc


def _get_nc():
    global _NC
    if _NC is None:
        _NC = _build()
    return _NC


def _bf16(a):
    return np.ascontiguousarray(np.asarray(a, np.float32).astype(ml_dtypes.bfloat16))


def _pack_gcd(w):
    # [512, 256] -> [128, (g, c, d)]: head-group-major weight layout so the
    # first group's columns are one contiguous DMA.
    return (np.asarray(w, np.float32)
            .reshape(4, 128, 2, 128).transpose(1, 2, 0, 3).reshape(128, 1024))


def _pack_chd(w):
    # [512, 256] -> [128, (c, h, d)]: pd-chunk-major layout for the V weights.
    return (np.asarray(w, np.float32)
            .reshape(4, 128, 256).transpose(1, 0, 2).reshape(128, 1024))


def _pack_wo(w):
    w = np.asarray(w, dtype=np.float32)
    return np.ascontiguousarray(
        w.reshape(2, 128, 512).transpose(1, 0, 2).reshape(128, 1024))


def kernel(pixels, coords, mask, W_qkv, W_qkc, W_out, b_out):
    global LAST_EXEC_NS
    pixels = np.asarray(pixels, dtype=np.float32)
    coords = np.asarray(coords, dtype=np.float32)
    W_qkv = np.asarray(W_qkv, dtype=np.float32)
    W_qkc = np.asarray(W_qkc, dtype=np.float32)
    W_out = np.asarray(W_out, dtype=np.float32)
    b_out = np.asarray(b_out, dtype=np.float32)

    nc = _get_nc()

    XTg = [np.ascontiguousarray(
        np.concatenate([pixels[b].T.reshape(4, 128, N),
                        coords[b].T.reshape(1, 128, N)], axis=0)
        .transpose(1, 0, 2).astype(ml_dtypes.bfloat16)) for b in range(B)]

    in_maps = []
    for c in range(8):
        b = c // 2
        h0 = (c % 2) * H_PER_CORE * DH     # 0 or 256: col offset within split
        kp = _pack_gcd(W_qkv[:, ID + h0:ID + h0 + 256])
        qp = _pack_gcd(W_qkv[:, h0:h0 + 256])
        v = _pack_chd(W_qkv[:, 2 * ID + h0:2 * ID + h0 + 256])
        kc = np.asarray(W_qkc[:, ID + h0:ID + h0 + 256], np.float32)
        qc = np.asarray(W_qkc[:, h0:h0 + 256], np.float32)
        wall = np.concatenate([
            kp[:, 0:512], kc[:, 0:128], qp[:, 0:512], qc[:, 0:128], v,
            kp[:, 512:1024], kc[:, 128:256], qp[:, 512:1024], qc[:, 128:256],
        ], axis=1)
        in_maps.append({
            "XTg": XTg[b],
            "Wall": _bf16(wall),
            "Wo": _pack_wo(W_out[h0:h0 + 256, :]),
            "Ident": np.eye(128, dtype=np.float32),
        })

    res = run_bass_kernel_spmd(nc, in_maps, core_ids=list(range(8)))
    LAST_EXEC_NS = getattr(res, "exec_time_ns", None)

    out = np.empty((B, N, OUT_D), np.float32)
    for b in range(B):
        out[b] = res.results[2 * b]["Y"] + res.results[2 * b + 1]["Y"]
    out += b_out[None, None, :]
    return tuple(np.split(out, [1024], axis=1))


# revision 75
# speedup vs baseline: 1.2853x; 1.0070x over previous
import numpy as np
import ml_dtypes

import concourse.bacc as bacc
import concourse.tile as tile
import concourse.mybir as mybir
from concourse.bass_utils import run_bass_kernel_spmd

F32 = mybir.dt.float32
F32R = mybir.dt.float32r
BF16 = mybir.dt.bfloat16
F8E4 = mybir.dt.float8e4
F8E5 = mybir.dt.float8e5

B = 4
N = 2048
PD = 512
CD = 128
ID = 512
OUT_D = 512
H_PER_CORE = 4
DH = 64
SCALE = 0.125          # dim_head ** -0.5
NT = 4                 # n chunks of 512
NP = 4                 # pd chunks of 128
NKT = 16               # key tiles of 128
EXP_BIAS = -2.0        # softmax logits shifted so exp() fits fp8e5 range

# column offsets inside the packed weight block (bf16), head-group-major:
# [kp_g0 512][kc_g0 128][qp_g0 512][qc_g0 128][v 1024]
# [kp_g1 512][kc_g1 128][qp_g1 512][qc_g1 128]
KP = {0: 0, 1: 2304}
KC = {0: 512, 1: 2816}
QP = {0: 640, 1: 2944}
QC = {0: 1152, 1: 3456}
WV = 1280
WALL_COLS = 3584

_NC = None
LAST_EXEC_NS = None


def _build():
    nc = bacc.Bacc("TRN2", target_bir_lowering=False, debug=False, num_devices=8)
    XTg = nc.declare_dram_parameter("XTg", [128, 5, N], BF16, isOutput=False)
    Wall = nc.declare_dram_parameter("Wall", [128, WALL_COLS], BF16, isOutput=False)
    Wo = nc.declare_dram_parameter("Wo", [128, 1024], F32, isOutput=False)
    Ident = nc.declare_dram_parameter("Ident", [128, 128], F32, isOutput=False)
    Y = nc.declare_dram_parameter("Y", [N, OUT_D], F32, isOutput=True)

    MIN = mybir.AluOpType.min
    MAX = mybir.AluOpType.max
    MULT = mybir.AluOpType.mult
    EXP = mybir.ActivationFunctionType.Exp
    DR = mybir.MatmulPerfMode.DoubleRow

    with tile.TileContext(nc) as tc, \
         tc.tile_pool(name="persist", bufs=1) as pp, \
         tc.tile_pool(name="work", bufs=2) as wk, \
         tc.tile_pool(name="pb_s", bufs=2, space="PSUM") as psp, \
         tc.tile_pool(name="pb_o", bufs=2, space="PSUM") as pop, \
         tc.tile_pool(name="pc_y", bufs=2, space="PSUM") as pyp:
        xtall = pp.tile([128, 5, N], BF16, name="xtall", tag="xtall")
        wall = pp.tile([128, WALL_COLS], BF16, name="wall", tag="wall")
        wo = pp.tile([128, 1024], F32R, name="wo", tag="wo")
        ident = pp.tile([128, 128], F32R, name="ident", tag="ident")
        qcat = [pp.tile([128, N], F32R, name=f"qcat{h}", tag=f"qcat{h}")
                for h in range(H_PER_CORE)]
        kcat = [pp.tile([128, N], F32R, name=f"kcat{h}", tag=f"kcat{h}")
                for h in range(H_PER_CORE)]
        # per key tile: 4 heads x (64 V columns | 1 ones column for the
        # softmax denominator), bf16
        vtall = pp.tile([128, NKT, 4, 65], BF16, name="vtall", tag="vtall")
        ebias = pp.tile([128, 1], F32, name="ebias", tag="ebias")
        ocat = [pp.tile([128, N], F32R, name=f"ocat{j}", tag=f"ocat{j}")
                for j in range(2)]

        # Input DMAs. Queue A (sync) carries the critical stream in priority
        # order; queue B (gpsimd) slips the later weights into bus gaps.
        # Big packed transfers amortize the ~650ns per-DMA issue cost.
        def dma_xt_t(t, a, b):
            nc.sync.dma_start(xtall[:, a:b, t * 512:(t + 1) * 512],
                              XTg[:, a:b, t * 512:(t + 1) * 512])

        nc.sync.dma_start(wall[:, 0:1280], Wall[:, 0:1280])        # g0 weights
        dma_xt_t(0, 0, 2)
        dma_xt_t(0, 4, 5)
        dma_xt_t(0, 2, 4)
        nc.sync.dma_start(wall[:, WV:WV + 512], Wall[:, WV:WV + 512])
        dma_xt_t(1, 0, 5)
        nc.sync.dma_start(wall[:, WV + 512:WV + 1024],
                          Wall[:, WV + 512:WV + 1024])
        dma_xt_t(2, 0, 5)
        dma_xt_t(3, 0, 5)
        nc.sync.dma_start(ident[:], Ident[:].bitcast(F32R))
        nc.sync.dma_start(wall[:, 2304:3584], Wall[:, 2304:3584])  # g1
        nc.sync.dma_start(wo[:], Wo[:].bitcast(F32R))

        NPAIR = NT * H_PER_CORE * (NKT // 2)
        ps_tiles = {}
        po_tiles = {}
        qstate = {}
        vstate = {}
        ph_state = {}
        trans_pending = {}

        CP = mybir.ActivationFunctionType.Copy

        def emit_clamps(ps, dest, h0, row0, t, mode):
            # GPSIMD cannot read PSUM, so clamps from PSUM route through:
            # 'prologue' - even-head clamp direct on DVE (shortest chain to
            #   the first dots); odd-head rows staged via an ACT Copy (Copy
            #   shares the Exp table, and ACT idles here) and clamped on
            #   Pool. 'act' - both halves ACT-staged + Pool-clamped (for
            #   blocks landing in ACT stall windows). 'steady' - one DVE
            #   copy + two Pool clamps.
            def clamp(eng, jj, src, srow):
                eng.tensor_scalar(
                    dest[h0 + jj][row0:row0 + 64, t * 512:(t + 1) * 512],
                    src[srow:srow + 64, :], 5.0, -5.0, op0=MIN, op1=MAX)

            if mode == 'prologue':
                clamp(nc.vector, 0, ps, 0)
                scr = wk.tile([64, 512], F32, name="sc6", tag="sc6", bufs=4)
                nc.scalar.activation(scr[:], ps[64:128, :], CP)
                clamp(nc.gpsimd, 1, scr, 0)
            elif mode == 'act':
                scr = wk.tile([128, 512], F32, name="scr", tag="scr", bufs=3)
                nc.scalar.activation(scr[:], ps[:], CP)
                for jj in (0, 1):
                    clamp(nc.gpsimd, jj, scr, jj * 64)
            else:
                scr = wk.tile([128, 512], F32, name="scr", tag="scr", bufs=3)
                nc.vector.tensor_copy(scr[:], ps[:])
                for jj in (0, 1):
                    clamp(nc.gpsimd, jj, scr, jj * 64)

        def kq_piece(t, g, pbase, cbase, dest, p, ps_ap=None, ps2_ap=None,
                     mode='steady'):
            # p 4: coord matmul + clamps (emitted first: its clamp chain is
            # independent of the pixel pieces). p 0..3: pixel-chunk matmuls
            # accumulating over pd; p 3 also clamps. One clamp goes to DVE,
            # the other to Pool, halving the chain latency.
            key = (t, g, id(dest))
            if p < 4:
                if p == 0:
                    qstate[key] = (ps_ap if ps_ap is not None else
                                   pyp.tile([128, 512], F32, name="py", tag="py"))
                ps = qstate[key]
                nc.tensor.matmul(
                    ps[:], wall[:, pbase + p * 128:pbase + (p + 1) * 128],
                    xtall[:, p, t * 512:(t + 1) * 512],
                    start=(p == 0), stop=(p == 3))
                if p == 3:
                    emit_clamps(ps, dest, 2 * g, 0, t, mode)
                    del qstate[key]
            else:
                ps2 = (ps2_ap if ps2_ap is not None else
                       pyp.tile([128, 512], F32, name="py", tag="py"))
                nc.tensor.matmul(ps2[:], wall[:, cbase:cbase + 128],
                                 xtall[:, 4, t * 512:(t + 1) * 512],
                                 start=True, stop=True)
                emit_clamps(ps2, dest, 2 * g, 64, t, mode)

        def kq_all(t, g, kind, mode='steady'):
            for p in (4, 0, 1, 2, 3):
                kq_piece_k(t, g, kind, p, mode)

        def kq_piece_k(t, g, kind, p, mode='steady'):
            if kind == 'k':
                kq_piece(t, g, KP[g], KC[g], kcat, p, mode=mode)
            else:
                kq_piece(t, g, QP[g], QC[g], qcat, p, mode=mode)

        def v_piece(kt, half):
            # half 0: pd chunks 0,1; half 1: pd chunks 2,3 + one fp8 copy
            # moving all four heads' V columns out in a single instruction.
            t, i = divmod(kt, 4)
            if half == 0:
                vstate[kt] = pyp.tile([128, 512], F32, name="py", tag="py")
            pv = vstate[kt]
            for p in (0, 1) if half == 0 else (2, 3):
                nc.tensor.matmul(
                    pv[:, 0:256],
                    xtall[:, p, t * 512 + i * 128:t * 512 + (i + 1) * 128],
                    wall[:, WV + p * 256:WV + (p + 1) * 256],
                    start=(p == 0), stop=(p == NP - 1))
            if half == 1:
                if kt < 0:
                    nc.scalar.activation(vtall[:, kt, :, 0:64], pv[:, 0:256],
                                         CP)
                else:
                    nc.vector.tensor_copy(vtall[:, kt, :, 0:64], pv[:, 0:256])
                del vstate[kt]

        def loc(g):
            # head-pair-major block order: all head-group-0 blocks (for all
            # query tiles) run first, so group-1 projection prep moves out
            # of the overloaded qi=0 window into the mid-kernel surplus.
            m, j = divmod(g, 8)
            hp, r = divmod(m, 8)
            qi, hl = divmod(r, 2)
            return qi, 2 * hp + hl, j

        def emit_dots(g):
            qi, h, j = loc(g)
            ps = psp.tile([128, 2, 512], F32, name="ps", tag="ps")
            ps_tiles[g] = ps
            k0, k1 = 2 * j, 2 * j + 1
            nc.tensor.matmul(
                ps[:, 0, :], kcat[h][:, k0 * 128:(k0 + 1) * 128],
                qcat[h][:, qi * 512:(qi + 1) * 512], start=True, stop=True)
            nc.tensor.matmul(
                ps[:, 1, :], kcat[h][:, k1 * 128:(k1 + 1) * 128],
                qcat[h][:, qi * 512:(qi + 1) * 512], start=True, stop=True)

        def emit_av(g, pe):
            # fp8 DoubleRow: one matmul covers both key tiles of the pair
            # (K=256) at 0.5 cyc/row; a second 1-row matmul accumulates the
            # softmax denominator into po[64].
            # Transposed AV in bf16: the exp scores [keys, 128q] are the
            # full-width stationary, V plus a ones column (denominator) is
            # the 65-wide moving operand. Output [q, 64v|den] accumulates
            # over all 16 key tiles. Four query subtiles share one PSUM
            # bank: since PSUM zero-on-start is bank-granular, the bank is
            # pre-zeroed by a DVE memset (two blocks ahead) and every
            # matmul is a pure accumulate.
            qi, h, j = loc(g)
            po = po_tiles[(qi, h)]
            for i in (0, 1):
                for q in range(4):
                    nc.tensor.matmul(
                        po[:, q, :], pe[:, i, q * 128:(q + 1) * 128],
                        vtall[:, 2 * j + i, h, :],
                        start=False,
                        stop=(j == NKT // 2 - 1 and i == 1),
                        skip_group_check=True)

        def emit_norm(qi, h, g):
            # scale the V columns by 1/den per query row; the transposes
            # back to [d, n] layout are deferred two iterations so the PE
            # queue never blocks on this DVE chain.
            po = po_tiles.pop((qi, h))
            rb = wk.tile([128, 4, 1], F32, name="rb", tag="rb")
            oc_t = wk.tile([128, 4, 64], F32R, name="oct", tag="oct")
            nc.vector.reciprocal(rb[:], po[:, :, 64])
            nc.vector.tensor_tensor(
                oc_t[:, :, :], po[:, :, 0:64],
                rb[:].to_broadcast([128, 4, 64]), op=MULT)

            def flush(qi=qi, h=h, oc_t=oc_t):
                pt = pyp.tile([128, 512], F32, name="py", tag="py")
                for q in range(4):
                    nc.tensor.transpose(
                        pt[0:64, q * 128:(q + 1) * 128].bitcast(F32R),
                        oc_t[:, q, :], ident[:])
                oj, rr = h // 2, (h % 2) * 64
                eng = nc.scalar if (qi, h) == (NT - 1, 3) else None
                if eng is not None:
                    eng.activation(
                        ocat[oj][rr:rr + 64, qi * 512:(qi + 1) * 512],
                        pt[0:64, :].bitcast(F32R), CP)
                else:
                    nc.vector.tensor_copy(
                        ocat[oj][rr:rr + 64, qi * 512:(qi + 1) * 512],
                        pt[0:64, :].bitcast(F32R))
                m = g // 8 + 2
                if m < NT * H_PER_CORE:
                    alloc_po(m)
            trans_pending.setdefault(g + 2, []).append(flush)

        def alloc_po(m):
            # pre-allocate + zero block m's accumulator bank (reuses the
            # buffer its predecessor's norm just released)
            qi_m, h_m, _ = loc(8 * m)
            po = pop.tile([128, 4, 65], F32, name="po", tag="po")
            po_tiles[(qi_m, h_m)] = po
            nc.vector.memset(po[:], 0.0)

        def phasec_piece(qi, i, p):
            # out-projection for a 128-row output chunk, two matmul pieces.
            # For the last query block the ocat[1] half (heads 2,3) finishes
            # first (HPERM3), so it goes in the first matmul.
            n0 = qi * 512 + i * 128
            oa, ob = 0, 1
            if p == 0:
                ph_state[(qi, i)] = pyp.tile([128, 512], F32, name="py", tag="py")
                nc.tensor.matmul(ph_state[(qi, i)][:], ocat[oa][:, n0:n0 + 128],
                                 wo[:, oa * 512:oa * 512 + 512],
                                 start=True, stop=False)
            else:
                py = ph_state.pop((qi, i))
                nc.tensor.matmul(py[:], ocat[ob][:, n0:n0 + 128],
                                 wo[:, ob * 512:ob * 512 + 512],
                                 start=False, stop=True)
                st = wk.tile([128, 512], F32, name="st", tag="st", bufs=4)
                if qi == NT - 1 and i % 2 == 1:
                    # ACT idles after the last exp; let it stage half the
                    # final output chunks so the DMAs start sooner
                    nc.scalar.activation(st[:], py[:], CP)
                else:
                    nc.vector.tensor_copy(st[:], py[:])
                nc.sync.dma_start(Y[n0:n0 + 128, :], st[:])

        # Static filler schedule: iteration -> emitters. PE is in-order, so
        # a block's pieces must all be emitted before the first dots that
        # reads its output (dots(g) is emitted at iter g-2), with ~2 iters
        # of margin for the clamp chain to land.
        sched = {}

        def at(g, *fs):
            sched.setdefault(g, []).extend(fs)

        # qi=0: V tiles (pair 2j,2j+1 before iter j), K for later t-blocks.
        at(0, lambda: v_piece(2, 0), lambda: v_piece(2, 1),
            lambda: v_piece(3, 0), lambda: v_piece(3, 1))
        at(1, lambda: v_piece(4, 0), lambda: v_piece(4, 1),
            lambda: v_piece(5, 0), lambda: v_piece(5, 1),
            lambda: kq_all(2, 0, 'k', 'prologue'))
        at(2, lambda: v_piece(6, 0), lambda: v_piece(6, 1),
            lambda: v_piece(7, 0), lambda: v_piece(7, 1))
        at(3, lambda: v_piece(8, 0), lambda: v_piece(8, 1),
            lambda: v_piece(9, 0), lambda: v_piece(9, 1),
            lambda: kq_all(3, 0, 'k', 'prologue'))
        at(4, lambda: v_piece(10, 0), lambda: v_piece(10, 1),
            lambda: v_piece(11, 0), lambda: v_piece(11, 1))
        at(5, lambda: v_piece(12, 0), lambda: v_piece(12, 1),
            lambda: v_piece(13, 0), lambda: v_piece(13, 1))
        at(6, lambda: v_piece(14, 0), lambda: v_piece(14, 1),
            lambda: v_piece(15, 0), lambda: v_piece(15, 1))
        # group-0 Q projections feed blocks (qi, h0/h1) at iters 16*qi
        def q_sched(base, t, g):
            for p, off in ((4, 0), (0, 1), (1, 2), (2, 3), (3, 4)):
                at(base + off, lambda t=t, p=p, g=g: kq_piece_k(t, g, 'q', p))

        q_sched(7, 1, 0)
        q_sched(18, 2, 0)
        q_sched(24, 3, 0)
        # group-1 K/Q projections: deadlines start at iter 62 (block q0,h2)
        KQ1 = [(33, 0, 'k'), (36, 1, 'k'), (39, 2, 'k'), (42, 3, 'k'),
               (45, 0, 'q')]
        for base, t, kind in KQ1:
            at(base, lambda t=t, kind=kind: kq_piece_k(t, 1, kind, 4),
               lambda t=t, kind=kind: kq_piece_k(t, 1, kind, 0))
            at(base + 1, lambda t=t, kind=kind: kq_piece_k(t, 1, kind, 1),
               lambda t=t, kind=kind: kq_piece_k(t, 1, kind, 2))
            at(base + 2, lambda t=t, kind=kind: kq_piece_k(t, 1, kind, 3))
        q_sched(55, 1, 1)
        q_sched(60, 2, 1)
        q_sched(65, 3, 1)
        # out-projection: query block qi complete after its (qi,h3) flush
        for qi in range(NT - 1):
            base = 84 + 16 * qi
            for i in range(4):
                at(base + 3 * i, lambda qi=qi, i=i: phasec_piece(qi, i, 0))
                at(base + 1 + 3 * i, lambda qi=qi, i=i: phasec_piece(qi, i, 1))

        # Prologue. The dummy exp preloads the ACT table during DMA wait.
        # The Q(0,0) block borrows the first dots PSUM buffer so all four
        # projection blocks run without pool-rotation WAR stalls; pieces are
        # ordered so the ones needing only the first half of the t0 DMA
        # start earliest.
        nc.vector.memset(vtall[:, :, :, 64], 1.0)   # denominator ones cols
        nc.vector.memset(ebias[:], EXP_BIAS)
        warm = wk.tile([128, 1], F32, name="warm", tag="warm")
        nc.scalar.activation(warm[:], ebias[:], EXP)
        # Q staging packs pixel/coord into the two banks of ONE dots-pool
        # tile, so the second tile is free for dots(0) with no
        # write-after-read stall on the Q clamps.
        qtile = psp.tile([128, 2, 512], F32, name="ps", tag="ps")
        # PE warm-up: dummy bf16 matmuls on a zeroed tile keep the PE busy
        # through the input-DMA wait so the p-state ramp (3us of continuous
        # execution) completes before the real projections start.
        pe_w = wk.tile([128, 2, 512], BF16, name="pe", tag="pe", bufs=12)
        nc.gpsimd.memset(pe_w[:], 0.0)
        for _ in range(12):
            nc.tensor.matmul(qtile[:, 0, 0:256], pe_w[:, 0, 0:128],
                             pe_w[:, 1, 0:256], start=True, stop=True)
        for p in (0, 1):
            kq_piece(0, 0, KP[0], KC[0], kcat, p, mode='prologue')
            kq_piece(0, 0, QP[0], QC[0], qcat, p, ps_ap=qtile[:, 0, :],
                     mode='prologue')
        kq_piece(0, 0, KP[0], KC[0], kcat, 4, mode='prologue')
        kq_piece(0, 0, QP[0], QC[0], qcat, 4, ps2_ap=qtile[:, 1, :],
                 mode='prologue')
        for p in (2, 3):
            kq_piece(0, 0, KP[0], KC[0], kcat, p, mode='prologue')
            kq_piece(0, 0, QP[0], QC[0], qcat, p, ps_ap=qtile[:, 0, :],
                     mode='prologue')
        emit_dots(0)
        emit_dots(1)
        kq_all(1, 0, 'k', mode='prologue')
        v_piece(0, 0)
        v_piece(0, 1)
        v_piece(1, 0)
        v_piece(1, 1)
        alloc_po(0)
        alloc_po(1)

        for g in range(NPAIR):
            qi, h, j = loc(g)
            pe = wk.tile([128, 2, 512], BF16, name="pe", tag="pe", bufs=12)
            nc.scalar.activation(pe[:], ps_tiles.pop(g)[:], EXP, scale=SCALE)
            if g + 2 < NPAIR:
                emit_dots(g + 2)
            emit_av(g, pe)
            if j == NKT // 2 - 1:
                emit_norm(qi, h, g)
            for f in trans_pending.pop(g, ()):
                f()
            for f in sched.get(g, ()):
                f()
        for fs in [trans_pending.pop(k) for k in sorted(trans_pending)]:
            for f in fs:
                f()
        for i in range(4):
            phasec_piece(NT - 1, i, 0)
            phasec_piece(NT - 1, i, 1)
    nc.compile()
    return nc


def _get_nc():
    global _NC
    if _NC is None:
        _NC = _build()
    return _NC


def _bf16(a):
    return np.ascontiguousarray(np.asarray(a, np.float32).astype(ml_dtypes.bfloat16))


def _pack_gcd(w):
    # [512, 256] -> [128, (g, c, d)]: head-group-major weight layout so the
    # first group's columns are one contiguous DMA.
    return (np.asarray(w, np.float32)
            .reshape(4, 128, 2, 128).transpose(1, 2, 0, 3).reshape(128, 1024))


def _pack_chd(w):
    # [512, 256] -> [128, (c, h, d)]: pd-chunk-major layout for the V weights.
    return (np.asarray(w, np.float32)
            .reshape(4, 128, 256).transpose(1, 0, 2).reshape(128, 1024))


def _pack_wo(w):
    w = np.asarray(w, dtype=np.float32)
    return np.ascontiguousarray(
        w.reshape(2, 128, 512).transpose(1, 0, 2).reshape(128, 1024))


def kernel(pixels, coords, mask, W_qkv, W_qkc, W_out, b_out):
    global LAST_EXEC_NS
    pixels = np.asarray(pixels, dtype=np.float32)
    coords = np.asarray(coords, dtype=np.float32)
    W_qkv = np.asarray(W_qkv, dtype=np.float32)
    W_qkc = np.asarray(W_qkc, dtype=np.float32)
    W_out = np.asarray(W_out, dtype=np.float32)
    b_out = np.asarray(b_out, dtype=np.float32)

    nc = _get_nc()

    XTg = [np.ascontiguousarray(
        np.concatenate([pixels[b].T.reshape(4, 128, N),
                        coords[b].T.reshape(1, 128, N)], axis=0)
        .transpose(1, 0, 2).astype(ml_dtypes.bfloat16)) for b in range(B)]

    in_maps = []
    for c in range(8):
        b = c // 2
        h0 = (c % 2) * H_PER_CORE * DH     # 0 or 256: col offset within split
        kp = _pack_gcd(W_qkv[:, ID + h0:ID + h0 + 256])
        qp = _pack_gcd(W_qkv[:, h0:h0 + 256])
        v = _pack_chd(W_qkv[:, 2 * ID + h0:2 * ID + h0 + 256])
        kc = np.asarray(W_qkc[:, ID + h0:ID + h0 + 256], np.float32)
        qc = np.asarray(W_qkc[:, h0:h0 + 256], np.float32)
        wall = np.concatenate([
            kp[:, 0:512], kc[:, 0:128], qp[:, 0:512], qc[:, 0:128], v,
            kp[:, 512:1024], kc[:, 128:256], qp[:, 512:1024], qc[:, 128:256],
        ], axis=1)
        in_maps.append({
            "XTg": XTg[b],
            "Wall": _bf16(wall),
            "Wo": _pack_wo(W_out[h0:h0 + 256, :]),
            "Ident": np.eye(128, dtype=np.float32),
        })

    res = run_bass_kernel_spmd(nc, in_maps, core_ids=list(range(8)))
    LAST_EXEC_NS = getattr(res, "exec_time_ns", None)

    out = np.empty((B, N, OUT_D), np.float32)
    for b in range(B):
        out[b] = res.results[2 * b]["Y"] + res.results[2 * b + 1]["Y"]
    out += b_out[None, None, :]
    return tuple(np.split(out, [1024], axis=1))


# revision 77
# speedup vs baseline: 1.2893x; 1.0031x over previous
import numpy as np
import ml_dtypes

import concourse.bacc as bacc
import concourse.tile as tile
import concourse.mybir as mybir
from concourse.bass_utils import run_bass_kernel_spmd

F32 = mybir.dt.float32
F32R = mybir.dt.float32r
BF16 = mybir.dt.bfloat16
F8E4 = mybir.dt.float8e4
F8E5 = mybir.dt.float8e5

B = 4
N = 2048
PD = 512
CD = 128
ID = 512
OUT_D = 512
H_PER_CORE = 4
DH = 64
SCALE = 0.125          # dim_head ** -0.5
NT = 4                 # n chunks of 512
NP = 4                 # pd chunks of 128
NKT = 16               # key tiles of 128
EXP_BIAS = -2.0        # softmax logits shifted so exp() fits fp8e5 range

# column offsets inside the packed weight block (bf16), head-group-major:
# [kp_g0 512][kc_g0 128][qp_g0 512][qc_g0 128][v 1024]
# [kp_g1 512][kc_g1 128][qp_g1 512][qc_g1 128]
KP = {0: 0, 1: 2304}
KC = {0: 512, 1: 2816}
QP = {0: 640, 1: 2944}
QC = {0: 1152, 1: 3456}
WV = 1280
WALL_COLS = 3584

_NC = None
LAST_EXEC_NS = None


def _build():
    nc = bacc.Bacc("TRN2", target_bir_lowering=False, debug=False, num_devices=8)
    XTg = nc.declare_dram_parameter("XTg", [128, 5, N], BF16, isOutput=False)
    Wall = nc.declare_dram_parameter("Wall", [128, WALL_COLS], BF16, isOutput=False)
    Wo = nc.declare_dram_parameter("Wo", [128, 1024], F32, isOutput=False)
    Ident = nc.declare_dram_parameter("Ident", [128, 128], F32, isOutput=False)
    Y = nc.declare_dram_parameter("Y", [N, OUT_D], F32, isOutput=True)

    MIN = mybir.AluOpType.min
    MAX = mybir.AluOpType.max
    MULT = mybir.AluOpType.mult
    EXP = mybir.ActivationFunctionType.Exp
    DR = mybir.MatmulPerfMode.DoubleRow

    with tile.TileContext(nc) as tc, \
         tc.tile_pool(name="persist", bufs=1) as pp, \
         tc.tile_pool(name="work", bufs=2) as wk, \
         tc.tile_pool(name="pb_s", bufs=2, space="PSUM") as psp, \
         tc.tile_pool(name="pb_o", bufs=2, space="PSUM") as pop, \
         tc.tile_pool(name="pc_y", bufs=2, space="PSUM") as pyp:
        xtall = pp.tile([128, 5, N], BF16, name="xtall", tag="xtall")
        wall = pp.tile([128, WALL_COLS], BF16, name="wall", tag="wall")
        wo = pp.tile([128, 1024], F32R, name="wo", tag="wo")
        ident = pp.tile([128, 128], F32R, name="ident", tag="ident")
        qcat = [pp.tile([128, N], F32R, name=f"qcat{h}", tag=f"qcat{h}")
                for h in range(H_PER_CORE)]
        kcat = [pp.tile([128, N], F32R, name=f"kcat{h}", tag=f"kcat{h}")
                for h in range(H_PER_CORE)]
        # per key tile: 4 heads x (64 V columns | 1 ones column for the
        # softmax denominator), bf16
        vtall = pp.tile([128, NKT, 4, 65], BF16, name="vtall", tag="vtall")
        ebias = pp.tile([128, 1], F32, name="ebias", tag="ebias")
        ocat = [pp.tile([128, N], F32R, name=f"ocat{j}", tag=f"ocat{j}")
                for j in range(2)]

        # Input DMAs. Queue A (sync) carries the critical stream in priority
        # order; queue B (gpsimd) slips the later weights into bus gaps.
        # Big packed transfers amortize the ~650ns per-DMA issue cost.
        def dma_xt_t(t, a, b):
            nc.sync.dma_start(xtall[:, a:b, t * 512:(t + 1) * 512],
                              XTg[:, a:b, t * 512:(t + 1) * 512])

        nc.sync.dma_start(wall[:, 0:1280], Wall[:, 0:1280])        # g0 weights
        dma_xt_t(0, 0, 2)
        dma_xt_t(0, 4, 5)
        dma_xt_t(0, 2, 4)
        dma_xt_t(1, 0, 5)
        nc.sync.dma_start(wall[:, WV:WV + 512], Wall[:, WV:WV + 512])
        dma_xt_t(2, 0, 5)
        nc.sync.dma_start(wall[:, WV + 512:WV + 1024],
                          Wall[:, WV + 512:WV + 1024])
        dma_xt_t(3, 0, 5)
        nc.sync.dma_start(ident[:], Ident[:].bitcast(F32R))
        nc.sync.dma_start(wall[:, 2304:3584], Wall[:, 2304:3584])  # g1
        nc.sync.dma_start(wo[:], Wo[:].bitcast(F32R))

        NPAIR = NT * H_PER_CORE * (NKT // 2)
        ps_tiles = {}
        po_tiles = {}
        qstate = {}
        vstate = {}
        ph_state = {}
        trans_pending = {}

        CP = mybir.ActivationFunctionType.Copy

        def emit_clamps(ps, dest, h0, row0, t, mode):
            # GPSIMD cannot read PSUM, so clamps from PSUM route through:
            # 'prologue' - even-head clamp direct on DVE (shortest chain to
            #   the first dots); odd-head rows staged via an ACT Copy (Copy
            #   shares the Exp table, and ACT idles here) and clamped on
            #   Pool. 'act' - both halves ACT-staged + Pool-clamped (for
            #   blocks landing in ACT stall windows). 'steady' - one DVE
            #   copy + two Pool clamps.
            def clamp(eng, jj, src, srow):
                eng.tensor_scalar(
                    dest[h0 + jj][row0:row0 + 64, t * 512:(t + 1) * 512],
                    src[srow:srow + 64, :], 5.0, -5.0, op0=MIN, op1=MAX)

            if mode == 'prologue':
                clamp(nc.vector, 0, ps, 0)
                scr = wk.tile([64, 512], F32, name="sc6", tag="sc6", bufs=4)
                nc.scalar.activation(scr[:], ps[64:128, :], CP)
                clamp(nc.gpsimd, 1, scr, 0)
            elif mode == 'act':
                scr = wk.tile([128, 512], F32, name="scr", tag="scr", bufs=3)
                nc.scalar.activation(scr[:], ps[:], CP)
                for jj in (0, 1):
                    clamp(nc.gpsimd, jj, scr, jj * 64)
            else:
                scr = wk.tile([128, 512], F32, name="scr", tag="scr", bufs=3)
                nc.vector.tensor_copy(scr[:], ps[:])
                for jj in (0, 1):
                    clamp(nc.gpsimd, jj, scr, jj * 64)

        def kq_piece(t, g, pbase, cbase, dest, p, ps_ap=None, ps2_ap=None,
                     mode='steady'):
            # p 4: coord matmul + clamps (emitted first: its clamp chain is
            # independent of the pixel pieces). p 0..3: pixel-chunk matmuls
            # accumulating over pd; p 3 also clamps. One clamp goes to DVE,
            # the other to Pool, halving the chain latency.
            key = (t, g, id(dest))
            if p < 4:
                if p == 0:
                    qstate[key] = (ps_ap if ps_ap is not None else
                                   pyp.tile([128, 512], F32, name="py", tag="py"))
                ps = qstate[key]
                nc.tensor.matmul(
                    ps[:], wall[:, pbase + p * 128:pbase + (p + 1) * 128],
                    xtall[:, p, t * 512:(t + 1) * 512],
                    start=(p == 0), stop=(p == 3))
                if p == 3:
                    emit_clamps(ps, dest, 2 * g, 0, t, mode)
                    del qstate[key]
            else:
                ps2 = (ps2_ap if ps2_ap is not None else
                       pyp.tile([128, 512], F32, name="py", tag="py"))
                nc.tensor.matmul(ps2[:], wall[:, cbase:cbase + 128],
                                 xtall[:, 4, t * 512:(t + 1) * 512],
                                 start=True, stop=True)
                emit_clamps(ps2, dest, 2 * g, 64, t, mode)

        def kq_all(t, g, kind, mode='steady'):
            for p in (4, 0, 1, 2, 3):
                kq_piece_k(t, g, kind, p, mode)

        def kq_piece_k(t, g, kind, p, mode='steady'):
            if kind == 'k':
                kq_piece(t, g, KP[g], KC[g], kcat, p, mode=mode)
            else:
                kq_piece(t, g, QP[g], QC[g], qcat, p, mode=mode)

        def v_piece(kt, half):
            # half 0: pd chunks 0,1; half 1: pd chunks 2,3 + one fp8 copy
            # moving all four heads' V columns out in a single instruction.
            t, i = divmod(kt, 4)
            if half == 0:
                vstate[kt] = pyp.tile([128, 512], F32, name="py", tag="py")
            pv = vstate[kt]
            for p in (0, 1) if half == 0 else (2, 3):
                nc.tensor.matmul(
                    pv[:, 0:256],
                    xtall[:, p, t * 512 + i * 128:t * 512 + (i + 1) * 128],
                    wall[:, WV + p * 256:WV + (p + 1) * 256],
                    start=(p == 0), stop=(p == NP - 1))
            if half == 1:
                if kt < 0:
                    nc.scalar.activation(vtall[:, kt, :, 0:64], pv[:, 0:256],
                                         CP)
                else:
                    nc.vector.tensor_copy(vtall[:, kt, :, 0:64], pv[:, 0:256])
                del vstate[kt]

        def loc(g):
            # head-pair-major block order: all head-group-0 blocks (for all
            # query tiles) run first, so group-1 projection prep moves out
            # of the overloaded qi=0 window into the mid-kernel surplus.
            m, j = divmod(g, 8)
            hp, r = divmod(m, 8)
            qi, hl = divmod(r, 2)
            return qi, 2 * hp + hl, j

        def emit_dots(g):
            qi, h, j = loc(g)
            ps = psp.tile([128, 2, 512], F32, name="ps", tag="ps")
            ps_tiles[g] = ps
            k0, k1 = 2 * j, 2 * j + 1
            nc.tensor.matmul(
                ps[:, 0, :], kcat[h][:, k0 * 128:(k0 + 1) * 128],
                qcat[h][:, qi * 512:(qi + 1) * 512], start=True, stop=True)
            nc.tensor.matmul(
                ps[:, 1, :], kcat[h][:, k1 * 128:(k1 + 1) * 128],
                qcat[h][:, qi * 512:(qi + 1) * 512], start=True, stop=True)

        def emit_av(g, pe):
            # fp8 DoubleRow: one matmul covers both key tiles of the pair
            # (K=256) at 0.5 cyc/row; a second 1-row matmul accumulates the
            # softmax denominator into po[64].
            # Transposed AV in bf16: the exp scores [keys, 128q] are the
            # full-width stationary, V plus a ones column (denominator) is
            # the 65-wide moving operand. Output [q, 64v|den] accumulates
            # over all 16 key tiles. Four query subtiles share one PSUM
            # bank: since PSUM zero-on-start is bank-granular, the bank is
            # pre-zeroed by a DVE memset (two blocks ahead) and every
            # matmul is a pure accumulate.
            qi, h, j = loc(g)
            po = po_tiles[(qi, h)]
            for i in (0, 1):
                for q in range(4):
                    nc.tensor.matmul(
                        po[:, q, :], pe[:, i, q * 128:(q + 1) * 128],
                        vtall[:, 2 * j + i, h, :],
                        start=False,
                        stop=(j == NKT // 2 - 1 and i == 1),
                        skip_group_check=True)

        def emit_norm(qi, h, g):
            # scale the V columns by 1/den per query row; the transposes
            # back to [d, n] layout are deferred two iterations so the PE
            # queue never blocks on this DVE chain.
            po = po_tiles.pop((qi, h))
            rb = wk.tile([128, 4, 1], F32, name="rb", tag="rb")
            oc_t = wk.tile([128, 4, 64], F32R, name="oct", tag="oct")
            nc.vector.reciprocal(rb[:], po[:, :, 64])
            nc.vector.tensor_tensor(
                oc_t[:, :, :], po[:, :, 0:64],
                rb[:].to_broadcast([128, 4, 64]), op=MULT)

            def flush(qi=qi, h=h, oc_t=oc_t):
                pt = pyp.tile([128, 512], F32, name="py", tag="py")
                for q in range(4):
                    nc.tensor.transpose(
                        pt[0:64, q * 128:(q + 1) * 128].bitcast(F32R),
                        oc_t[:, q, :], ident[:])
                oj, rr = h // 2, (h % 2) * 64
                eng = nc.scalar if (qi, h) == (NT - 1, 3) else None
                if eng is not None:
                    eng.activation(
                        ocat[oj][rr:rr + 64, qi * 512:(qi + 1) * 512],
                        pt[0:64, :].bitcast(F32R), CP)
                else:
                    nc.vector.tensor_copy(
                        ocat[oj][rr:rr + 64, qi * 512:(qi + 1) * 512],
                        pt[0:64, :].bitcast(F32R))
                m = g // 8 + 2
                if m < NT * H_PER_CORE:
                    alloc_po(m)
            trans_pending.setdefault(g + 2, []).append(flush)

        def alloc_po(m):
            # pre-allocate + zero block m's accumulator bank (reuses the
            # buffer its predecessor's norm just released)
            qi_m, h_m, _ = loc(8 * m)
            po = pop.tile([128, 4, 65], F32, name="po", tag="po")
            po_tiles[(qi_m, h_m)] = po
            nc.vector.memset(po[:], 0.0)

        def phasec_piece(qi, i, p):
            # out-projection for a 128-row output chunk, two matmul pieces.
            # For the last query block the ocat[1] half (heads 2,3) finishes
            # first (HPERM3), so it goes in the first matmul.
            n0 = qi * 512 + i * 128
            oa, ob = 0, 1
            if p == 0:
                ph_state[(qi, i)] = pyp.tile([128, 512], F32, name="py", tag="py")
                nc.tensor.matmul(ph_state[(qi, i)][:], ocat[oa][:, n0:n0 + 128],
                                 wo[:, oa * 512:oa * 512 + 512],
                                 start=True, stop=False)
            else:
                py = ph_state.pop((qi, i))
                nc.tensor.matmul(py[:], ocat[ob][:, n0:n0 + 128],
                                 wo[:, ob * 512:ob * 512 + 512],
                                 start=False, stop=True)
                st = wk.tile([128, 512], F32, name="st", tag="st", bufs=4)
                if qi == NT - 1 and i % 2 == 1:
                    # ACT idles after the last exp; let it stage half the
                    # final output chunks so the DMAs start sooner
                    nc.scalar.activation(st[:], py[:], CP)
                else:
                    nc.vector.tensor_copy(st[:], py[:])
                nc.sync.dma_start(Y[n0:n0 + 128, :], st[:])

        # Static filler schedule: iteration -> emitters. PE is in-order, so
        # a block's pieces must all be emitted before the first dots that
        # reads its output (dots(g) is emitted at iter g-2), with ~2 iters
        # of margin for the clamp chain to land.
        sched = {}

        def at(g, *fs):
            sched.setdefault(g, []).extend(fs)

        # qi=0: V tiles (pair 2j,2j+1 before iter j), K for later t-blocks.
        at(0, lambda: v_piece(2, 0), lambda: v_piece(2, 1),
            lambda: v_piece(3, 0), lambda: v_piece(3, 1))
        at(1, lambda: v_piece(4, 0), lambda: v_piece(4, 1),
            lambda: v_piece(5, 0), lambda: v_piece(5, 1),
            lambda: kq_all(2, 0, 'k', 'prologue'))
        at(2, lambda: v_piece(6, 0), lambda: v_piece(6, 1),
            lambda: v_piece(7, 0), lambda: v_piece(7, 1))
        at(3, lambda: v_piece(8, 0), lambda: v_piece(8, 1),
            lambda: v_piece(9, 0), lambda: v_piece(9, 1),
            lambda: kq_all(3, 0, 'k', 'prologue'))
        at(4, lambda: v_piece(10, 0), lambda: v_piece(10, 1),
            lambda: v_piece(11, 0), lambda: v_piece(11, 1))
        at(5, lambda: v_piece(12, 0), lambda: v_piece(12, 1),
            lambda: v_piece(13, 0), lambda: v_piece(13, 1))
        at(6, lambda: v_piece(14, 0), lambda: v_piece(14, 1),
            lambda: v_piece(15, 0), lambda: v_piece(15, 1))
        # group-0 Q projections feed blocks (qi, h0/h1) at iters 16*qi
        def q_sched(base, t, g):
            for p, off in ((4, 0), (0, 1), (1, 2), (2, 3), (3, 4)):
                at(base + off, lambda t=t, p=p, g=g: kq_piece_k(t, g, 'q', p))

        q_sched(7, 1, 0)
        q_sched(18, 2, 0)
        q_sched(24, 3, 0)
        # group-1 K/Q projections: deadlines start at iter 62 (block q0,h2)
        KQ1 = [(33, 0, 'k'), (36, 1, 'k'), (39, 2, 'k'), (42, 3, 'k'),
               (45, 0, 'q')]
        for base, t, kind in KQ1:
            at(base, lambda t=t, kind=kind: kq_piece_k(t, 1, kind, 4),
               lambda t=t, kind=kind: kq_piece_k(t, 1, kind, 0))
            at(base + 1, lambda t=t, kind=kind: kq_piece_k(t, 1, kind, 1),
               lambda t=t, kind=kind: kq_piece_k(t, 1, kind, 2))
            at(base + 2, lambda t=t, kind=kind: kq_piece_k(t, 1, kind, 3))
        q_sched(55, 1, 1)
        q_sched(60, 2, 1)
        q_sched(65, 3, 1)
        # out-projection: query block qi complete after its (qi,h3) flush
        for qi in range(NT - 1):
            base = 84 + 16 * qi
            for i in range(4):
                at(base + 3 * i, lambda qi=qi, i=i: phasec_piece(qi, i, 0))
                at(base + 1 + 3 * i, lambda qi=qi, i=i: phasec_piece(qi, i, 1))

        # Prologue. The dummy exp preloads the ACT table during DMA wait.
        # The Q(0,0) block borrows the first dots PSUM buffer so all four
        # projection blocks run without pool-rotation WAR stalls; pieces are
        # ordered so the ones needing only the first half of the t0 DMA
        # start earliest.
        nc.vector.memset(vtall[:, :, :, 64], 1.0)   # denominator ones cols
        nc.vector.memset(ebias[:], EXP_BIAS)
        warm = wk.tile([128, 1], F32, name="warm", tag="warm")
        nc.scalar.activation(warm[:], ebias[:], EXP)
        # Q staging packs pixel/coord into the two banks of ONE dots-pool
        # tile, so the second tile is free for dots(0) with no
        # write-after-read stall on the Q clamps.
        qtile = psp.tile([128, 2, 512], F32, name="ps", tag="ps")
        # PE warm-up: dummy bf16 matmuls on a zeroed tile keep the PE busy
        # through the input-DMA wait so the p-state ramp (3us of continuous
        # execution) completes before the real projections start.
        pe_w = wk.tile([128, 2, 512], BF16, name="pe", tag="pe", bufs=12)
        nc.gpsimd.memset(pe_w[:], 0.0)
        for _ in range(12):
            nc.tensor.matmul(qtile[:, 0, 0:256], pe_w[:, 0, 0:128],
                             pe_w[:, 1, 0:256], start=True, stop=True)
        for p in (0, 1):
            kq_piece(0, 0, KP[0], KC[0], kcat, p, mode='prologue')
            kq_piece(0, 0, QP[0], QC[0], qcat, p, ps_ap=qtile[:, 0, :],
                     mode='prologue')
        kq_piece(0, 0, KP[0], KC[0], kcat, 4, mode='prologue')
        kq_piece(0, 0, QP[0], QC[0], qcat, 4, ps2_ap=qtile[:, 1, :],
                 mode='prologue')
        for p in (2, 3):
            kq_piece(0, 0, KP[0], KC[0], kcat, p, mode='prologue')
            kq_piece(0, 0, QP[0], QC[0], qcat, p, ps_ap=qtile[:, 0, :],
                     mode='prologue')
        emit_dots(0)
        emit_dots(1)
        kq_all(1, 0, 'k', mode='prologue')
        v_piece(0, 0)
        v_piece(0, 1)
        v_piece(1, 0)
        v_piece(1, 1)
        alloc_po(0)
        alloc_po(1)

        for g in range(NPAIR):
            qi, h, j = loc(g)
            pe = wk.tile([128, 2, 512], BF16, name="pe", tag="pe", bufs=12)
            nc.scalar.activation(pe[:], ps_tiles.pop(g)[:], EXP, scale=SCALE)
            if g + 2 < NPAIR:
                emit_dots(g + 2)
            emit_av(g, pe)
            if j == NKT // 2 - 1:
                emit_norm(qi, h, g)
            for f in trans_pending.pop(g, ()):
                f()
            for f in sched.get(g, ()):
                f()
        for fs in [trans_pending.pop(k) for k in sorted(trans_pending)]:
            for f in fs:
                f()
        for i in range(4):
            phasec_piece(NT - 1, i, 0)
            phasec_piece(NT - 1, i, 1)
    nc.compile()
    return nc


def _get_nc():
    global _NC
    if _NC is None:
        _NC = _build()
    return _NC


def _bf16(a):
    return np.ascontiguousarray(np.asarray(a, np.float32).astype(ml_dtypes.bfloat16))


def _pack_gcd(w):
    # [512, 256] -> [128, (g, c, d)]: head-group-major weight layout so the
    # first group's columns are one contiguous DMA.
    return (np.asarray(w, np.float32)
            .reshape(4, 128, 2, 128).transpose(1, 2, 0, 3).reshape(128, 1024))


def _pack_chd(w):
    # [512, 256] -> [128, (c, h, d)]: pd-chunk-major layout for the V weights.
    return (np.asarray(w, np.float32)
            .reshape(4, 128, 256).transpose(1, 0, 2).reshape(128, 1024))


def _pack_wo(w):
    w = np.asarray(w, dtype=np.float32)
    return np.ascontiguousarray(
        w.reshape(2, 128, 512).transpose(1, 0, 2).reshape(128, 1024))


def kernel(pixels, coords, mask, W_qkv, W_qkc, W_out, b_out):
    global LAST_EXEC_NS
    pixels = np.asarray(pixels, dtype=np.float32)
    coords = np.asarray(coords, dtype=np.float32)
    W_qkv = np.asarray(W_qkv, dtype=np.float32)
    W_qkc = np.asarray(W_qkc, dtype=np.float32)
    W_out = np.asarray(W_out, dtype=np.float32)
    b_out = np.asarray(b_out, dtype=np.float32)

    nc = _get_nc()

    XTg = [np.ascontiguousarray(
        np.concatenate([pixels[b].T.reshape(4, 128, N),
                        coords[b].T.reshape(1, 128, N)], axis=0)
        .transpose(1, 0, 2).astype(ml_dtypes.bfloat16)) for b in range(B)]

    in_maps = []
    for c in range(8):
        b = c // 2
        h0 = (c % 2) * H_PER_CORE * DH     # 0 or 256: col offset within split
        kp = _pack_gcd(W_qkv[:, ID + h0:ID + h0 + 256])
        qp = _pack_gcd(W_qkv[:, h0:h0 + 256])
        v = _pack_chd(W_qkv[:, 2 * ID + h0:2 * ID + h0 + 256])
        kc = np.asarray(W_qkc[:, ID + h0:ID + h0 + 256], np.float32)
        qc = np.asarray(W_qkc[:, h0:h0 + 256], np.float32)
        wall = np.concatenate([
            kp[:, 0:512], kc[:, 0:128], qp[:, 0:512], qc[:, 0:128], v,
            kp[:, 512:1024], kc[:, 128:256], qp[:, 512:1024], qc[:, 128:256],
        ], axis=1)
        in_maps.append({
            "XTg": XTg[b],
            "Wall": _bf16(wall),
            "Wo": _pack_wo(W_out[h0:h0 + 256, :]),
            "Ident": np.eye(128, dtype=np.float32),
        })

    res = run_bass_kernel_spmd(nc, in_maps, core_ids=list(range(8)))
    LAST_EXEC_NS = getattr(res, "exec_time_ns", None)

    out = np.empty((B, N, OUT_D), np.float32)
    for b in range(B):
        out[b] = res.results[2 * b]["Y"] + res.results[2 * b + 1]["Y"]
    out += b_out[None, None, :]
    return tuple(np.split(out, [1024], axis=1))
